# revision 10
# baseline (speedup 1.0000x reference)
"""Trainium2 Bass kernel for nn_BlockNet (GNN message passing + block-sim MLP).

Strategy (8 NeuronCores, SPMD, single NEFF):
  - GCN aggregation sharded by destination-node tile ranges (tpc x 128-node
    tiles per core).  Edges sorted by dst on host; segment-sum done as one-hot
    matmuls accumulating in PSUM per dst block.  Self loops kept as edges.
  - conv1 needs no device gather: host stages edge-ordered rows
    xe = (x * dinv)[src]; kernel computes (A @ xe) @ W1 per dst block.
  - conv2 messages fetched with per-dst-block dma_gather ops (2304 idx each,
    256B rows) from the Shared AllGather'd node table; SWDGE queues
    round-robin, deep-buffered so descriptor-gen pipelines with transfers.
  - Degree normalization folded into scalar-engine epilogues (biases are
    zero in this problem -- verified on host, generic fallback otherwise).
  - (x @ emb_sim) @ sim_block @ w_sim collapsed to x @ M3 with
    M3 = emb_sim @ sim_block @ w_sim; the BxB block-sim math is replicated
    per core in bf16.
  - Two Shared-output AllGathers exchange (1) the conv1 row table and
    (2) the final 40-wide node feature table.
  - Query phase: data-parallel over query edges; ONE transposed dma_gather
    per 2048 queries fetches both endpoints (4096 idx); MLP runs on
    512-query slices with biases folded into scalar activations.

kernel(**inputs) takes full unsharded inputs, returns the full [NQ] f32
output.
"""

import math
import os
import sys

import numpy as np

for _p in ("/opt/trn_rl_repo", "/root/.axon_site/_ro/trn_rl_repo"):
    if os.path.isdir(_p) and _p not in sys.path:
        sys.path.insert(0, _p)

import concourse.bass as bass
import concourse.bacc as bacc
import concourse.mybir as mybir
import concourse.tile as tile
from concourse import bass_utils
from concourse.masks import make_identity

BF16 = mybir.dt.bfloat16
F32 = mybir.dt.float32
I16 = mybir.dt.int16
I32 = mybir.dt.int32
NP_BF16 = mybir.dt.np(BF16)

P = 128
NCORES = 8
LEAKY_SLOPE = 0.2
ALPHA, BETA = 1.0, 0.1
CLAMP_MAX = 40.0

AF = mybir.ActivationFunctionType
OP = mybir.AluOpType

NQUEUES = 4       # SWDGE queues; gathers round-robin
# SWDGE descriptor-ring capacity is dynamic_dma_scratch_size/16 per queue;
# a single dma_gather's num_idxs must stay below it.
DMA_SCRATCH = int(os.environ.get("DMA_SCRATCH", "16384"))
RING = DMA_SCRATCH // 16
# queries per combined (i0|i1) transposed gather op (2*QOP idx per op)
QOP = int(os.environ.get("QOP", "256"))
GSZ = RING // P   # conv2 message chunks per gather op


def _bc(ap, n):
    """Append a stride-0 broadcast inner dim of size n to an AP."""
    return bass.AP(ap.tensor, ap.offset, list(ap.ap) + [[0, n]])


# ----------------------------------------------------------------------------
# host-side data prep
# ----------------------------------------------------------------------------

def _wrap16(idx):
    """int16 index array in dma_gather wrapped layout [128, n/16]."""
    idx = np.asarray(idx, np.int64)
    n = idx.shape[0]
    assert n % 16 == 0
    w = idx.reshape(n // 16, 16).T.astype(np.int16)        # [16, n/16]
    return np.ascontiguousarray(np.tile(w, (8, 1)))         # [128, n/16]


def build_host(inputs, ncores=NCORES):
    x = np.asarray(inputs["x"], np.float32)
    L0 = np.asarray(inputs["L0"], np.float32)
    L1 = np.asarray(inputs["L1"], np.float32)
    ei = np.asarray(inputs["edge_index"]).astype(np.int64)
    te = np.asarray(inputs["total_edges"]).astype(np.int64)
    W1 = np.asarray(inputs["conv1_W"], np.float32)
    b1 = np.asarray(inputs["conv1_b"], np.float32)
    W2 = np.asarray(inputs["conv2_W"], np.float32)
    b2 = np.asarray(inputs["conv2_b"], np.float32)
    w_sim = np.asarray(inputs["weights_sim"], np.float32)
    emb_sim = np.asarray(inputs["embeddings_sim"], np.float32)
    w_od = np.asarray(inputs["weights_off_diagonal"], np.float32)
    wL0 = np.asarray(inputs["weights_L_0"], np.float32)
    wL1 = np.asarray(inputs["weights_L_1"], np.float32)
    lin1_W = np.asarray(inputs["lin1_W"], np.float32)
    lin1_b = np.asarray(inputs["lin1_b"], np.float32)
    lin_W = np.asarray(inputs["lin_W"], np.float32)
    lin_b = np.asarray(inputs["lin_b"], np.float32)

    N, F0 = x.shape
    F1 = W1.shape[1]
    F2 = W2.shape[1]
    DIM = w_sim.shape[1]
    B = L0.shape[0]
    NQ = te.shape[0]

    # zero-bias fast path (true for this problem; checked, not assumed)
    zb = (not b1.any()) and (not b2.any()) and (not lin1_b.any())

    tpc = math.ceil(math.ceil(N / P) / ncores)
    ntiles = ncores * tpc
    nslot = ntiles * P

    src, dst = ei[0], ei[1]
    deg = (np.bincount(dst, minlength=N) + 1).astype(np.float32)
    dinv = (1.0 / np.sqrt(deg)).astype(np.float32)
    xs = x * dinv[:, None]                                  # x~ = dinv * x

    loops = np.arange(N, dtype=np.int64)
    src_s = np.concatenate([src, loops])
    dst_s = np.concatenate([dst, loops])
    order = np.argsort(dst_s, kind="stable")
    src_s, dst_s = src_s[order], dst_s[order]

    blk_of = dst_s // P
    counts = np.bincount(blk_of, minlength=ntiles)
    starts = np.concatenate([[0], np.cumsum(counts)])
    cb = []
    for b in range(tpc):
        mx = max(int(counts[k * tpc + b]) for k in range(ncores))
        cb.append(max(1, math.ceil(mx / P)))
    CT = int(sum(cb))
    EPAD = CT * P

    src16_cores, dstloc_cores, xe_cores = [], [], []
    for k in range(ncores):
        s_pad = np.zeros(EPAD, np.int64)
        d_pad = np.full(EPAD, -1.0, np.float32)
        off = 0
        for b in range(tpc):
            t = k * tpc + b
            e0, e1 = int(starts[t]), int(starts[t + 1])
            cnt = e1 - e0
            s_pad[off : off + cnt] = src_s[e0:e1]
            d_pad[off : off + cnt] = (dst_s[e0:e1] - t * P).astype(np.float32)
            off += cb[b] * P
        src16_cores.append(_wrap16(s_pad))
        dstloc_cores.append(
            np.ascontiguousarray(d_pad.reshape(CT, P).T.astype(NP_BF16)))
        # edge-ordered x~ rows: edge e at [e%128, e//128, :]
        xe = xs[s_pad].reshape(CT, P, F0).transpose(1, 0, 2)
        xe_cores.append(np.ascontiguousarray(xe.astype(NP_BF16)))

    # queries: pad each core's slice to a multiple of QOP; per-op combined
    # index list [i0 (QOP) | i1 (QOP)]
    nqc = math.ceil(NQ / ncores)
    nqcp = math.ceil(nqc / QOP) * QOP
    nqops = nqcp // QOP
    qidx_cores = []
    for k in range(ncores):
        q = np.zeros((nqcp, 2), np.int64)
        chunk = te[k * nqc : min((k + 1) * nqc, NQ)]
        q[: chunk.shape[0]] = chunk
        per_op = q.reshape(nqops, QOP, 2)
        comb = np.concatenate([per_op[:, :, 0], per_op[:, :, 1]],
                              axis=1).reshape(-1)            # [nqops*2*QOP]
        qidx_cores.append(_wrap16(comb))

    # per-core per-partition 1/sqrt(deg) and 1/deg for own tiles
    dinv_all = np.ones((nslot,), np.float32)
    dinv_all[:N] = dinv
    dinv_all = dinv_all.reshape(ntiles, P).T                 # [P, ntiles]

    shared = {
        "W1": W1.astype(NP_BF16),
        "W2": W2.astype(NP_BF16),
        "L0b": L0.astype(NP_BF16),
        "L0Tb": np.ascontiguousarray(L0.T).astype(NP_BF16),
        "L1b": L1.astype(NP_BF16),
        "L1Tb": np.ascontiguousarray(L1.T).astype(NP_BF16),
        "wodT": np.ascontiguousarray(w_od.T).astype(NP_BF16),
        "wL0": wL0.astype(NP_BF16),
        "wL1": wL1.astype(NP_BF16),
        "embT": np.ascontiguousarray(emb_sim.T).astype(NP_BF16),
        "wsim": w_sim.astype(NP_BF16),
        "lin1Wb": lin1_W.astype(NP_BF16),
        "linWb": lin_W.astype(NP_BF16),
        "lin1bcol": np.ascontiguousarray(lin1_b[:, None]).astype(np.float32),
    }
    if not zb:
        shared["b1bc"] = np.ascontiguousarray(
            np.tile(b1, (P, 1)).astype(np.float32))
        shared["b2bc"] = np.ascontiguousarray(
            np.tile(b2, (P, 1)).astype(np.float32))

    in_maps = []
    for k in range(ncores):
        m = dict(shared)
        m["xe"] = xe_cores[k].reshape(P, CT * F0)
        m["src16"] = src16_cores[k]
        m["dstloc"] = dstloc_cores[k]
        m["qidx16"] = qidx_cores[k]
        dv = dinv_all[:, k * tpc:(k + 1) * tpc]
        m["dinv_own"] = np.ascontiguousarray(dv)
        m["dinv2_own"] = np.ascontiguousarray(dv * dv)
        xo = np.zeros((P, tpc * P), np.float32)
        lo, hi = k * tpc * P, min((k + 1) * tpc * P, N)
        if hi > lo:
            xo[:, : hi - lo] = x[lo:hi].T
        m["xTown"] = xo.astype(NP_BF16)
        in_maps.append(m)

    geom = dict(
        N=N, F0=F0, F1=F1, F2=F2, DIM=DIM, B=B, BT=B // P, SBT=2 * B // P,
        NQ=NQ, tpc=tpc, ntiles=ntiles, nslot=nslot,
        cb=cb, CT=CT, nqc=nqc, nqcp=nqcp, nqops=nqops, ncores=ncores,
        zb=zb, lin_b=float(lin_b[0]),
    )
    return geom, in_maps


# ----------------------------------------------------------------------------
# device kernel
# ----------------------------------------------------------------------------

def build_nc(g):
    tpc, ntiles, nslot = g["tpc"], g["ntiles"], g["nslot"]
    F0, F1, F2, DIM = g["F0"], g["F1"], g["F2"], g["DIM"]
    B, BT, SBT = g["B"], g["BT"], g["SBT"]
    cb, CT = g["cb"], g["CT"]
    nqops = g["nqops"]
    ncores = g["ncores"]
    zb = g["zb"]
    cmax = max(cb)
    rg = [list(range(ncores))]
    FQ = F2 + DIM
    nzcol = nqops * (QOP // P)        # output cols in zps per... per op: QOP/128

    coff = [0]
    for c in cb:
        coff.append(coff[-1] + c)

    nc = bacc.Bacc("TRN2", target_bir_lowering=False, debug=False,
                   num_devices=ncores, num_swdge_queues=NQUEUES,
                   dynamic_dma_scratch_size=DMA_SCRATCH)
    qrr = [0]

    def next_q():
        q = qrr[0] % NQUEUES
        qrr[0] += 1
        return q

    def din(name, shape, dt):
        return nc.dram_tensor(name, shape, dt, kind="ExternalInput")

    xe_d = din("xe", [P, CT * F0], BF16)
    W1_d = din("W1", [F0, F1], BF16)
    W2_d = din("W2", [F1, F2], BF16)
    dinv_own_d = din("dinv_own", [P, tpc], F32)
    dinv2_own_d = din("dinv2_own", [P, tpc], F32)
    xTown_d = din("xTown", [P, tpc * P], BF16)
    src16_d = din("src16", [P, CT * 8], I16)
    dstloc_d = din("dstloc", [P, CT], BF16)
    qidx_d = din("qidx16", [P, nqops * 2 * QOP // 16], I16)
    L0b_d = din("L0b", [B, B], BF16)
    L0Tb_d = din("L0Tb", [B, B], BF16)
    L1b_d = din("L1b", [B, B], BF16)
    L1Tb_d = din("L1Tb", [B, B], BF16)
    wodT_d = din("wodT", [B, B], BF16)
    wL0_d = din("wL0", [B, F2], BF16)
    wL1_d = din("wL1", [B, F2], BF16)
    embT_d = din("embT", [2 * B, F0], BF16)
    wsim_d = din("wsim", [2 * B, DIM], BF16)
    lin1Wb_d = din("lin1Wb", [FQ, F2], BF16)
    linWb_d = din("linWb", [F2, 1], BF16)
    lin1bcol_d = din("lin1bcol", [F2, 1], F32)
    if not zb:
        b1bc_d = din("b1bc", [P, F1], F32)
        b2bc_d = din("b2bc", [P, F2], F32)
    out_d = nc.dram_tensor("out", [g["nqcp"] // P, P], F32,
                           kind="ExternalOutput")

    with tile.TileContext(nc) as tc, (
        tc.tile_pool(name="const", bufs=1)) as cpool, (
        tc.tile_pool(name="persist", bufs=1)) as ppool, (
        tc.tile_pool(name="ps", bufs=2, space="PSUM")) as pspool, (
        tc.tile_pool(name="psagg", bufs=2, space="PSUM")) as psapool, (
        tc.tile_pool(name="dram", bufs=1, space="DRAM")) as dpool:

        # ------------------------------------------------- constants / loads
        ident = cpool.tile([P, P], BF16)
        make_identity(nc, ident[:])

        iota_b = cpool.tile([P, cmax * P], BF16)
        with tc.tile_pool(name="iotatmp", bufs=1) as itpool:
            iota_i = itpool.tile([P, cmax * P], I32)
            nc.gpsimd.iota(iota_i[:], pattern=[[0, cmax], [1, P]], base=0,
                           channel_multiplier=0)
            nc.vector.tensor_copy(iota_b[:], iota_i[:])

        def load(pool, dram_t, shape, dt=BF16, rearr=None):
            t = pool.tile(shape, dt, tag="ld_" + dram_t.name)
            src = dram_t.ap()
            if rearr is not None:
                src = src.rearrange(rearr, p=P)
            nc.sync.dma_start(out=t[:], in_=src)
            return t

        W1sb = load(cpool, W1_d, [F0, F1])
        W2sb = load(cpool, W2_d, [F1, F2])
        dinv_own = load(cpool, dinv_own_d, [P, tpc], F32)
        dinv2_own = load(cpool, dinv2_own_d, [P, tpc], F32)
        src16sb = load(cpool, src16_d, [P, CT * 8], I16)
        dstlocsb = load(cpool, dstloc_d, [P, CT])
        qidxsb = load(cpool, qidx_d, [P, nqops * 2 * QOP // 16], I16)
        lin1Wsb = load(cpool, lin1Wb_d, [FQ, F2])
        linWsb = load(cpool, linWb_d, [F2, 1])
        lin1bcol = load(cpool, lin1bcol_d, [F2, 1], F32)
        xTownsb = load(cpool, xTown_d, [P, tpc * P])
        if not zb:
            b1sb = load(cpool, b1bc_d, [P, F1], F32)
            b2sb = load(cpool, b2bc_d, [P, F2], F32)

        ag_shared = os.environ.get("AG_SHARED", "1") == "1"
        ag_space = "Shared" if ag_shared else "Local"
        ag1_in = dpool.tile([tpc * P, P], BF16)
        ag1_out = dpool.tile([ncores, tpc * P, P], BF16, addr_space=ag_space)
        ag2_in = dpool.tile([tpc * P, P], BF16)
        ag2_out = dpool.tile([ncores, tpc * P, P], BF16, addr_space=ag_space)
        T2g = ag1_out[:].rearrange("r n f -> (r n) f")
        TQ = ag2_out[:].rearrange("r n f -> (r n) f")

        g1rows = ppool.tile([P, tpc, P], BF16)
        g2_all = ppool.tile([P, tpc, F2], F32)
        s_all = ppool.tile([P, tpc, DIM], F32)
        Tqown = ppool.tile([P, tpc, P], BF16)
        nc.vector.memset(Tqown[:], 0.0)
        nc.vector.memset(g1rows[:], 0.0)
        M3sb = ppool.tile([F0, DIM], BF16)

        def build_onehot(b, ohpool):
            cbb = cb[b]
            oh = ohpool.tile([P, cmax * P], BF16, tag="oh")
            nc.vector.tensor_tensor(
                out=oh[:, 0:cbb * P].rearrange("p (c e) -> p c e", e=P),
                in0=iota_b[:, 0:cbb * P].rearrange("p (c e) -> p c e", e=P),
                in1=_bc(dstlocsb[:, coff[b]:coff[b] + cbb], P),
                op=OP.is_equal)
            return oh

        # ------------------------------------------------ conv1 (gather-free)
        with nc.named_scope("conv1"), (
                tc.tile_pool(name="msg1", bufs=3)) as msgpool, (
                tc.tile_pool(name="oh1", bufs=3)) as ohpool, (
                tc.tile_pool(name="epi1", bufs=3)) as epipool:
            for b in range(tpc):
                cbb = cb[b]
                mt = msgpool.tile([P, cmax, F0], BF16, tag="msg")
                nc.sync.dma_start(
                    out=mt[:, 0:cbb, :],
                    in_=xe_d.ap().rearrange(
                        "p (c f) -> p c f", f=F0)[:, coff[b]:coff[b] + cbb, :])
                oh = build_onehot(b, ohpool)
                xps = psapool.tile([P, P], F32, tag="agg")
                for ci in range(cbb):
                    nc.tensor.matmul(
                        out=xps[:], lhsT=mt[:, ci, :],
                        rhs=oh[:, ci * P:(ci + 1) * P],
                        start=(ci == 0), stop=(ci == cbb - 1))
                xaggt = epipool.tile([P, P], BF16, tag="xaggt")
                nc.vector.tensor_copy(xaggt[:], xps[:])
                hps = pspool.tile([P, F1], F32, tag="ps")
                nc.tensor.matmul(out=hps[:], lhsT=xaggt[:], rhs=W1sb[:],
                                 start=True, stop=True)
                # g1s = dinv*relu(dinv*agg + b1); zero-bias: dinv2*relu(agg)
                if zb:
                    nc.scalar.activation(g1rows[:, b, 0:F1], hps[:], AF.Relu,
                                         scale=dinv2_own[:, b:b + 1])
                else:
                    ta = epipool.tile([P, F1], F32, tag="epi1a")
                    nc.scalar.activation(ta[:], hps[:], AF.Copy,
                                         scale=dinv_own[:, b:b + 1])
                    nc.vector.tensor_add(ta[:], ta[:], b1sb[:])
                    nc.vector.tensor_relu(ta[:], ta[:])
                    nc.scalar.activation(g1rows[:, b, 0:F1], ta[:], AF.Copy,
                                         scale=dinv_own[:, b:b + 1])

        with nc.named_scope("ag1"):
            nc.sync.dma_start(
                out=ag1_in[:].rearrange("(t p) f -> p t f", p=P),
                in_=g1rows[:])
            nc.gpsimd.collective_compute(
                "AllGather", OP.bypass, replica_groups=rg,
                ins=[ag1_in.opt()], outs=[ag1_out.opt()])

        # ------------------------------------------------- sim block (bf16)
        with nc.named_scope("sim"), (
                tc.tile_pool(name="sim", bufs=1)) as spool, (
                tc.tile_pool(name="simw", bufs=1)) as swpool, (
                tc.tile_pool(name="pssim", bufs=2, space="PSUM")) as psbpool:
            L0sb = load(spool, L0b_d, [P, BT, B], rearr="(t p) c -> p t c")
            L0Tsb = load(spool, L0Tb_d, [P, BT, B], rearr="(t p) c -> p t c")
            L1sb = load(spool, L1b_d, [P, BT, B], rearr="(t p) c -> p t c")
            L1Tsb = load(spool, L1Tb_d, [P, BT, B], rearr="(t p) c -> p t c")
            wodTsb = load(spool, wodT_d, [P, BT, B], rearr="(t p) c -> p t c")
            wL0sb = load(spool, wL0_d, [P, BT, F2], rearr="(t p) c -> p t c")
            wL1sb = load(spool, wL1_d, [P, BT, F2], rearr="(t p) c -> p t c")
            embTsb = load(spool, embT_d, [P, SBT, F0],
                          rearr="(t p) c -> p t c")
            wsimsb = load(spool, wsim_d, [P, SBT, DIM],
                          rearr="(t p) c -> p t c")

            def mm_accum(out_ap, pairs):
                for i, (lhsT, rhs) in enumerate(pairs):
                    nc.tensor.matmul(out=out_ap, lhsT=lhsT, rhs=rhs,
                                     start=(i == 0),
                                     stop=(i == len(pairs) - 1))

            def big_mm(dst_sb, lhsT_tiles, rhs_tiles, nf):
                for m in range(BT):
                    ps = psbpool.tile([P, nf], F32, tag="simps")
                    mm_accum(ps[:], [(lhsT_tiles(k, m), rhs_tiles(k))
                                     for k in range(BT)])
                    nc.vector.tensor_copy(dst_sb[:, m, :], ps[:])

            L0r = spool.tile([P, BT, B], BF16)
            L0rT = spool.tile([P, BT, B], BF16)
            L1r = spool.tile([P, BT, B], BF16)
            L1rT = spool.tile([P, BT, B], BF16)
            big_mm(L0r, lambda k, m: L0Tsb[:, k, m * P:(m + 1) * P],
                   lambda k: L0sb[:, k, :], B)
            big_mm(L0rT, lambda k, m: L0sb[:, k, m * P:(m + 1) * P],
                   lambda k: L0Tsb[:, k, :], B)
            big_mm(L1r, lambda k, m: L1Tsb[:, k, m * P:(m + 1) * P],
                   lambda k: L1sb[:, k, :], B)
            big_mm(L1rT, lambda k, m: L1sb[:, k, m * P:(m + 1) * P],
                   lambda k: L1Tsb[:, k, :], B)

            P0 = swpool.tile([P, BT, F2], BF16)
            P1 = swpool.tile([P, BT, F2], BF16)
            Qm = swpool.tile([P, BT, F2], BF16)
            big_mm(P0, lambda k, m: L0rT[:, k, m * P:(m + 1) * P],
                   lambda k: wL0sb[:, k, :], F2)
            big_mm(P1, lambda k, m: L1rT[:, k, m * P:(m + 1) * P],
                   lambda k: wL1sb[:, k, :], F2)
            big_mm(Qm, lambda k, m: wodTsb[:, k, m * P:(m + 1) * P],
                   lambda k: P0[:, k, :], F2)

            def transp_small(src_sb, tg):
                dst = swpool.tile([F2, BT, P], BF16, tag=tg)
                for m in range(BT):
                    pt = pspool.tile([P, P], BF16, tag="ps")
                    nc.tensor.transpose(out=pt[0:F2, :], in_=src_sb[:, m, :],
                                        identity=ident[:])
                    nc.vector.tensor_copy(dst[:, m, :], pt[0:F2, :])
                return dst

            Qt = transp_small(Qm, "Qt")
            P1t = transp_small(P1, "P1t")
            relm = spool.tile([P, BT, B], BF16)
            relT = spool.tile([P, BT, B], BF16)
            for m in range(BT):
                ps = psbpool.tile([P, B], F32, tag="simps")
                nc.tensor.matmul(out=ps[:], lhsT=Qt[:, m, :],
                                 rhs=P1t[:].rearrange("p t c -> p (t c)"),
                                 start=True, stop=True)
                nc.vector.tensor_copy(relm[:, m, :], ps[:])
                ps2 = psbpool.tile([P, B], F32, tag="simps")
                nc.tensor.matmul(out=ps2[:], lhsT=P1t[:, m, :],
                                 rhs=Qt[:].rearrange("p t c -> p (t c)"),
                                 start=True, stop=True)
                nc.vector.tensor_copy(relT[:, m, :], ps2[:])

            # softmax(relu(x)): E = max(1, exp(x)); 1/rowsum scales embT cols
            Esb = spool.tile([P, SBT, 2 * B], BF16)
            Ssum = swpool.tile([P, SBT, 2], F32)
            for rt in range(SBT):
                if rt < BT:
                    left, right = L0r[:, rt, :], relm[:, rt, :]
                else:
                    left, right = relT[:, rt - BT, :], L1r[:, rt - BT, :]
                nc.scalar.activation(Esb[:, rt, 0:B], left, AF.Exp)
                nc.scalar.activation(Esb[:, rt, B:2 * B], right, AF.Exp)
                nc.vector.tensor_scalar(
                    out=Esb[:, rt, 0:B], in0=Esb[:, rt, 0:B], scalar1=1.0,
                    scalar2=None, op0=OP.max, op1=OP.add,
                    accum_out=Ssum[:, rt, 0:1])
                nc.vector.tensor_scalar(
                    out=Esb[:, rt, B:2 * B], in0=Esb[:, rt, B:2 * B],
                    scalar1=1.0, scalar2=None, op0=OP.max, op1=OP.add,
                    accum_out=Ssum[:, rt, 1:2])
            rsc = swpool.tile([P, SBT], F32)
            nc.vector.reduce_sum(rsc[:], Ssum[:], axis=mybir.AxisListType.X)
            nc.vector.reciprocal(rsc[:], rsc[:])

            embS = swpool.tile([P, SBT, F0], BF16)
            for kt in range(SBT):
                nc.scalar.activation(embS[:, kt, :], embTsb[:, kt, :],
                                     AF.Copy, scale=rsc[:, kt:kt + 1])

            with tc.tile_pool(name="pst", bufs=1, space="PSUM") as pstpool:
                Tps = pstpool.tile([P, 2 * B], F32, tag="Tps")
                for half in range(2):
                    mm_accum(Tps[:, half * B:(half + 1) * B],
                             [(embS[:, kt, :],
                               Esb[:, kt, half * B:(half + 1) * B])
                              for kt in range(SBT)])
                Tsb = swpool.tile([P, 2 * B], BF16)
                nc.vector.tensor_copy(Tsb[:], Tps[:])
            Tt = swpool.tile([P, SBT, P], BF16)
            for j in range(SBT):
                pt = pspool.tile([P, P], BF16, tag="ps")
                nc.tensor.transpose(out=pt[0:F0, :],
                                    in_=Tsb[:, j * P:(j + 1) * P],
                                    identity=ident[:])
                nc.vector.tensor_copy(Tt[:, j, :], pt[0:F0, :])
            M3ps = pspool.tile([P, DIM], F32, tag="ps")
            mm_accum(M3ps[:], [(Tt[:, kt, :], wsimsb[:, kt, :])
                               for kt in range(SBT)])
            nc.vector.tensor_copy(M3sb[:], M3ps[0:F0, :])

        # s_emb (own rows): accumulate all tiles into one PSUM, one copy out
        with tc.tile_pool(name="pse", bufs=1, space="PSUM") as psepool:
            sps = psepool.tile([P, tpc * DIM], F32, tag="sps")
            for j in range(tpc):
                nc.tensor.matmul(out=sps[:, j * DIM:(j + 1) * DIM],
                                 lhsT=xTownsb[:, j * P:(j + 1) * P],
                                 rhs=M3sb[:], start=True, stop=True)
            nc.vector.tensor_copy(
                s_all[:].rearrange("p t d -> p (t d)"), sps[:])

        with tc.tile_pool(name="rn", bufs=1) as rnpool:
            def renorm_write(src_all, fdim, col0, post_scale):
                sq = rnpool.tile([P, tpc, fdim], F32, tag=f"rn{col0}")
                nc.vector.tensor_mul(sq[:], src_all[:], src_all[:])
                s2 = rnpool.tile([P, tpc], F32, tag=f"rns{col0}")
                nc.vector.reduce_sum(s2[:], sq[:], axis=mybir.AxisListType.X)
                nc.scalar.activation(s2[:], s2[:], AF.Sqrt)
                nc.vector.tensor_scalar_add(s2[:], s2[:], 1e-7)
                nc.vector.reciprocal(s2[:], s2[:])
                if post_scale != 1.0:
                    nc.vector.tensor_scalar(
                        out=s2[:], in0=s2[:], scalar1=post_scale,
                        scalar2=post_scale, op0=OP.mult, op1=OP.min)
                else:
                    nc.vector.tensor_scalar_min(s2[:], s2[:], 1.0)
                nc.vector.tensor_tensor(
                    out=Tqown[:, :, col0:col0 + fdim], in0=src_all[:],
                    in1=_bc(s2[:], fdim), op=OP.mult)

            renorm_write(s_all, DIM, F2, math.sqrt(BETA))

            # --------------------------------------------- conv2 agg
            # gather ops are capped at RING idx; groups of GSZ chunks,
            # deep-buffered so desc-gen pipelines with transfers/compute
            with nc.named_scope("conv2"), (
                    tc.tile_pool(name="msg2", bufs=6)) as msg2pool, (
                    tc.tile_pool(name="oh2", bufs=3)) as oh2pool, (
                    tc.tile_pool(name="epi2", bufs=3)) as epi2pool:
                g2tiles = {}

                def fetch2(c0, gc):
                    mt = msg2pool.tile([P, GSZ, P], BF16, tag="msg")
                    nc.gpsimd.dma_gather(
                        out_ap=mt[:, 0:gc, :], in_ap=T2g,
                        idxs_ap=src16sb[:, c0 * 8:(c0 + gc) * 8],
                        num_idxs=gc * P, num_idxs_reg=gc * P, elem_size=P,
                        queue_num=next_q())
                    g2tiles[c0] = (gc, mt)

                for c0 in range(0, CT, GSZ):
                    fetch2(c0, min(GSZ, CT - c0))

                def chunk2(c):
                    c0 = c // GSZ * GSZ
                    gc, mt = g2tiles[c0]
                    return mt[:, c - c0, 0:F1]

                for b in range(tpc):
                    cbb = cb[b]
                    oh = build_onehot(b, oh2pool)
                    # agg.T directly: lhsT=msg rows, rhs=onehot
                    aps = psapool.tile([F1, P], F32, tag="agg")
                    for ci in range(cbb):
                        c = coff[b] + ci
                        nc.tensor.matmul(
                            out=aps[:], lhsT=chunk2(c),
                            rhs=oh[:, ci * P:(ci + 1) * P],
                            start=(ci == 0), stop=(ci == cbb - 1))
                    a2t = epi2pool.tile([F1, P], BF16, tag="e2a")
                    nc.vector.tensor_copy(a2t[:], aps[:])
                    hps = pspool.tile([P, F2], F32, tag="ps")
                    nc.tensor.matmul(out=hps[:], lhsT=a2t[:], rhs=W2sb[:],
                                     start=True, stop=True)
                    # g2 = relu(dinv*agg2 + b2); zero-bias: relu(dinv*agg2)
                    if zb:
                        nc.scalar.activation(g2_all[:, b, :], hps[:], AF.Relu,
                                             scale=dinv_own[:, b:b + 1])
                    else:
                        nc.scalar.activation(g2_all[:, b, :], hps[:], AF.Copy,
                                             scale=dinv_own[:, b:b + 1])
                        nc.vector.tensor_add(g2_all[:, b, :], g2_all[:, b, :],
                                             b2sb[:])
                        nc.vector.tensor_relu(g2_all[:, b, :],
                                              g2_all[:, b, :])

            renorm_write(g2_all, F2, 0, math.sqrt(ALPHA))

        with nc.named_scope("ag2"):
            nc.sync.dma_start(
                out=ag2_in[:].rearrange("(t p) f -> p t f", p=P),
                in_=Tqown[:])
            nc.gpsimd.collective_compute(
                "AllGather", OP.bypass, replica_groups=rg,
                ins=[ag2_in.opt()], outs=[ag2_out.opt()])

        # ------------------------------------------------- query phase
        SLC = min(512, QOP)
        nslice = QOP // SLC
        with nc.named_scope("query"), (
                tc.tile_pool(name="qg", bufs=3)) as qgpool, (
                tc.tile_pool(name="qw", bufs=3)) as qwpool, (
                tc.tile_pool(name="psq", bufs=2, space="PSUM")) as psqpool, (
                tc.tile_pool(name="psz", bufs=1, space="PSUM")) as pszpool:
            zps = pszpool.tile([P, nzcol], F32)
            for op_i in range(nqops):
                qt = qgpool.tile([P, 1, 2 * QOP], BF16, tag="qga")
                i0 = op_i * (2 * QOP // 16)
                nc.gpsimd.dma_gather(
                    out_ap=qt[:], in_ap=TQ,
                    idxs_ap=qidxsb[:, i0:i0 + 2 * QOP // 16],
                    num_idxs=2 * QOP, num_idxs_reg=2 * QOP, elem_size=P,
                    transpose=True, queue_num=next_q())
                dd = qwpool.tile([FQ, QOP], BF16, tag="qd")
                nc.vector.tensor_sub(dd[:], qt[0:FQ, 0, 0:QOP],
                                     qt[0:FQ, 0, QOP:2 * QOP])
                sq = qwpool.tile([FQ, QOP], BF16, tag="qsq")
                nc.scalar.activation(sq[:], dd[:], AF.Square)
                for s in range(nslice):
                    hps = psqpool.tile([F2, SLC], F32, tag="qps")
                    nc.tensor.matmul(out=hps[:], lhsT=lin1Wsb[:],
                                     rhs=sq[:, s * SLC:(s + 1) * SLC],
                                     start=True, stop=True)
                    hq = qwpool.tile([F2, SLC], BF16, tag="qhq")
                    if not zb:
                        nc.vector.tensor_tensor(out=hps[:], in0=hps[:],
                                                in1=_bc(lin1bcol[:], SLC),
                                                op=OP.add)
                    tmp = qwpool.tile([F2, SLC], F32, tag="qtmp")
                    nc.vector.tensor_scalar_mul(tmp[:], hps[:], LEAKY_SLOPE)
                    nc.vector.tensor_max(hq[:], hps[:], tmp[:])
                    c0 = op_i * (QOP // P) + s * (SLC // P)
                    for t in range(SLC // P):
                        nc.tensor.matmul(
                            out=zps[:, c0 + t:c0 + t + 1],
                            lhsT=hq[:, t * P:(t + 1) * P],
                            rhs=linWsb[:], start=True, stop=True)

            za = ppool.tile([P, nzcol], F32)
            two = cpool.tile([P, 1], F32)
            nc.vector.memset(two[:], 2.0)
            nc.scalar.activation(za[:], zps[:], AF.Abs, bias=g["lin_b"])
            nc.vector.tensor_scalar_min(za[:], za[:], CLAMP_MAX)
            nc.scalar.activation(za[:], za[:], AF.Sigmoid, bias=two[:],
                                 scale=-1.0)
            nc.sync.dma_start(out=out_d.ap().rearrange("j p -> p j"),
                              in_=za[:])

    nc.compile()
    return nc


# ----------------------------------------------------------------------------
# entry point
# ----------------------------------------------------------------------------

def kernel(**inputs):
    geom, in_maps = build_host(inputs, NCORES)
    nc = build_nc(geom)
    res = bass_utils.run_bass_kernel_spmd(
        nc, in_maps, core_ids=list(range(NCORES)))
    outs = []
    for k in range(NCORES):
        o = np.asarray(res.results[k]["out"], np.float32).reshape(-1)
        lo = k * geom["nqc"]
        hi = min((k + 1) * geom["nqc"], geom["NQ"])
        outs.append(o[: hi - lo])
    return np.concatenate(outs).astype(np.float32)


# revision 12
# speedup vs baseline: 1.1424x; 1.1424x over previous
"""Trainium2 Bass kernel for nn_BlockNet (GNN message passing + block-sim MLP).

Strategy (8 NeuronCores, SPMD, single NEFF):
  - GCN aggregation sharded by destination-node tile ranges (tpc x 128-node
    tiles per core).  Edges sorted by dst on host; segment-sum done as one-hot
    matmuls accumulating in PSUM per dst block.  Self loops kept as edges.
  - conv1 needs no device gather: host stages edge-ordered rows
    xe = (x * dinv)[src]; kernel computes (A @ xe) @ W1 per dst block.
  - conv2 messages fetched with per-dst-block dma_gather ops (2304 idx each,
    256B rows) from the Shared AllGather'd node table; SWDGE queues
    round-robin, deep-buffered so descriptor-gen pipelines with transfers.
  - Degree normalization folded into scalar-engine epilogues (biases are
    zero in this problem -- verified on host, generic fallback otherwise).
  - (x @ emb_sim) @ sim_block @ w_sim collapsed to x @ M3 with
    M3 = emb_sim @ sim_block @ w_sim; the BxB block-sim math is replicated
    per core in bf16.
  - Two Shared-output AllGathers exchange (1) the conv1 row table and
    (2) the final 40-wide node feature table.
  - Query phase: data-parallel over query edges; ONE transposed dma_gather
    per 2048 queries fetches both endpoints (4096 idx); MLP runs on
    512-query slices with biases folded into scalar activations.

kernel(**inputs) takes full unsharded inputs, returns the full [NQ] f32
output.
"""

import math
import os
import sys

import numpy as np

for _p in ("/opt/trn_rl_repo", "/root/.axon_site/_ro/trn_rl_repo"):
    if os.path.isdir(_p) and _p not in sys.path:
        sys.path.insert(0, _p)

import concourse.bass as bass
import concourse.bacc as bacc
import concourse.mybir as mybir
import concourse.tile as tile
from concourse import bass_utils
from concourse.masks import make_identity

BF16 = mybir.dt.bfloat16
F32 = mybir.dt.float32
I16 = mybir.dt.int16
I32 = mybir.dt.int32
NP_BF16 = mybir.dt.np(BF16)

P = 128
NCORES = 8
LEAKY_SLOPE = 0.2
ALPHA, BETA = 1.0, 0.1
CLAMP_MAX = 40.0

AF = mybir.ActivationFunctionType
OP = mybir.AluOpType

NQUEUES = 4       # SWDGE queues; gathers round-robin
# SWDGE descriptor-ring capacity is dynamic_dma_scratch_size/16 per queue;
# a single dma_gather's num_idxs must stay below it.
DMA_SCRATCH = int(os.environ.get("DMA_SCRATCH", "16384"))
RING = DMA_SCRATCH // 16
# queries per combined (i0|i1) transposed gather op (2*QOP idx per op)
QOP = int(os.environ.get("QOP", "256"))
GSZ = RING // P   # conv2 message chunks per gather op


def _bc(ap, n):
    """Append a stride-0 broadcast inner dim of size n to an AP."""
    return bass.AP(ap.tensor, ap.offset, list(ap.ap) + [[0, n]])


# ----------------------------------------------------------------------------
# host-side data prep
# ----------------------------------------------------------------------------

def _wrap16(idx):
    """int16 index array in dma_gather wrapped layout [128, n/16]."""
    idx = np.asarray(idx, np.int64)
    n = idx.shape[0]
    assert n % 16 == 0
    w = idx.reshape(n // 16, 16).T.astype(np.int16)        # [16, n/16]
    return np.ascontiguousarray(np.tile(w, (8, 1)))         # [128, n/16]


def build_host(inputs, ncores=NCORES):
    x = np.asarray(inputs["x"], np.float32)
    L0 = np.asarray(inputs["L0"], np.float32)
    L1 = np.asarray(inputs["L1"], np.float32)
    ei = np.asarray(inputs["edge_index"]).astype(np.int64)
    te = np.asarray(inputs["total_edges"]).astype(np.int64)
    W1 = np.asarray(inputs["conv1_W"], np.float32)
    b1 = np.asarray(inputs["conv1_b"], np.float32)
    W2 = np.asarray(inputs["conv2_W"], np.float32)
    b2 = np.asarray(inputs["conv2_b"], np.float32)
    w_sim = np.asarray(inputs["weights_sim"], np.float32)
    emb_sim = np.asarray(inputs["embeddings_sim"], np.float32)
    w_od = np.asarray(inputs["weights_off_diagonal"], np.float32)
    wL0 = np.asarray(inputs["weights_L_0"], np.float32)
    wL1 = np.asarray(inputs["weights_L_1"], np.float32)
    lin1_W = np.asarray(inputs["lin1_W"], np.float32)
    lin1_b = np.asarray(inputs["lin1_b"], np.float32)
    lin_W = np.asarray(inputs["lin_W"], np.float32)
    lin_b = np.asarray(inputs["lin_b"], np.float32)

    N, F0 = x.shape
    F1 = W1.shape[1]
    F2 = W2.shape[1]
    DIM = w_sim.shape[1]
    B = L0.shape[0]
    NQ = te.shape[0]

    # zero-bias fast path (true for this problem; checked, not assumed)
    zb = (not b1.any()) and (not b2.any()) and (not lin1_b.any())

    tpc = math.ceil(math.ceil(N / P) / ncores)
    ntiles = ncores * tpc
    nslot = ntiles * P

    src, dst = ei[0], ei[1]
    deg = (np.bincount(dst, minlength=N) + 1).astype(np.float32)
    dinv = (1.0 / np.sqrt(deg)).astype(np.float32)
    xs = x * dinv[:, None]                                  # x~ = dinv * x

    loops = np.arange(N, dtype=np.int64)
    src_s = np.concatenate([src, loops])
    dst_s = np.concatenate([dst, loops])
    order = np.argsort(dst_s, kind="stable")
    src_s, dst_s = src_s[order], dst_s[order]

    blk_of = dst_s // P
    counts = np.bincount(blk_of, minlength=ntiles)
    starts = np.concatenate([[0], np.cumsum(counts)])
    cb = []
    for b in range(tpc):
        mx = max(int(counts[k * tpc + b]) for k in range(ncores))
        cb.append(max(1, math.ceil(mx / P)))
    CT = int(sum(cb))
    EPAD = CT * P

    src16_cores, dstloc_cores, xe_cores = [], [], []
    # padding slots gather irrelevant data (their one-hot rows are all zero)
    # but MUST spread across the node table: thousands of same-address
    # gathers serialize in the DMA path (cost core 7 ~100us of skew).
    spread = (np.arange(EPAD, dtype=np.int64) * 97) % N
    for k in range(ncores):
        s_pad = spread.copy()
        d_pad = np.full(EPAD, -1.0, np.float32)
        off = 0
        for b in range(tpc):
            t = k * tpc + b
            e0, e1 = int(starts[t]), int(starts[t + 1])
            cnt = e1 - e0
            s_pad[off : off + cnt] = src_s[e0:e1]
            d_pad[off : off + cnt] = (dst_s[e0:e1] - t * P).astype(np.float32)
            off += cb[b] * P
        src16_cores.append(_wrap16(s_pad))
        dstloc_cores.append(
            np.ascontiguousarray(d_pad.reshape(CT, P).T.astype(NP_BF16)))
        # edge-ordered x~ rows: edge e at [e%128, e//128, :]
        xe = xs[s_pad].reshape(CT, P, F0).transpose(1, 0, 2)
        xe_cores.append(np.ascontiguousarray(xe.astype(NP_BF16)))

    # queries: pad each core's slice to a multiple of QOP; per-op combined
    # index list [i0 (QOP) | i1 (QOP)]
    nqc = math.ceil(NQ / ncores)
    nqcp = math.ceil(nqc / QOP) * QOP
    nqops = nqcp // QOP
    qidx_cores = []
    qspread = (np.arange(nqcp, dtype=np.int64) * 89) % N
    for k in range(ncores):
        q = np.stack([qspread, qspread], axis=1)
        chunk = te[k * nqc : min((k + 1) * nqc, NQ)]
        q[: chunk.shape[0]] = chunk
        per_op = q.reshape(nqops, QOP, 2)
        comb = np.concatenate([per_op[:, :, 0], per_op[:, :, 1]],
                              axis=1).reshape(-1)            # [nqops*2*QOP]
        qidx_cores.append(_wrap16(comb))

    # per-core per-partition 1/sqrt(deg) and 1/deg for own tiles
    dinv_all = np.ones((nslot,), np.float32)
    dinv_all[:N] = dinv
    dinv_all = dinv_all.reshape(ntiles, P).T                 # [P, ntiles]

    shared = {
        "W1": W1.astype(NP_BF16),
        "W2": W2.astype(NP_BF16),
        "L0b": L0.astype(NP_BF16),
        "L0Tb": np.ascontiguousarray(L0.T).astype(NP_BF16),
        "L1b": L1.astype(NP_BF16),
        "L1Tb": np.ascontiguousarray(L1.T).astype(NP_BF16),
        "wodT": np.ascontiguousarray(w_od.T).astype(NP_BF16),
        "wL0": wL0.astype(NP_BF16),
        "wL1": wL1.astype(NP_BF16),
        "embT": np.ascontiguousarray(emb_sim.T).astype(NP_BF16),
        "wsim": w_sim.astype(NP_BF16),
        "lin1Wb": lin1_W.astype(NP_BF16),
        "linWb": lin_W.astype(NP_BF16),
        "lin1bcol": np.ascontiguousarray(lin1_b[:, None]).astype(np.float32),
    }
    if not zb:
        shared["b1bc"] = np.ascontiguousarray(
            np.tile(b1, (P, 1)).astype(np.float32))
        shared["b2bc"] = np.ascontiguousarray(
            np.tile(b2, (P, 1)).astype(np.float32))

    in_maps = []
    for k in range(ncores):
        m = dict(shared)
        m["xe"] = xe_cores[k].reshape(P, CT * F0)
        m["src16"] = src16_cores[k]
        m["dstloc"] = dstloc_cores[k]
        m["qidx16"] = qidx_cores[k]
        dv = dinv_all[:, k * tpc:(k + 1) * tpc]
        m["dinv_own"] = np.ascontiguousarray(dv)
        m["dinv2_own"] = np.ascontiguousarray(dv * dv)
        xo = np.zeros((P, tpc * P), np.float32)
        lo, hi = k * tpc * P, min((k + 1) * tpc * P, N)
        if hi > lo:
            xo[:, : hi - lo] = x[lo:hi].T
        m["xTown"] = xo.astype(NP_BF16)
        in_maps.append(m)

    geom = dict(
        N=N, F0=F0, F1=F1, F2=F2, DIM=DIM, B=B, BT=B // P, SBT=2 * B // P,
        NQ=NQ, tpc=tpc, ntiles=ntiles, nslot=nslot,
        cb=cb, CT=CT, nqc=nqc, nqcp=nqcp, nqops=nqops, ncores=ncores,
        zb=zb, lin_b=float(lin_b[0]),
    )
    return geom, in_maps


# ----------------------------------------------------------------------------
# device kernel
# ----------------------------------------------------------------------------

def build_nc(g):
    tpc, ntiles, nslot = g["tpc"], g["ntiles"], g["nslot"]
    F0, F1, F2, DIM = g["F0"], g["F1"], g["F2"], g["DIM"]
    B, BT, SBT = g["B"], g["BT"], g["SBT"]
    cb, CT = g["cb"], g["CT"]
    nqops = g["nqops"]
    ncores = g["ncores"]
    zb = g["zb"]
    cmax = max(cb)
    rg = [list(range(ncores))]
    FQ = F2 + DIM
    nzcol = nqops * (QOP // P)        # output cols in zps per... per op: QOP/128

    coff = [0]
    for c in cb:
        coff.append(coff[-1] + c)

    nc = bacc.Bacc("TRN2", target_bir_lowering=False, debug=False,
                   num_devices=ncores, num_swdge_queues=NQUEUES,
                   dynamic_dma_scratch_size=DMA_SCRATCH)
    qrr = [0]

    def next_q():
        q = qrr[0] % NQUEUES
        qrr[0] += 1
        return q

    def din(name, shape, dt):
        return nc.dram_tensor(name, shape, dt, kind="ExternalInput")

    xe_d = din("xe", [P, CT * F0], BF16)
    W1_d = din("W1", [F0, F1], BF16)
    W2_d = din("W2", [F1, F2], BF16)
    dinv_own_d = din("dinv_own", [P, tpc], F32)
    dinv2_own_d = din("dinv2_own", [P, tpc], F32)
    xTown_d = din("xTown", [P, tpc * P], BF16)
    src16_d = din("src16", [P, CT * 8], I16)
    dstloc_d = din("dstloc", [P, CT], BF16)
    qidx_d = din("qidx16", [P, nqops * 2 * QOP // 16], I16)
    L0b_d = din("L0b", [B, B], BF16)
    L0Tb_d = din("L0Tb", [B, B], BF16)
    L1b_d = din("L1b", [B, B], BF16)
    L1Tb_d = din("L1Tb", [B, B], BF16)
    wodT_d = din("wodT", [B, B], BF16)
    wL0_d = din("wL0", [B, F2], BF16)
    wL1_d = din("wL1", [B, F2], BF16)
    embT_d = din("embT", [2 * B, F0], BF16)
    wsim_d = din("wsim", [2 * B, DIM], BF16)
    lin1Wb_d = din("lin1Wb", [FQ, F2], BF16)
    linWb_d = din("linWb", [F2, 1], BF16)
    lin1bcol_d = din("lin1bcol", [F2, 1], F32)
    if not zb:
        b1bc_d = din("b1bc", [P, F1], F32)
        b2bc_d = din("b2bc", [P, F2], F32)
    out_d = nc.dram_tensor("out", [g["nqcp"] // P, P], F32,
                           kind="ExternalOutput")

    with tile.TileContext(nc) as tc, (
        tc.tile_pool(name="const", bufs=1)) as cpool, (
        tc.tile_pool(name="persist", bufs=1)) as ppool, (
        tc.tile_pool(name="ps", bufs=2, space="PSUM")) as pspool, (
        tc.tile_pool(name="psagg", bufs=2, space="PSUM")) as psapool, (
        tc.tile_pool(name="dram", bufs=1, space="DRAM")) as dpool:

        # ------------------------------------------------- constants / loads
        ident = cpool.tile([P, P], BF16)
        make_identity(nc, ident[:])

        iota_b = cpool.tile([P, cmax * P], BF16)
        with tc.tile_pool(name="iotatmp", bufs=1) as itpool:
            iota_i = itpool.tile([P, cmax * P], I32)
            nc.gpsimd.iota(iota_i[:], pattern=[[0, cmax], [1, P]], base=0,
                           channel_multiplier=0)
            nc.vector.tensor_copy(iota_b[:], iota_i[:])

        def load(pool, dram_t, shape, dt=BF16, rearr=None):
            t = pool.tile(shape, dt, tag="ld_" + dram_t.name)
            src = dram_t.ap()
            if rearr is not None:
                src = src.rearrange(rearr, p=P)
            nc.sync.dma_start(out=t[:], in_=src)
            return t

        W1sb = load(cpool, W1_d, [F0, F1])
        W2sb = load(cpool, W2_d, [F1, F2])
        dinv_own = load(cpool, dinv_own_d, [P, tpc], F32)
        dinv2_own = load(cpool, dinv2_own_d, [P, tpc], F32)
        src16sb = load(cpool, src16_d, [P, CT * 8], I16)
        dstlocsb = load(cpool, dstloc_d, [P, CT])
        qidxsb = load(cpool, qidx_d, [P, nqops * 2 * QOP // 16], I16)
        lin1Wsb = load(cpool, lin1Wb_d, [FQ, F2])
        linWsb = load(cpool, linWb_d, [F2, 1])
        lin1bcol = load(cpool, lin1bcol_d, [F2, 1], F32)
        xTownsb = load(cpool, xTown_d, [P, tpc * P])
        if not zb:
            b1sb = load(cpool, b1bc_d, [P, F1], F32)
            b2sb = load(cpool, b2bc_d, [P, F2], F32)

        ag_shared = os.environ.get("AG_SHARED", "1") == "1"
        ag_space = "Shared" if ag_shared else "Local"
        ag1_in = dpool.tile([tpc * P, P], BF16)
        ag1_out = dpool.tile([ncores, tpc * P, P], BF16, addr_space=ag_space)
        ag2_in = dpool.tile([tpc * P, P], BF16)
        ag2_out = dpool.tile([ncores, tpc * P, P], BF16, addr_space=ag_space)
        T2g = ag1_out[:].rearrange("r n f -> (r n) f")
        TQ = ag2_out[:].rearrange("r n f -> (r n) f")

        g1rows = ppool.tile([P, tpc, P], BF16)
        g2_all = ppool.tile([P, tpc, F2], F32)
        s_all = ppool.tile([P, tpc, DIM], F32)
        Tqown = ppool.tile([P, tpc, P], BF16)
        nc.vector.memset(Tqown[:], 0.0)
        nc.vector.memset(g1rows[:], 0.0)
        M3sb = ppool.tile([F0, DIM], BF16)

        def build_onehot(b, ohpool):
            cbb = cb[b]
            oh = ohpool.tile([P, cmax * P], BF16, tag="oh")
            nc.vector.tensor_tensor(
                out=oh[:, 0:cbb * P].rearrange("p (c e) -> p c e", e=P),
                in0=iota_b[:, 0:cbb * P].rearrange("p (c e) -> p c e", e=P),
                in1=_bc(dstlocsb[:, coff[b]:coff[b] + cbb], P),
                op=OP.is_equal)
            return oh

        # ------------------------------------------------ conv1 (gather-free)
        with nc.named_scope("conv1"), (
                tc.tile_pool(name="msg1", bufs=3)) as msgpool, (
                tc.tile_pool(name="oh1", bufs=3)) as ohpool, (
                tc.tile_pool(name="epi1", bufs=3)) as epipool:
            for b in range(tpc):
                cbb = cb[b]
                mt = msgpool.tile([P, cmax, F0], BF16, tag="msg")
                nc.sync.dma_start(
                    out=mt[:, 0:cbb, :],
                    in_=xe_d.ap().rearrange(
                        "p (c f) -> p c f", f=F0)[:, coff[b]:coff[b] + cbb, :])
                oh = build_onehot(b, ohpool)
                xps = psapool.tile([P, P], F32, tag="agg")
                for ci in range(cbb):
                    nc.tensor.matmul(
                        out=xps[:], lhsT=mt[:, ci, :],
                        rhs=oh[:, ci * P:(ci + 1) * P],
                        start=(ci == 0), stop=(ci == cbb - 1))
                xaggt = epipool.tile([P, P], BF16, tag="xaggt")
                nc.vector.tensor_copy(xaggt[:], xps[:])
                hps = pspool.tile([P, F1], F32, tag="ps")
                nc.tensor.matmul(out=hps[:], lhsT=xaggt[:], rhs=W1sb[:],
                                 start=True, stop=True)
                # g1s = dinv*relu(dinv*agg + b1); zero-bias: dinv2*relu(agg)
                if zb:
                    nc.scalar.activation(g1rows[:, b, 0:F1], hps[:], AF.Relu,
                                         scale=dinv2_own[:, b:b + 1])
                else:
                    ta = epipool.tile([P, F1], F32, tag="epi1a")
                    nc.scalar.activation(ta[:], hps[:], AF.Copy,
                                         scale=dinv_own[:, b:b + 1])
                    nc.vector.tensor_add(ta[:], ta[:], b1sb[:])
                    nc.vector.tensor_relu(ta[:], ta[:])
                    nc.scalar.activation(g1rows[:, b, 0:F1], ta[:], AF.Copy,
                                         scale=dinv_own[:, b:b + 1])

        with nc.named_scope("ag1"):
            nc.sync.dma_start(
                out=ag1_in[:].rearrange("(t p) f -> p t f", p=P),
                in_=g1rows[:])
            nc.gpsimd.collective_compute(
                "AllGather", OP.bypass, replica_groups=rg,
                ins=[ag1_in.opt()], outs=[ag1_out.opt()])

        # ------------------------------------------------- sim block (bf16)
        with nc.named_scope("sim"), (
                tc.tile_pool(name="sim", bufs=1)) as spool, (
                tc.tile_pool(name="simw", bufs=1)) as swpool, (
                tc.tile_pool(name="pssim", bufs=2, space="PSUM")) as psbpool:
            L0sb = load(spool, L0b_d, [P, BT, B], rearr="(t p) c -> p t c")
            L0Tsb = load(spool, L0Tb_d, [P, BT, B], rearr="(t p) c -> p t c")
            L1sb = load(spool, L1b_d, [P, BT, B], rearr="(t p) c -> p t c")
            L1Tsb = load(spool, L1Tb_d, [P, BT, B], rearr="(t p) c -> p t c")
            wodTsb = load(spool, wodT_d, [P, BT, B], rearr="(t p) c -> p t c")
            wL0sb = load(spool, wL0_d, [P, BT, F2], rearr="(t p) c -> p t c")
            wL1sb = load(spool, wL1_d, [P, BT, F2], rearr="(t p) c -> p t c")
            embTsb = load(spool, embT_d, [P, SBT, F0],
                          rearr="(t p) c -> p t c")
            wsimsb = load(spool, wsim_d, [P, SBT, DIM],
                          rearr="(t p) c -> p t c")

            def mm_accum(out_ap, pairs):
                for i, (lhsT, rhs) in enumerate(pairs):
                    nc.tensor.matmul(out=out_ap, lhsT=lhsT, rhs=rhs,
                                     start=(i == 0),
                                     stop=(i == len(pairs) - 1))

            def big_mm(dst_sb, lhsT_tiles, rhs_tiles, nf):
                for m in range(BT):
                    ps = psbpool.tile([P, nf], F32, tag="simps")
                    mm_accum(ps[:], [(lhsT_tiles(k, m), rhs_tiles(k))
                                     for k in range(BT)])
                    nc.vector.tensor_copy(dst_sb[:, m, :], ps[:])

            L0r = spool.tile([P, BT, B], BF16)
            L0rT = spool.tile([P, BT, B], BF16)
            L1r = spool.tile([P, BT, B], BF16)
            L1rT = spool.tile([P, BT, B], BF16)
            big_mm(L0r, lambda k, m: L0Tsb[:, k, m * P:(m + 1) * P],
                   lambda k: L0sb[:, k, :], B)
            big_mm(L0rT, lambda k, m: L0sb[:, k, m * P:(m + 1) * P],
                   lambda k: L0Tsb[:, k, :], B)
            big_mm(L1r, lambda k, m: L1Tsb[:, k, m * P:(m + 1) * P],
                   lambda k: L1sb[:, k, :], B)
            big_mm(L1rT, lambda k, m: L1sb[:, k, m * P:(m + 1) * P],
                   lambda k: L1Tsb[:, k, :], B)

            P0 = swpool.tile([P, BT, F2], BF16)
            P1 = swpool.tile([P, BT, F2], BF16)
            Qm = swpool.tile([P, BT, F2], BF16)
            big_mm(P0, lambda k, m: L0rT[:, k, m * P:(m + 1) * P],
                   lambda k: wL0sb[:, k, :], F2)
            big_mm(P1, lambda k, m: L1rT[:, k, m * P:(m + 1) * P],
                   lambda k: wL1sb[:, k, :], F2)
            big_mm(Qm, lambda k, m: wodTsb[:, k, m * P:(m + 1) * P],
                   lambda k: P0[:, k, :], F2)

            def transp_small(src_sb, tg):
                dst = swpool.tile([F2, BT, P], BF16, tag=tg)
                for m in range(BT):
                    pt = pspool.tile([P, P], BF16, tag="ps")
                    nc.tensor.transpose(out=pt[0:F2, :], in_=src_sb[:, m, :],
                                        identity=ident[:])
                    nc.vector.tensor_copy(dst[:, m, :], pt[0:F2, :])
                return dst

            Qt = transp_small(Qm, "Qt")
            P1t = transp_small(P1, "P1t")
            relm = spool.tile([P, BT, B], BF16)
            relT = spool.tile([P, BT, B], BF16)
            for m in range(BT):
                ps = psbpool.tile([P, B], F32, tag="simps")
                nc.tensor.matmul(out=ps[:], lhsT=Qt[:, m, :],
                                 rhs=P1t[:].rearrange("p t c -> p (t c)"),
                                 start=True, stop=True)
                nc.vector.tensor_copy(relm[:, m, :], ps[:])
                ps2 = psbpool.tile([P, B], F32, tag="simps")
                nc.tensor.matmul(out=ps2[:], lhsT=P1t[:, m, :],
                                 rhs=Qt[:].rearrange("p t c -> p (t c)"),
                                 start=True, stop=True)
                nc.vector.tensor_copy(relT[:, m, :], ps2[:])

            # softmax(relu(x)): E = max(1, exp(x)); 1/rowsum scales embT cols
            Esb = spool.tile([P, SBT, 2 * B], BF16)
            Ssum = swpool.tile([P, SBT, 2], F32)
            for rt in range(SBT):
                if rt < BT:
                    left, right = L0r[:, rt, :], relm[:, rt, :]
                else:
                    left, right = relT[:, rt - BT, :], L1r[:, rt - BT, :]
                nc.scalar.activation(Esb[:, rt, 0:B], left, AF.Exp)
                nc.scalar.activation(Esb[:, rt, B:2 * B], right, AF.Exp)
                nc.vector.tensor_scalar(
                    out=Esb[:, rt, 0:B], in0=Esb[:, rt, 0:B], scalar1=1.0,
                    scalar2=None, op0=OP.max, op1=OP.add,
                    accum_out=Ssum[:, rt, 0:1])
                nc.vector.tensor_scalar(
                    out=Esb[:, rt, B:2 * B], in0=Esb[:, rt, B:2 * B],
                    scalar1=1.0, scalar2=None, op0=OP.max, op1=OP.add,
                    accum_out=Ssum[:, rt, 1:2])
            rsc = swpool.tile([P, SBT], F32)
            nc.vector.reduce_sum(rsc[:], Ssum[:], axis=mybir.AxisListType.X)
            nc.vector.reciprocal(rsc[:], rsc[:])

            embS = swpool.tile([P, SBT, F0], BF16)
            for kt in range(SBT):
                nc.scalar.activation(embS[:, kt, :], embTsb[:, kt, :],
                                     AF.Copy, scale=rsc[:, kt:kt + 1])

            with tc.tile_pool(name="pst", bufs=1, space="PSUM") as pstpool:
                Tps = pstpool.tile([P, 2 * B], F32, tag="Tps")
                for half in range(2):
                    mm_accum(Tps[:, half * B:(half + 1) * B],
                             [(embS[:, kt, :],
                               Esb[:, kt, half * B:(half + 1) * B])
                              for kt in range(SBT)])
                Tsb = swpool.tile([P, 2 * B], BF16)
                nc.vector.tensor_copy(Tsb[:], Tps[:])
            Tt = swpool.tile([P, SBT, P], BF16)
            for j in range(SBT):
                pt = pspool.tile([P, P], BF16, tag="ps")
                nc.tensor.transpose(out=pt[0:F0, :],
                                    in_=Tsb[:, j * P:(j + 1) * P],
                                    identity=ident[:])
                nc.vector.tensor_copy(Tt[:, j, :], pt[0:F0, :])
            M3ps = pspool.tile([P, DIM], F32, tag="ps")
            mm_accum(M3ps[:], [(Tt[:, kt, :], wsimsb[:, kt, :])
                               for kt in range(SBT)])
            nc.vector.tensor_copy(M3sb[:], M3ps[0:F0, :])

        # s_emb (own rows): accumulate all tiles into one PSUM, one copy out
        with tc.tile_pool(name="pse", bufs=1, space="PSUM") as psepool:
            sps = psepool.tile([P, tpc * DIM], F32, tag="sps")
            for j in range(tpc):
                nc.tensor.matmul(out=sps[:, j * DIM:(j + 1) * DIM],
                                 lhsT=xTownsb[:, j * P:(j + 1) * P],
                                 rhs=M3sb[:], start=True, stop=True)
            nc.vector.tensor_copy(
                s_all[:].rearrange("p t d -> p (t d)"), sps[:])

        with tc.tile_pool(name="rn", bufs=1) as rnpool:
            def renorm_write(src_all, fdim, col0, post_scale):
                sq = rnpool.tile([P, tpc, fdim], F32, tag=f"rn{col0}")
                nc.vector.tensor_mul(sq[:], src_all[:], src_all[:])
                s2 = rnpool.tile([P, tpc], F32, tag=f"rns{col0}")
                nc.vector.reduce_sum(s2[:], sq[:], axis=mybir.AxisListType.X)
                nc.scalar.activation(s2[:], s2[:], AF.Sqrt)
                nc.vector.tensor_scalar_add(s2[:], s2[:], 1e-7)
                nc.vector.reciprocal(s2[:], s2[:])
                if post_scale != 1.0:
                    nc.vector.tensor_scalar(
                        out=s2[:], in0=s2[:], scalar1=post_scale,
                        scalar2=post_scale, op0=OP.mult, op1=OP.min)
                else:
                    nc.vector.tensor_scalar_min(s2[:], s2[:], 1.0)
                nc.vector.tensor_tensor(
                    out=Tqown[:, :, col0:col0 + fdim], in0=src_all[:],
                    in1=_bc(s2[:], fdim), op=OP.mult)

            renorm_write(s_all, DIM, F2, math.sqrt(BETA))

            # --------------------------------------------- conv2 agg
            # gather ops are capped at RING idx; groups of GSZ chunks,
            # deep-buffered so desc-gen pipelines with transfers/compute
            with nc.named_scope("conv2"), (
                    tc.tile_pool(name="msg2", bufs=6)) as msg2pool, (
                    tc.tile_pool(name="oh2", bufs=3)) as oh2pool, (
                    tc.tile_pool(name="epi2", bufs=3)) as epi2pool:
                g2tiles = {}

                def fetch2(c0, gc):
                    mt = msg2pool.tile([P, GSZ, P], BF16, tag="msg")
                    nc.gpsimd.dma_gather(
                        out_ap=mt[:, 0:gc, :], in_ap=T2g,
                        idxs_ap=src16sb[:, c0 * 8:(c0 + gc) * 8],
                        num_idxs=gc * P, num_idxs_reg=gc * P, elem_size=P,
                        queue_num=next_q())
                    g2tiles[c0] = (gc, mt)

                for c0 in range(0, CT, GSZ):
                    fetch2(c0, min(GSZ, CT - c0))

                def chunk2(c):
                    c0 = c // GSZ * GSZ
                    gc, mt = g2tiles[c0]
                    return mt[:, c - c0, 0:F1]

                for b in range(tpc):
                    cbb = cb[b]
                    oh = build_onehot(b, oh2pool)
                    # agg.T directly: lhsT=msg rows, rhs=onehot
                    aps = psapool.tile([F1, P], F32, tag="agg")
                    for ci in range(cbb):
                        c = coff[b] + ci
                        nc.tensor.matmul(
                            out=aps[:], lhsT=chunk2(c),
                            rhs=oh[:, ci * P:(ci + 1) * P],
                            start=(ci == 0), stop=(ci == cbb - 1))
                    a2t = epi2pool.tile([F1, P], BF16, tag="e2a")
                    nc.vector.tensor_copy(a2t[:], aps[:])
                    hps = pspool.tile([P, F2], F32, tag="ps")
                    nc.tensor.matmul(out=hps[:], lhsT=a2t[:], rhs=W2sb[:],
                                     start=True, stop=True)
                    # g2 = relu(dinv*agg2 + b2); zero-bias: relu(dinv*agg2)
                    if zb:
                        nc.scalar.activation(g2_all[:, b, :], hps[:], AF.Relu,
                                             scale=dinv_own[:, b:b + 1])
                    else:
                        nc.scalar.activation(g2_all[:, b, :], hps[:], AF.Copy,
                                             scale=dinv_own[:, b:b + 1])
                        nc.vector.tensor_add(g2_all[:, b, :], g2_all[:, b, :],
                                             b2sb[:])
                        nc.vector.tensor_relu(g2_all[:, b, :],
                                              g2_all[:, b, :])

            renorm_write(g2_all, F2, 0, math.sqrt(ALPHA))

        with nc.named_scope("ag2"):
            nc.sync.dma_start(
                out=ag2_in[:].rearrange("(t p) f -> p t f", p=P),
                in_=Tqown[:])
            nc.gpsimd.collective_compute(
                "AllGather", OP.bypass, replica_groups=rg,
                ins=[ag2_in.opt()], outs=[ag2_out.opt()])

        # ------------------------------------------------- query phase
        SLC = min(512, QOP)
        nslice = QOP // SLC
        with nc.named_scope("query"), (
                tc.tile_pool(name="qg", bufs=3)) as qgpool, (
                tc.tile_pool(name="qw", bufs=3)) as qwpool, (
                tc.tile_pool(name="psq", bufs=2, space="PSUM")) as psqpool, (
                tc.tile_pool(name="psz", bufs=1, space="PSUM")) as pszpool:
            zps = pszpool.tile([P, nzcol], F32)
            for op_i in range(nqops):
                qt = qgpool.tile([P, 1, 2 * QOP], BF16, tag="qga")
                i0 = op_i * (2 * QOP // 16)
                nc.gpsimd.dma_gather(
                    out_ap=qt[:], in_ap=TQ,
                    idxs_ap=qidxsb[:, i0:i0 + 2 * QOP // 16],
                    num_idxs=2 * QOP, num_idxs_reg=2 * QOP, elem_size=P,
                    transpose=True, queue_num=next_q())
                dd = qwpool.tile([FQ, QOP], BF16, tag="qd")
                nc.vector.tensor_sub(dd[:], qt[0:FQ, 0, 0:QOP],
                                     qt[0:FQ, 0, QOP:2 * QOP])
                sq = qwpool.tile([FQ, QOP], BF16, tag="qsq")
                nc.scalar.activation(sq[:], dd[:], AF.Square)
                for s in range(nslice):
                    hps = psqpool.tile([F2, SLC], F32, tag="qps")
                    nc.tensor.matmul(out=hps[:], lhsT=lin1Wsb[:],
                                     rhs=sq[:, s * SLC:(s + 1) * SLC],
                                     start=True, stop=True)
                    hq = qwpool.tile([F2, SLC], BF16, tag="qhq")
                    if not zb:
                        nc.vector.tensor_tensor(out=hps[:], in0=hps[:],
                                                in1=_bc(lin1bcol[:], SLC),
                                                op=OP.add)
                    tmp = qwpool.tile([F2, SLC], F32, tag="qtmp")
                    nc.vector.tensor_scalar_mul(tmp[:], hps[:], LEAKY_SLOPE)
                    nc.vector.tensor_max(hq[:], hps[:], tmp[:])
                    c0 = op_i * (QOP // P) + s * (SLC // P)
                    for t in range(SLC // P):
                        nc.tensor.matmul(
                            out=zps[:, c0 + t:c0 + t + 1],
                            lhsT=hq[:, t * P:(t + 1) * P],
                            rhs=linWsb[:], start=True, stop=True)

            za = ppool.tile([P, nzcol], F32)
            two = cpool.tile([P, 1], F32)
            nc.vector.memset(two[:], 2.0)
            nc.scalar.activation(za[:], zps[:], AF.Abs, bias=g["lin_b"])
            nc.vector.tensor_scalar_min(za[:], za[:], CLAMP_MAX)
            nc.scalar.activation(za[:], za[:], AF.Sigmoid, bias=two[:],
                                 scale=-1.0)
            nc.sync.dma_start(out=out_d.ap().rearrange("j p -> p j"),
                              in_=za[:])

    nc.compile()
    return nc


# ----------------------------------------------------------------------------
# entry point
# ----------------------------------------------------------------------------

def kernel(**inputs):
    geom, in_maps = build_host(inputs, NCORES)
    nc = build_nc(geom)
    res = bass_utils.run_bass_kernel_spmd(
        nc, in_maps, core_ids=list(range(NCORES)))
    outs = []
    for k in range(NCORES):
        o = np.asarray(res.results[k]["out"], np.float32).reshape(-1)
        lo = k * geom["nqc"]
        hi = min((k + 1) * geom["nqc"], geom["NQ"])
        outs.append(o[: hi - lo])
    return np.concatenate(outs).astype(np.float32)


# revision 13
# speedup vs baseline: 1.2623x; 1.1049x over previous
"""Trainium2 Bass kernel for nn_BlockNet (GNN message passing + block-sim MLP).

Strategy (8 NeuronCores, SPMD, single NEFF):
  - GCN aggregation sharded by destination-node tile ranges (tpc x 128-node
    tiles per core).  Edges sorted by dst on host; segment-sum done as one-hot
    matmuls accumulating in PSUM per dst block.  Self loops kept as edges.
  - conv1 needs no device gather: host stages edge-ordered rows
    xe = (x * dinv)[src]; kernel computes (A @ xe) @ W1 per dst block.
  - conv2 messages fetched with per-dst-block dma_gather ops (2304 idx each,
    256B rows) from the Shared AllGather'd node table; SWDGE queues
    round-robin, deep-buffered so descriptor-gen pipelines with transfers.
  - Degree normalization folded into scalar-engine epilogues (biases are
    zero in this problem -- verified on host, generic fallback otherwise).
  - (x @ emb_sim) @ sim_block @ w_sim collapsed to x @ M3 with
    M3 = emb_sim @ sim_block @ w_sim; the BxB block-sim math is replicated
    per core in bf16.
  - Two Shared-output AllGathers exchange (1) the conv1 row table and
    (2) the final 40-wide node feature table.
  - Query phase: data-parallel over query edges; ONE transposed dma_gather
    per 2048 queries fetches both endpoints (4096 idx); MLP runs on
    512-query slices with biases folded into scalar activations.

kernel(**inputs) takes full unsharded inputs, returns the full [NQ] f32
output.
"""

import math
import os
import sys

import numpy as np

for _p in ("/opt/trn_rl_repo", "/root/.axon_site/_ro/trn_rl_repo"):
    if os.path.isdir(_p) and _p not in sys.path:
        sys.path.insert(0, _p)

import concourse.bass as bass
import concourse.bacc as bacc
import concourse.mybir as mybir
import concourse.tile as tile
from concourse import bass_utils
from concourse.masks import make_identity

BF16 = mybir.dt.bfloat16
F32 = mybir.dt.float32
I16 = mybir.dt.int16
I32 = mybir.dt.int32
NP_BF16 = mybir.dt.np(BF16)

P = 128
NCORES = 8
LEAKY_SLOPE = 0.2
ALPHA, BETA = 1.0, 0.1
CLAMP_MAX = 40.0

AF = mybir.ActivationFunctionType
OP = mybir.AluOpType

NQUEUES = 4       # SWDGE queues; gathers round-robin
# SWDGE descriptor-ring capacity is dynamic_dma_scratch_size/16 per queue;
# a single dma_gather's num_idxs must stay below it.
DMA_SCRATCH = int(os.environ.get("DMA_SCRATCH", "16384"))
RING = DMA_SCRATCH // 16
# queries per combined (i0|i1) transposed gather op (2*QOP idx per op)
QOP = int(os.environ.get("QOP", "256"))
GSZ = RING // P   # conv2 message chunks per gather op


def _bc(ap, n):
    """Append a stride-0 broadcast inner dim of size n to an AP."""
    return bass.AP(ap.tensor, ap.offset, list(ap.ap) + [[0, n]])


# ----------------------------------------------------------------------------
# host-side data prep
# ----------------------------------------------------------------------------

def _wrap16(idx):
    """int16 index array in dma_gather wrapped layout [128, n/16]."""
    idx = np.asarray(idx, np.int64)
    n = idx.shape[0]
    assert n % 16 == 0
    w = idx.reshape(n // 16, 16).T.astype(np.int16)        # [16, n/16]
    return np.ascontiguousarray(np.tile(w, (8, 1)))         # [128, n/16]


def build_host(inputs, ncores=NCORES):
    x = np.asarray(inputs["x"], np.float32)
    L0 = np.asarray(inputs["L0"], np.float32)
    L1 = np.asarray(inputs["L1"], np.float32)
    ei = np.asarray(inputs["edge_index"]).astype(np.int64)
    te = np.asarray(inputs["total_edges"]).astype(np.int64)
    W1 = np.asarray(inputs["conv1_W"], np.float32)
    b1 = np.asarray(inputs["conv1_b"], np.float32)
    W2 = np.asarray(inputs["conv2_W"], np.float32)
    b2 = np.asarray(inputs["conv2_b"], np.float32)
    w_sim = np.asarray(inputs["weights_sim"], np.float32)
    emb_sim = np.asarray(inputs["embeddings_sim"], np.float32)
    w_od = np.asarray(inputs["weights_off_diagonal"], np.float32)
    wL0 = np.asarray(inputs["weights_L_0"], np.float32)
    wL1 = np.asarray(inputs["weights_L_1"], np.float32)
    lin1_W = np.asarray(inputs["lin1_W"], np.float32)
    lin1_b = np.asarray(inputs["lin1_b"], np.float32)
    lin_W = np.asarray(inputs["lin_W"], np.float32)
    lin_b = np.asarray(inputs["lin_b"], np.float32)

    N, F0 = x.shape
    F1 = W1.shape[1]
    F2 = W2.shape[1]
    DIM = w_sim.shape[1]
    B = L0.shape[0]
    NQ = te.shape[0]

    # zero-bias fast path (true for this problem; checked, not assumed)
    zb = (not b1.any()) and (not b2.any()) and (not lin1_b.any())

    tpc = math.ceil(math.ceil(N / P) / ncores)
    ntiles = ncores * tpc
    nslot = ntiles * P

    src, dst = ei[0], ei[1]
    deg = (np.bincount(dst, minlength=N) + 1).astype(np.float32)
    dinv = (1.0 / np.sqrt(deg)).astype(np.float32)
    xs = x * dinv[:, None]                                  # x~ = dinv * x

    loops = np.arange(N, dtype=np.int64)
    src_s = np.concatenate([src, loops])
    dst_s = np.concatenate([dst, loops])
    order = np.argsort(dst_s, kind="stable")
    src_s, dst_s = src_s[order], dst_s[order]

    blk_of = dst_s // P
    counts = np.bincount(blk_of, minlength=ntiles)
    starts = np.concatenate([[0], np.cumsum(counts)])
    cb = []
    for b in range(tpc):
        mx = max(int(counts[k * tpc + b]) for k in range(ncores))
        cb.append(max(1, math.ceil(mx / P)))
    CT = int(sum(cb))
    EPAD = CT * P

    src16_cores, dstloc_cores, xe_cores = [], [], []
    # padding slots gather irrelevant data (their one-hot rows are all zero)
    # but MUST spread across the node table: thousands of same-address
    # gathers serialize in the DMA path (cost core 7 ~100us of skew).
    spread = (np.arange(EPAD, dtype=np.int64) * 97) % N
    for k in range(ncores):
        s_pad = spread.copy()
        d_pad = np.full(EPAD, -1.0, np.float32)
        off = 0
        for b in range(tpc):
            t = k * tpc + b
            e0, e1 = int(starts[t]), int(starts[t + 1])
            cnt = e1 - e0
            s_pad[off : off + cnt] = src_s[e0:e1]
            d_pad[off : off + cnt] = (dst_s[e0:e1] - t * P).astype(np.float32)
            off += cb[b] * P
        src16_cores.append(_wrap16(s_pad))
        dstloc_cores.append(
            np.ascontiguousarray(d_pad.reshape(CT, P).T.astype(NP_BF16)))
        # edge-ordered x~ rows: edge e at [e%128, e//128, :]
        xe = xs[s_pad].reshape(CT, P, F0).transpose(1, 0, 2)
        xe_cores.append(np.ascontiguousarray(xe.astype(NP_BF16)))

    # queries: pad each core's slice to a multiple of QOP; per-op combined
    # index list [i0 (QOP) | i1 (QOP)]
    nqc = math.ceil(NQ / ncores)
    nqcp = math.ceil(nqc / QOP) * QOP
    nqops = nqcp // QOP
    qidx_cores = []
    qspread = (np.arange(nqcp, dtype=np.int64) * 89) % N
    for k in range(ncores):
        q = np.stack([qspread, qspread], axis=1)
        chunk = te[k * nqc : min((k + 1) * nqc, NQ)]
        q[: chunk.shape[0]] = chunk
        per_op = q.reshape(nqops, QOP, 2)
        comb = np.concatenate([per_op[:, :, 0], per_op[:, :, 1]],
                              axis=1).reshape(-1)            # [nqops*2*QOP]
        qidx_cores.append(_wrap16(comb))

    # per-core per-partition 1/sqrt(deg) and 1/deg for own tiles
    dinv_all = np.ones((nslot,), np.float32)
    dinv_all[:N] = dinv
    dinv_all = dinv_all.reshape(ntiles, P).T                 # [P, ntiles]

    shared = {
        "W1": W1.astype(NP_BF16),
        "W2": W2.astype(NP_BF16),
        "L0b": L0.astype(NP_BF16),
        "L0Tb": np.ascontiguousarray(L0.T).astype(NP_BF16),
        "L1b": L1.astype(NP_BF16),
        "L1Tb": np.ascontiguousarray(L1.T).astype(NP_BF16),
        "wodT": np.ascontiguousarray(w_od.T).astype(NP_BF16),
        "wL0": wL0.astype(NP_BF16),
        "wL1": wL1.astype(NP_BF16),
        "embT": np.ascontiguousarray(emb_sim.T).astype(NP_BF16),
        "wsim": w_sim.astype(NP_BF16),
        "lin1Wb": lin1_W.astype(NP_BF16),
        "linWb": lin_W.astype(NP_BF16),
        "lin1bcol": np.ascontiguousarray(lin1_b[:, None]).astype(np.float32),
    }
    if not zb:
        shared["b1bc"] = np.ascontiguousarray(
            np.tile(b1, (P, 1)).astype(np.float32))
        shared["b2bc"] = np.ascontiguousarray(
            np.tile(b2, (P, 1)).astype(np.float32))

    in_maps = []
    for k in range(ncores):
        m = dict(shared)
        m["xe"] = xe_cores[k].reshape(P, CT * F0)
        m["src16"] = src16_cores[k]
        m["dstloc"] = dstloc_cores[k]
        m["qidx16"] = qidx_cores[k]
        dv = dinv_all[:, k * tpc:(k + 1) * tpc]
        m["dinv_own"] = np.ascontiguousarray(dv)
        m["dinv2_own"] = np.ascontiguousarray(dv * dv)
        xo = np.zeros((P, tpc * P), np.float32)
        lo, hi = k * tpc * P, min((k + 1) * tpc * P, N)
        if hi > lo:
            xo[:, : hi - lo] = x[lo:hi].T
        m["xTown"] = xo.astype(NP_BF16)
        in_maps.append(m)

    geom = dict(
        N=N, F0=F0, F1=F1, F2=F2, DIM=DIM, B=B, BT=B // P, SBT=2 * B // P,
        NQ=NQ, tpc=tpc, ntiles=ntiles, nslot=nslot,
        cb=cb, CT=CT, nqc=nqc, nqcp=nqcp, nqops=nqops, ncores=ncores,
        zb=zb, lin_b=float(lin_b[0]),
    )
    return geom, in_maps


# ----------------------------------------------------------------------------
# device kernel
# ----------------------------------------------------------------------------

def build_nc(g):
    tpc, ntiles, nslot = g["tpc"], g["ntiles"], g["nslot"]
    F0, F1, F2, DIM = g["F0"], g["F1"], g["F2"], g["DIM"]
    B, BT, SBT = g["B"], g["BT"], g["SBT"]
    cb, CT = g["cb"], g["CT"]
    nqops = g["nqops"]
    ncores = g["ncores"]
    zb = g["zb"]
    cmax = max(cb)
    rg = [list(range(ncores))]
    FQ = F2 + DIM
    nzcol = nqops * (QOP // P)        # output cols in zps per... per op: QOP/128

    coff = [0]
    for c in cb:
        coff.append(coff[-1] + c)

    nc = bacc.Bacc("TRN2", target_bir_lowering=False, debug=False,
                   num_devices=ncores, num_swdge_queues=NQUEUES,
                   dynamic_dma_scratch_size=DMA_SCRATCH)
    qrr = [0]

    def next_q():
        q = qrr[0] % NQUEUES
        qrr[0] += 1
        return q

    def din(name, shape, dt):
        return nc.dram_tensor(name, shape, dt, kind="ExternalInput")

    xe_d = din("xe", [P, CT * F0], BF16)
    W1_d = din("W1", [F0, F1], BF16)
    W2_d = din("W2", [F1, F2], BF16)
    dinv_own_d = din("dinv_own", [P, tpc], F32)
    dinv2_own_d = din("dinv2_own", [P, tpc], F32)
    xTown_d = din("xTown", [P, tpc * P], BF16)
    src16_d = din("src16", [P, CT * 8], I16)
    dstloc_d = din("dstloc", [P, CT], BF16)
    qidx_d = din("qidx16", [P, nqops * 2 * QOP // 16], I16)
    L0b_d = din("L0b", [B, B], BF16)
    L0Tb_d = din("L0Tb", [B, B], BF16)
    L1b_d = din("L1b", [B, B], BF16)
    L1Tb_d = din("L1Tb", [B, B], BF16)
    wodT_d = din("wodT", [B, B], BF16)
    wL0_d = din("wL0", [B, F2], BF16)
    wL1_d = din("wL1", [B, F2], BF16)
    embT_d = din("embT", [2 * B, F0], BF16)
    wsim_d = din("wsim", [2 * B, DIM], BF16)
    lin1Wb_d = din("lin1Wb", [FQ, F2], BF16)
    linWb_d = din("linWb", [F2, 1], BF16)
    lin1bcol_d = din("lin1bcol", [F2, 1], F32)
    if not zb:
        b1bc_d = din("b1bc", [P, F1], F32)
        b2bc_d = din("b2bc", [P, F2], F32)
    out_d = nc.dram_tensor("out", [g["nqcp"] // P, P], F32,
                           kind="ExternalOutput")

    with tile.TileContext(nc) as tc, (
        tc.tile_pool(name="const", bufs=1)) as cpool, (
        tc.tile_pool(name="persist", bufs=1)) as ppool, (
        tc.tile_pool(name="ps", bufs=2, space="PSUM")) as pspool, (
        tc.tile_pool(name="psagg", bufs=2, space="PSUM")) as psapool, (
        tc.tile_pool(name="dram", bufs=1, space="DRAM")) as dpool:

        # ------------------------------------------------- constants / loads
        ident = cpool.tile([P, P], BF16)
        make_identity(nc, ident[:])

        iota_b = cpool.tile([P, cmax * P], BF16)
        with tc.tile_pool(name="iotatmp", bufs=1) as itpool:
            iota_i = itpool.tile([P, cmax * P], I32)
            nc.gpsimd.iota(iota_i[:], pattern=[[0, cmax], [1, P]], base=0,
                           channel_multiplier=0)
            nc.vector.tensor_copy(iota_b[:], iota_i[:])

        def load(pool, dram_t, shape, dt=BF16, rearr=None):
            t = pool.tile(shape, dt, tag="ld_" + dram_t.name)
            src = dram_t.ap()
            if rearr is not None:
                src = src.rearrange(rearr, p=P)
            nc.sync.dma_start(out=t[:], in_=src)
            return t

        W1sb = load(cpool, W1_d, [F0, F1])
        W2sb = load(cpool, W2_d, [F1, F2])
        dinv_own = load(cpool, dinv_own_d, [P, tpc], F32)
        dinv2_own = load(cpool, dinv2_own_d, [P, tpc], F32)
        src16sb = load(cpool, src16_d, [P, CT * 8], I16)
        dstlocsb = load(cpool, dstloc_d, [P, CT])
        qidxsb = load(cpool, qidx_d, [P, nqops * 2 * QOP // 16], I16)
        lin1Wsb = load(cpool, lin1Wb_d, [FQ, F2])
        linWsb = load(cpool, linWb_d, [F2, 1])
        lin1bcol = load(cpool, lin1bcol_d, [F2, 1], F32)
        xTownsb = load(cpool, xTown_d, [P, tpc * P])
        if not zb:
            b1sb = load(cpool, b1bc_d, [P, F1], F32)
            b2sb = load(cpool, b2bc_d, [P, F2], F32)

        ag_shared = os.environ.get("AG_SHARED", "1") == "1"
        ag_space = "Shared" if ag_shared else "Local"
        ag1_in = dpool.tile([tpc * P, P], BF16)
        ag1_out = dpool.tile([ncores, tpc * P, P], BF16, addr_space=ag_space)
        ag2_in = dpool.tile([tpc * P, P], BF16)
        ag2_out = dpool.tile([ncores, tpc * P, P], BF16, addr_space=ag_space)
        T2g = ag1_out[:].rearrange("r n f -> (r n) f")
        TQ = ag2_out[:].rearrange("r n f -> (r n) f")

        g1rows = ppool.tile([P, tpc, P], BF16)
        g2_all = ppool.tile([P, tpc, F2], F32)
        s_all = ppool.tile([P, tpc, DIM], F32)
        Tqown = ppool.tile([P, tpc, P], BF16)
        nc.vector.memset(Tqown[:], 0.0)
        nc.vector.memset(g1rows[:], 0.0)
        M3sb = ppool.tile([F0, DIM], BF16)

        def build_onehot(b, ohpool):
            cbb = cb[b]
            oh = ohpool.tile([P, cmax * P], BF16, tag="oh")
            nc.vector.tensor_tensor(
                out=oh[:, 0:cbb * P].rearrange("p (c e) -> p c e", e=P),
                in0=iota_b[:, 0:cbb * P].rearrange("p (c e) -> p c e", e=P),
                in1=_bc(dstlocsb[:, coff[b]:coff[b] + cbb], P),
                op=OP.is_equal)
            return oh

        # ------------------------------------------------ conv1 (gather-free)
        with nc.named_scope("conv1"), (
                tc.tile_pool(name="msg1", bufs=3)) as msgpool, (
                tc.tile_pool(name="oh1", bufs=3)) as ohpool, (
                tc.tile_pool(name="epi1", bufs=3)) as epipool:
            for b in range(tpc):
                cbb = cb[b]
                mt = msgpool.tile([P, cmax, F0], BF16, tag="msg")
                nc.sync.dma_start(
                    out=mt[:, 0:cbb, :],
                    in_=xe_d.ap().rearrange(
                        "p (c f) -> p c f", f=F0)[:, coff[b]:coff[b] + cbb, :])
                oh = build_onehot(b, ohpool)
                xps = psapool.tile([P, P], F32, tag="agg")
                for ci in range(cbb):
                    nc.tensor.matmul(
                        out=xps[:], lhsT=mt[:, ci, :],
                        rhs=oh[:, ci * P:(ci + 1) * P],
                        start=(ci == 0), stop=(ci == cbb - 1))
                xaggt = epipool.tile([P, P], BF16, tag="xaggt")
                nc.vector.tensor_copy(xaggt[:], xps[:])
                hps = pspool.tile([P, F1], F32, tag="ps")
                nc.tensor.matmul(out=hps[:], lhsT=xaggt[:], rhs=W1sb[:],
                                 start=True, stop=True)
                # g1s = dinv*relu(dinv*agg + b1); zero-bias: dinv2*relu(agg)
                if zb:
                    nc.scalar.activation(g1rows[:, b, 0:F1], hps[:], AF.Relu,
                                         scale=dinv2_own[:, b:b + 1])
                else:
                    ta = epipool.tile([P, F1], F32, tag="epi1a")
                    nc.scalar.activation(ta[:], hps[:], AF.Copy,
                                         scale=dinv_own[:, b:b + 1])
                    nc.vector.tensor_add(ta[:], ta[:], b1sb[:])
                    nc.vector.tensor_relu(ta[:], ta[:])
                    nc.scalar.activation(g1rows[:, b, 0:F1], ta[:], AF.Copy,
                                         scale=dinv_own[:, b:b + 1])

        with nc.named_scope("ag1"):
            nc.sync.dma_start(
                out=ag1_in[:].rearrange("(t p) f -> p t f", p=P),
                in_=g1rows[:])
            nc.gpsimd.collective_compute(
                "AllGather", OP.bypass, replica_groups=rg,
                ins=[ag1_in.opt()], outs=[ag1_out.opt()])

        # ------------------------------------------------- sim block (bf16)
        with nc.named_scope("sim"), (
                tc.tile_pool(name="sim", bufs=1)) as spool, (
                tc.tile_pool(name="simw", bufs=1)) as swpool, (
                tc.tile_pool(name="pssim", bufs=2, space="PSUM")) as psbpool:
            L0sb = load(spool, L0b_d, [P, BT, B], rearr="(t p) c -> p t c")
            L0Tsb = load(spool, L0Tb_d, [P, BT, B], rearr="(t p) c -> p t c")
            L1sb = load(spool, L1b_d, [P, BT, B], rearr="(t p) c -> p t c")
            L1Tsb = load(spool, L1Tb_d, [P, BT, B], rearr="(t p) c -> p t c")
            wodTsb = load(spool, wodT_d, [P, BT, B], rearr="(t p) c -> p t c")
            wL0sb = load(spool, wL0_d, [P, BT, F2], rearr="(t p) c -> p t c")
            wL1sb = load(spool, wL1_d, [P, BT, F2], rearr="(t p) c -> p t c")
            embTsb = load(spool, embT_d, [P, SBT, F0],
                          rearr="(t p) c -> p t c")
            wsimsb = load(spool, wsim_d, [P, SBT, DIM],
                          rearr="(t p) c -> p t c")

            def mm_accum(out_ap, pairs):
                for i, (lhsT, rhs) in enumerate(pairs):
                    nc.tensor.matmul(out=out_ap, lhsT=lhsT, rhs=rhs,
                                     start=(i == 0),
                                     stop=(i == len(pairs) - 1))

            def big_mm(dst_sb, lhsT_tiles, rhs_tiles, nf):
                for m in range(BT):
                    ps = psbpool.tile([P, nf], F32, tag="simps")
                    mm_accum(ps[:], [(lhsT_tiles(k, m), rhs_tiles(k))
                                     for k in range(BT)])
                    nc.vector.tensor_copy(dst_sb[:, m, :], ps[:])

            L0r = spool.tile([P, BT, B], BF16)
            L0rT = spool.tile([P, BT, B], BF16)
            L1r = spool.tile([P, BT, B], BF16)
            L1rT = spool.tile([P, BT, B], BF16)
            big_mm(L0r, lambda k, m: L0Tsb[:, k, m * P:(m + 1) * P],
                   lambda k: L0sb[:, k, :], B)
            big_mm(L0rT, lambda k, m: L0sb[:, k, m * P:(m + 1) * P],
                   lambda k: L0Tsb[:, k, :], B)
            big_mm(L1r, lambda k, m: L1Tsb[:, k, m * P:(m + 1) * P],
                   lambda k: L1sb[:, k, :], B)
            big_mm(L1rT, lambda k, m: L1sb[:, k, m * P:(m + 1) * P],
                   lambda k: L1Tsb[:, k, :], B)

            P0 = swpool.tile([P, BT, F2], BF16)
            P1 = swpool.tile([P, BT, F2], BF16)
            Qm = swpool.tile([P, BT, F2], BF16)
            big_mm(P0, lambda k, m: L0rT[:, k, m * P:(m + 1) * P],
                   lambda k: wL0sb[:, k, :], F2)
            big_mm(P1, lambda k, m: L1rT[:, k, m * P:(m + 1) * P],
                   lambda k: wL1sb[:, k, :], F2)
            big_mm(Qm, lambda k, m: wodTsb[:, k, m * P:(m + 1) * P],
                   lambda k: P0[:, k, :], F2)

            def transp_small(src_sb, tg):
                dst = swpool.tile([F2, BT, P], BF16, tag=tg)
                for m in range(BT):
                    pt = pspool.tile([P, P], BF16, tag="ps")
                    nc.tensor.transpose(out=pt[0:F2, :], in_=src_sb[:, m, :],
                                        identity=ident[:])
                    nc.vector.tensor_copy(dst[:, m, :], pt[0:F2, :])
                return dst

            Qt = transp_small(Qm, "Qt")
            P1t = transp_small(P1, "P1t")
            relm = spool.tile([P, BT, B], BF16)
            relT = spool.tile([P, BT, B], BF16)
            for m in range(BT):
                ps = psbpool.tile([P, B], F32, tag="simps")
                nc.tensor.matmul(out=ps[:], lhsT=Qt[:, m, :],
                                 rhs=P1t[:].rearrange("p t c -> p (t c)"),
                                 start=True, stop=True)
                nc.vector.tensor_copy(relm[:, m, :], ps[:])
                ps2 = psbpool.tile([P, B], F32, tag="simps")
                nc.tensor.matmul(out=ps2[:], lhsT=P1t[:, m, :],
                                 rhs=Qt[:].rearrange("p t c -> p (t c)"),
                                 start=True, stop=True)
                nc.vector.tensor_copy(relT[:, m, :], ps2[:])

            # softmax(relu(x)): E = max(1, exp(x)); 1/rowsum scales embT cols
            Esb = spool.tile([P, SBT, 2 * B], BF16)
            Ssum = swpool.tile([P, SBT, 2], F32)
            for rt in range(SBT):
                if rt < BT:
                    left, right = L0r[:, rt, :], relm[:, rt, :]
                else:
                    left, right = relT[:, rt - BT, :], L1r[:, rt - BT, :]
                nc.scalar.activation(Esb[:, rt, 0:B], left, AF.Exp)
                nc.scalar.activation(Esb[:, rt, B:2 * B], right, AF.Exp)
                nc.vector.tensor_scalar(
                    out=Esb[:, rt, 0:B], in0=Esb[:, rt, 0:B], scalar1=1.0,
                    scalar2=None, op0=OP.max, op1=OP.add,
                    accum_out=Ssum[:, rt, 0:1])
                nc.vector.tensor_scalar(
                    out=Esb[:, rt, B:2 * B], in0=Esb[:, rt, B:2 * B],
                    scalar1=1.0, scalar2=None, op0=OP.max, op1=OP.add,
                    accum_out=Ssum[:, rt, 1:2])
            rsc = swpool.tile([P, SBT], F32)
            nc.vector.reduce_sum(rsc[:], Ssum[:], axis=mybir.AxisListType.X)
            nc.vector.reciprocal(rsc[:], rsc[:])

            embS = swpool.tile([P, SBT, F0], BF16)
            for kt in range(SBT):
                nc.scalar.activation(embS[:, kt, :], embTsb[:, kt, :],
                                     AF.Copy, scale=rsc[:, kt:kt + 1])

            with tc.tile_pool(name="pst", bufs=1, space="PSUM") as pstpool:
                Tps = pstpool.tile([P, 2 * B], F32, tag="Tps")
                for half in range(2):
                    mm_accum(Tps[:, half * B:(half + 1) * B],
                             [(embS[:, kt, :],
                               Esb[:, kt, half * B:(half + 1) * B])
                              for kt in range(SBT)])
                Tsb = swpool.tile([P, 2 * B], BF16)
                nc.vector.tensor_copy(Tsb[:], Tps[:])
            Tt = swpool.tile([P, SBT, P], BF16)
            for j in range(SBT):
                pt = pspool.tile([P, P], BF16, tag="ps")
                nc.tensor.transpose(out=pt[0:F0, :],
                                    in_=Tsb[:, j * P:(j + 1) * P],
                                    identity=ident[:])
                nc.vector.tensor_copy(Tt[:, j, :], pt[0:F0, :])
            M3ps = pspool.tile([P, DIM], F32, tag="ps")
            mm_accum(M3ps[:], [(Tt[:, kt, :], wsimsb[:, kt, :])
                               for kt in range(SBT)])
            nc.vector.tensor_copy(M3sb[:], M3ps[0:F0, :])

        # s_emb (own rows): accumulate all tiles into one PSUM, one copy out
        with tc.tile_pool(name="pse", bufs=1, space="PSUM") as psepool:
            sps = psepool.tile([P, tpc * DIM], F32, tag="sps")
            for j in range(tpc):
                nc.tensor.matmul(out=sps[:, j * DIM:(j + 1) * DIM],
                                 lhsT=xTownsb[:, j * P:(j + 1) * P],
                                 rhs=M3sb[:], start=True, stop=True)
            nc.vector.tensor_copy(
                s_all[:].rearrange("p t d -> p (t d)"), sps[:])

        with tc.tile_pool(name="rn", bufs=1) as rnpool:
            def renorm_write(src_all, fdim, col0, post_scale):
                sq = rnpool.tile([P, tpc, fdim], F32, tag=f"rn{col0}")
                nc.vector.tensor_mul(sq[:], src_all[:], src_all[:])
                s2 = rnpool.tile([P, tpc], F32, tag=f"rns{col0}")
                nc.vector.reduce_sum(s2[:], sq[:], axis=mybir.AxisListType.X)
                nc.scalar.activation(s2[:], s2[:], AF.Sqrt)
                nc.vector.tensor_scalar_add(s2[:], s2[:], 1e-7)
                nc.vector.reciprocal(s2[:], s2[:])
                if post_scale != 1.0:
                    nc.vector.tensor_scalar(
                        out=s2[:], in0=s2[:], scalar1=post_scale,
                        scalar2=post_scale, op0=OP.mult, op1=OP.min)
                else:
                    nc.vector.tensor_scalar_min(s2[:], s2[:], 1.0)
                nc.vector.tensor_tensor(
                    out=Tqown[:, :, col0:col0 + fdim], in0=src_all[:],
                    in1=_bc(s2[:], fdim), op=OP.mult)

            renorm_write(s_all, DIM, F2, math.sqrt(BETA))

            # --------------------------------------------- conv2 agg
            # gather ops are capped at RING idx; groups of GSZ chunks,
            # deep-buffered so desc-gen pipelines with transfers/compute
            with nc.named_scope("conv2"), (
                    tc.tile_pool(name="msg2", bufs=6)) as msg2pool, (
                    tc.tile_pool(name="oh2", bufs=3)) as oh2pool, (
                    tc.tile_pool(name="epi2", bufs=3)) as epi2pool:
                g2tiles = {}

                def fetch2(c0, gc):
                    mt = msg2pool.tile([P, GSZ, P], BF16, tag="msg")
                    nc.gpsimd.dma_gather(
                        out_ap=mt[:, 0:gc, :], in_ap=T2g,
                        idxs_ap=src16sb[:, c0 * 8:(c0 + gc) * 8],
                        num_idxs=gc * P, num_idxs_reg=gc * P, elem_size=P,
                        queue_num=next_q())
                    g2tiles[c0] = (gc, mt)

                for c0 in range(0, CT, GSZ):
                    fetch2(c0, min(GSZ, CT - c0))

                def chunk2(c):
                    c0 = c // GSZ * GSZ
                    gc, mt = g2tiles[c0]
                    return mt[:, c - c0, 0:F1]

                for b in range(tpc):
                    cbb = cb[b]
                    oh = build_onehot(b, oh2pool)
                    # agg.T directly: lhsT=msg rows, rhs=onehot
                    aps = psapool.tile([F1, P], F32, tag="agg")
                    for ci in range(cbb):
                        c = coff[b] + ci
                        nc.tensor.matmul(
                            out=aps[:], lhsT=chunk2(c),
                            rhs=oh[:, ci * P:(ci + 1) * P],
                            start=(ci == 0), stop=(ci == cbb - 1))
                    a2t = epi2pool.tile([F1, P], BF16, tag="e2a")
                    nc.vector.tensor_copy(a2t[:], aps[:])
                    hps = pspool.tile([P, F2], F32, tag="ps")
                    nc.tensor.matmul(out=hps[:], lhsT=a2t[:], rhs=W2sb[:],
                                     start=True, stop=True)
                    # g2 = relu(dinv*agg2 + b2); zero-bias: relu(dinv*agg2)
                    if zb:
                        nc.scalar.activation(g2_all[:, b, :], hps[:], AF.Relu,
                                             scale=dinv_own[:, b:b + 1])
                    else:
                        nc.scalar.activation(g2_all[:, b, :], hps[:], AF.Copy,
                                             scale=dinv_own[:, b:b + 1])
                        nc.vector.tensor_add(g2_all[:, b, :], g2_all[:, b, :],
                                             b2sb[:])
                        nc.vector.tensor_relu(g2_all[:, b, :],
                                              g2_all[:, b, :])

            renorm_write(g2_all, F2, 0, math.sqrt(ALPHA))

        with nc.named_scope("ag2"):
            nc.sync.dma_start(
                out=ag2_in[:].rearrange("(t p) f -> p t f", p=P),
                in_=Tqown[:])
            nc.gpsimd.collective_compute(
                "AllGather", OP.bypass, replica_groups=rg,
                ins=[ag2_in.opt()], outs=[ag2_out.opt()])

        # ------------------------------------------------- query phase
        SLC = min(512, QOP)
        nslice = QOP // SLC
        with nc.named_scope("query"), (
                tc.tile_pool(name="qg", bufs=8)) as qgpool, (
                tc.tile_pool(name="qw", bufs=3)) as qwpool, (
                tc.tile_pool(name="psq", bufs=2, space="PSUM")) as psqpool, (
                tc.tile_pool(name="psz", bufs=1, space="PSUM")) as pszpool:
            zps = pszpool.tile([P, nzcol], F32)
            for op_i in range(nqops):
                qt = qgpool.tile([P, 1, 2 * QOP], BF16, tag="qga")
                i0 = op_i * (2 * QOP // 16)
                nc.gpsimd.dma_gather(
                    out_ap=qt[:], in_ap=TQ,
                    idxs_ap=qidxsb[:, i0:i0 + 2 * QOP // 16],
                    num_idxs=2 * QOP, num_idxs_reg=2 * QOP, elem_size=P,
                    transpose=True, queue_num=next_q())
                dd = qwpool.tile([FQ, QOP], BF16, tag="qd")
                nc.vector.tensor_sub(dd[:], qt[0:FQ, 0, 0:QOP],
                                     qt[0:FQ, 0, QOP:2 * QOP])
                sq = qwpool.tile([FQ, QOP], BF16, tag="qsq")
                nc.scalar.activation(sq[:], dd[:], AF.Square)
                for s in range(nslice):
                    hps = psqpool.tile([F2, SLC], F32, tag="qps")
                    nc.tensor.matmul(out=hps[:], lhsT=lin1Wsb[:],
                                     rhs=sq[:, s * SLC:(s + 1) * SLC],
                                     start=True, stop=True)
                    hq = qwpool.tile([F2, SLC], BF16, tag="qhq")
                    if not zb:
                        nc.vector.tensor_tensor(out=hps[:], in0=hps[:],
                                                in1=_bc(lin1bcol[:], SLC),
                                                op=OP.add)
                    tmp = qwpool.tile([F2, SLC], F32, tag="qtmp")
                    nc.vector.tensor_scalar_mul(tmp[:], hps[:], LEAKY_SLOPE)
                    nc.vector.tensor_max(hq[:], hps[:], tmp[:])
                    c0 = op_i * (QOP // P) + s * (SLC // P)
                    for t in range(SLC // P):
                        nc.tensor.matmul(
                            out=zps[:, c0 + t:c0 + t + 1],
                            lhsT=hq[:, t * P:(t + 1) * P],
                            rhs=linWsb[:], start=True, stop=True)

            za = ppool.tile([P, nzcol], F32)
            two = cpool.tile([P, 1], F32)
            nc.vector.memset(two[:], 2.0)
            nc.scalar.activation(za[:], zps[:], AF.Abs, bias=g["lin_b"])
            nc.vector.tensor_scalar_min(za[:], za[:], CLAMP_MAX)
            nc.scalar.activation(za[:], za[:], AF.Sigmoid, bias=two[:],
                                 scale=-1.0)
            nc.sync.dma_start(out=out_d.ap().rearrange("j p -> p j"),
                              in_=za[:])

    nc.compile()
    return nc


# ----------------------------------------------------------------------------
# entry point
# ----------------------------------------------------------------------------

def kernel(**inputs):
    geom, in_maps = build_host(inputs, NCORES)
    nc = build_nc(geom)
    res = bass_utils.run_bass_kernel_spmd(
        nc, in_maps, core_ids=list(range(NCORES)))
    outs = []
    for k in range(NCORES):
        o = np.asarray(res.results[k]["out"], np.float32).reshape(-1)
        lo = k * geom["nqc"]
        hi = min((k + 1) * geom["nqc"], geom["NQ"])
        outs.append(o[: hi - lo])
    return np.concatenate(outs).astype(np.float32)


# revision 15
# speedup vs baseline: 1.3231x; 1.0482x over previous
"""Trainium2 Bass kernel for nn_BlockNet (GNN message passing + block-sim MLP).

Strategy (8 NeuronCores, SPMD, single NEFF):
  - GCN aggregation sharded by destination-node tile ranges (tpc x 128-node
    tiles per core).  Edges sorted by dst on host; segment-sum done as one-hot
    matmuls accumulating in PSUM per dst block.  Self loops kept as edges.
  - conv1 needs no device gather: host stages edge-ordered rows
    xe = (x * dinv)[src]; kernel computes (A @ xe) @ W1 per dst block.
  - conv2 messages fetched with per-dst-block dma_gather ops (2304 idx each,
    256B rows) from the Shared AllGather'd node table; SWDGE queues
    round-robin, deep-buffered so descriptor-gen pipelines with transfers.
  - Degree normalization folded into scalar-engine epilogues (biases are
    zero in this problem -- verified on host, generic fallback otherwise).
  - (x @ emb_sim) @ sim_block @ w_sim collapsed to x @ M3 with
    M3 = emb_sim @ sim_block @ w_sim; the BxB block-sim math is replicated
    per core in bf16.
  - Two Shared-output AllGathers exchange (1) the conv1 row table and
    (2) the final 40-wide node feature table.
  - Query phase: data-parallel over query edges; ONE transposed dma_gather
    per 2048 queries fetches both endpoints (4096 idx); MLP runs on
    512-query slices with biases folded into scalar activations.

kernel(**inputs) takes full unsharded inputs, returns the full [NQ] f32
output.
"""

import math
import os
import sys

import numpy as np

for _p in ("/opt/trn_rl_repo", "/root/.axon_site/_ro/trn_rl_repo"):
    if os.path.isdir(_p) and _p not in sys.path:
        sys.path.insert(0, _p)

import concourse.bass as bass
import concourse.bacc as bacc
import concourse.mybir as mybir
import concourse.tile as tile
from concourse import bass_utils
from concourse.masks import make_identity

BF16 = mybir.dt.bfloat16
F32 = mybir.dt.float32
I16 = mybir.dt.int16
I32 = mybir.dt.int32
NP_BF16 = mybir.dt.np(BF16)

P = 128
NCORES = 8
LEAKY_SLOPE = 0.2
ALPHA, BETA = 1.0, 0.1
CLAMP_MAX = 40.0

AF = mybir.ActivationFunctionType
OP = mybir.AluOpType

NQUEUES = 4       # SWDGE queues; gathers round-robin
# SWDGE descriptor-ring capacity is dynamic_dma_scratch_size/16 per queue;
# a single dma_gather's num_idxs must stay below it.
DMA_SCRATCH = int(os.environ.get("DMA_SCRATCH", "16384"))
RING = DMA_SCRATCH // 16
# queries per combined (i0|i1) transposed gather op (2*QOP idx per op)
QOP = int(os.environ.get("QOP", "256"))
GSZ = RING // P   # conv2 message chunks per gather op


def _bc(ap, n):
    """Append a stride-0 broadcast inner dim of size n to an AP."""
    return bass.AP(ap.tensor, ap.offset, list(ap.ap) + [[0, n]])


# ----------------------------------------------------------------------------
# host-side data prep
# ----------------------------------------------------------------------------

def _wrap16(idx):
    """int16 index array in dma_gather wrapped layout [128, n/16]."""
    idx = np.asarray(idx, np.int64)
    n = idx.shape[0]
    assert n % 16 == 0
    w = idx.reshape(n // 16, 16).T.astype(np.int16)        # [16, n/16]
    return np.ascontiguousarray(np.tile(w, (8, 1)))         # [128, n/16]


def build_host(inputs, ncores=NCORES):
    x = np.asarray(inputs["x"], np.float32)
    L0 = np.asarray(inputs["L0"], np.float32)
    L1 = np.asarray(inputs["L1"], np.float32)
    ei = np.asarray(inputs["edge_index"]).astype(np.int64)
    te = np.asarray(inputs["total_edges"]).astype(np.int64)
    W1 = np.asarray(inputs["conv1_W"], np.float32)
    b1 = np.asarray(inputs["conv1_b"], np.float32)
    W2 = np.asarray(inputs["conv2_W"], np.float32)
    b2 = np.asarray(inputs["conv2_b"], np.float32)
    w_sim = np.asarray(inputs["weights_sim"], np.float32)
    emb_sim = np.asarray(inputs["embeddings_sim"], np.float32)
    w_od = np.asarray(inputs["weights_off_diagonal"], np.float32)
    wL0 = np.asarray(inputs["weights_L_0"], np.float32)
    wL1 = np.asarray(inputs["weights_L_1"], np.float32)
    lin1_W = np.asarray(inputs["lin1_W"], np.float32)
    lin1_b = np.asarray(inputs["lin1_b"], np.float32)
    lin_W = np.asarray(inputs["lin_W"], np.float32)
    lin_b = np.asarray(inputs["lin_b"], np.float32)

    N, F0 = x.shape
    F1 = W1.shape[1]
    F2 = W2.shape[1]
    DIM = w_sim.shape[1]
    B = L0.shape[0]
    NQ = te.shape[0]

    # zero-bias fast path (true for this problem; checked, not assumed)
    zb = (not b1.any()) and (not b2.any()) and (not lin1_b.any())

    tpc = math.ceil(math.ceil(N / P) / ncores)
    ntiles = ncores * tpc
    nslot = ntiles * P

    src, dst = ei[0], ei[1]
    deg = (np.bincount(dst, minlength=N) + 1).astype(np.float32)
    dinv = (1.0 / np.sqrt(deg)).astype(np.float32)
    xs = x * dinv[:, None]                                  # x~ = dinv * x

    loops = np.arange(N, dtype=np.int64)
    src_s = np.concatenate([src, loops])
    dst_s = np.concatenate([dst, loops])
    order = np.argsort(dst_s, kind="stable")
    src_s, dst_s = src_s[order], dst_s[order]

    blk_of = dst_s // P
    counts = np.bincount(blk_of, minlength=ntiles)
    starts = np.concatenate([[0], np.cumsum(counts)])
    cb = []
    for b in range(tpc):
        mx = max(int(counts[k * tpc + b]) for k in range(ncores))
        cb.append(max(1, math.ceil(mx / P)))
    CT = int(sum(cb))
    EPAD = CT * P

    src16_cores, dstloc_cores, xe_cores = [], [], []
    # padding slots gather irrelevant data (their one-hot rows are all zero)
    # but MUST spread across the node table: thousands of same-address
    # gathers serialize in the DMA path (cost core 7 ~100us of skew).
    spread = (np.arange(EPAD, dtype=np.int64) * 97) % N
    for k in range(ncores):
        s_pad = spread.copy()
        d_pad = np.full(EPAD, -1.0, np.float32)
        off = 0
        for b in range(tpc):
            t = k * tpc + b
            e0, e1 = int(starts[t]), int(starts[t + 1])
            cnt = e1 - e0
            s_pad[off : off + cnt] = src_s[e0:e1]
            d_pad[off : off + cnt] = (dst_s[e0:e1] - t * P).astype(np.float32)
            off += cb[b] * P
        src16_cores.append(_wrap16(s_pad))
        dstloc_cores.append(
            np.ascontiguousarray(d_pad.reshape(CT, P).T.astype(NP_BF16)))
        # edge-ordered x~ rows: edge e at [e%128, e//128, :]
        xe = xs[s_pad].reshape(CT, P, F0).transpose(1, 0, 2)
        xe_cores.append(np.ascontiguousarray(xe.astype(NP_BF16)))

    # queries: pad each core's slice to a multiple of QOP; per-op combined
    # index list [i0 (QOP) | i1 (QOP)]
    nqc = math.ceil(NQ / ncores)
    nqcp = math.ceil(nqc / QOP) * QOP
    nqops = nqcp // QOP
    qidx_cores = []
    qspread = (np.arange(nqcp, dtype=np.int64) * 89) % N
    for k in range(ncores):
        q = np.stack([qspread, qspread], axis=1)
        chunk = te[k * nqc : min((k + 1) * nqc, NQ)]
        q[: chunk.shape[0]] = chunk
        per_op = q.reshape(nqops, QOP, 2)
        comb = np.concatenate([per_op[:, :, 0], per_op[:, :, 1]],
                              axis=1).reshape(-1)            # [nqops*2*QOP]
        qidx_cores.append(_wrap16(comb))

    # per-core per-partition 1/sqrt(deg) and 1/deg for own tiles
    dinv_all = np.ones((nslot,), np.float32)
    dinv_all[:N] = dinv
    dinv_all = dinv_all.reshape(ntiles, P).T                 # [P, ntiles]

    shared = {
        "W1": W1.astype(NP_BF16),
        "W2": W2.astype(NP_BF16),
        "L0b": L0.astype(NP_BF16),
        "L0Tb": np.ascontiguousarray(L0.T).astype(NP_BF16),
        "L1b": L1.astype(NP_BF16),
        "L1Tb": np.ascontiguousarray(L1.T).astype(NP_BF16),
        "wodT": np.ascontiguousarray(w_od.T).astype(NP_BF16),
        "wL0": wL0.astype(NP_BF16),
        "wL1": wL1.astype(NP_BF16),
        "embT": np.ascontiguousarray(emb_sim.T).astype(NP_BF16),
        "wsim": w_sim.astype(NP_BF16),
        "lin1Wb": lin1_W.astype(NP_BF16),
        "linWb": lin_W.astype(NP_BF16),
        "lin1bcol": np.ascontiguousarray(lin1_b[:, None]).astype(np.float32),
    }
    if not zb:
        shared["b1bc"] = np.ascontiguousarray(
            np.tile(b1, (P, 1)).astype(np.float32))
        shared["b2bc"] = np.ascontiguousarray(
            np.tile(b2, (P, 1)).astype(np.float32))

    in_maps = []
    for k in range(ncores):
        m = dict(shared)
        m["xe"] = xe_cores[k].reshape(P, CT * F0)
        m["src16"] = src16_cores[k]
        m["dstloc"] = dstloc_cores[k]
        m["qidx16"] = qidx_cores[k]
        dv = dinv_all[:, k * tpc:(k + 1) * tpc]
        m["dinv_own"] = np.ascontiguousarray(dv)
        m["dinv2_own"] = np.ascontiguousarray(dv * dv)
        xo = np.zeros((P, tpc * P), np.float32)
        lo, hi = k * tpc * P, min((k + 1) * tpc * P, N)
        if hi > lo:
            xo[:, : hi - lo] = x[lo:hi].T
        m["xTown"] = xo.astype(NP_BF16)
        in_maps.append(m)

    geom = dict(
        N=N, F0=F0, F1=F1, F2=F2, DIM=DIM, B=B, BT=B // P, SBT=2 * B // P,
        NQ=NQ, tpc=tpc, ntiles=ntiles, nslot=nslot,
        cb=cb, CT=CT, nqc=nqc, nqcp=nqcp, nqops=nqops, ncores=ncores,
        zb=zb, lin_b=float(lin_b[0]),
    )
    return geom, in_maps


# ----------------------------------------------------------------------------
# device kernel
# ----------------------------------------------------------------------------

def build_nc(g):
    tpc, ntiles, nslot = g["tpc"], g["ntiles"], g["nslot"]
    F0, F1, F2, DIM = g["F0"], g["F1"], g["F2"], g["DIM"]
    B, BT, SBT = g["B"], g["BT"], g["SBT"]
    cb, CT = g["cb"], g["CT"]
    nqops = g["nqops"]
    ncores = g["ncores"]
    zb = g["zb"]
    cmax = max(cb)
    rg = [list(range(ncores))]
    FQ = F2 + DIM
    nzcol = nqops * (QOP // P)        # output cols in zps per... per op: QOP/128

    coff = [0]
    for c in cb:
        coff.append(coff[-1] + c)

    nc = bacc.Bacc("TRN2", target_bir_lowering=False, debug=False,
                   num_devices=ncores, num_swdge_queues=NQUEUES,
                   dynamic_dma_scratch_size=DMA_SCRATCH)
    qrr = [0]

    def next_q():
        q = qrr[0] % NQUEUES
        qrr[0] += 1
        return q

    def din(name, shape, dt):
        return nc.dram_tensor(name, shape, dt, kind="ExternalInput")

    xe_d = din("xe", [P, CT * F0], BF16)
    W1_d = din("W1", [F0, F1], BF16)
    W2_d = din("W2", [F1, F2], BF16)
    dinv_own_d = din("dinv_own", [P, tpc], F32)
    dinv2_own_d = din("dinv2_own", [P, tpc], F32)
    xTown_d = din("xTown", [P, tpc * P], BF16)
    src16_d = din("src16", [P, CT * 8], I16)
    dstloc_d = din("dstloc", [P, CT], BF16)
    qidx_d = din("qidx16", [P, nqops * 2 * QOP // 16], I16)
    L0b_d = din("L0b", [B, B], BF16)
    L0Tb_d = din("L0Tb", [B, B], BF16)
    L1b_d = din("L1b", [B, B], BF16)
    L1Tb_d = din("L1Tb", [B, B], BF16)
    wodT_d = din("wodT", [B, B], BF16)
    wL0_d = din("wL0", [B, F2], BF16)
    wL1_d = din("wL1", [B, F2], BF16)
    embT_d = din("embT", [2 * B, F0], BF16)
    wsim_d = din("wsim", [2 * B, DIM], BF16)
    lin1Wb_d = din("lin1Wb", [FQ, F2], BF16)
    linWb_d = din("linWb", [F2, 1], BF16)
    lin1bcol_d = din("lin1bcol", [F2, 1], F32)
    if not zb:
        b1bc_d = din("b1bc", [P, F1], F32)
        b2bc_d = din("b2bc", [P, F2], F32)
    out_d = nc.dram_tensor("out", [g["nqcp"] // P, P], F32,
                           kind="ExternalOutput")

    with tile.TileContext(nc) as tc, (
        tc.tile_pool(name="const", bufs=1)) as cpool, (
        tc.tile_pool(name="persist", bufs=1)) as ppool, (
        tc.tile_pool(name="ps", bufs=2, space="PSUM")) as pspool, (
        tc.tile_pool(name="psagg", bufs=2, space="PSUM")) as psapool, (
        tc.tile_pool(name="dram", bufs=1, space="DRAM")) as dpool:

        # Warm up the collective path first: the first collective of a NEFF
        # pays ~57us of CC cold-start (trigger -> ALGO_MESH_BEGIN); a dummy
        # AllGather absorbs it while conv1 runs.
        with nc.named_scope("agwarm"):
            warm_in = dpool.tile([16, 16], F32)
            warm_out = dpool.tile([ncores, 16, 16], F32)
            warm_sb = cpool.tile([16, 16], F32)
            nc.vector.memset(warm_sb[:], 0.0)
            nc.sync.dma_start(out=warm_in[:], in_=warm_sb[:])
            nc.gpsimd.collective_compute(
                "AllGather", OP.bypass, replica_groups=rg,
                ins=[warm_in.opt()], outs=[warm_out.opt()])

        # ------------------------------------------------- constants / loads
        ident = cpool.tile([P, P], BF16)
        make_identity(nc, ident[:])

        iota_b = cpool.tile([P, cmax * P], BF16)
        with tc.tile_pool(name="iotatmp", bufs=1) as itpool:
            iota_i = itpool.tile([P, cmax * P], I32)
            nc.gpsimd.iota(iota_i[:], pattern=[[0, cmax], [1, P]], base=0,
                           channel_multiplier=0)
            nc.vector.tensor_copy(iota_b[:], iota_i[:])

        def load(pool, dram_t, shape, dt=BF16, rearr=None):
            t = pool.tile(shape, dt, tag="ld_" + dram_t.name)
            src = dram_t.ap()
            if rearr is not None:
                src = src.rearrange(rearr, p=P)
            nc.sync.dma_start(out=t[:], in_=src)
            return t

        W1sb = load(cpool, W1_d, [F0, F1])
        W2sb = load(cpool, W2_d, [F1, F2])
        dinv_own = load(cpool, dinv_own_d, [P, tpc], F32)
        dinv2_own = load(cpool, dinv2_own_d, [P, tpc], F32)
        src16sb = load(cpool, src16_d, [P, CT * 8], I16)
        dstlocsb = load(cpool, dstloc_d, [P, CT])
        qidxsb = load(cpool, qidx_d, [P, nqops * 2 * QOP // 16], I16)
        lin1Wsb = load(cpool, lin1Wb_d, [FQ, F2])
        linWsb = load(cpool, linWb_d, [F2, 1])
        lin1bcol = load(cpool, lin1bcol_d, [F2, 1], F32)
        xTownsb = load(cpool, xTown_d, [P, tpc * P])
        if not zb:
            b1sb = load(cpool, b1bc_d, [P, F1], F32)
            b2sb = load(cpool, b2bc_d, [P, F2], F32)

        ag_shared = os.environ.get("AG_SHARED", "1") == "1"
        ag_space = "Shared" if ag_shared else "Local"
        ag1_in = dpool.tile([tpc * P, P], BF16)
        ag1_out = dpool.tile([ncores, tpc * P, P], BF16, addr_space=ag_space)
        ag2_in = dpool.tile([tpc * P, P], BF16)
        ag2_out = dpool.tile([ncores, tpc * P, P], BF16, addr_space=ag_space)
        T2g = ag1_out[:].rearrange("r n f -> (r n) f")
        TQ = ag2_out[:].rearrange("r n f -> (r n) f")

        g1rows = ppool.tile([P, tpc, P], BF16)
        g2_all = ppool.tile([P, tpc, F2], F32)
        s_all = ppool.tile([P, tpc, DIM], F32)
        Tqown = ppool.tile([P, tpc, P], BF16)
        nc.vector.memset(Tqown[:], 0.0)
        nc.vector.memset(g1rows[:], 0.0)
        M3sb = ppool.tile([F0, DIM], BF16)

        def build_onehot(b, ohpool):
            cbb = cb[b]
            oh = ohpool.tile([P, cmax * P], BF16, tag="oh")
            nc.vector.tensor_tensor(
                out=oh[:, 0:cbb * P].rearrange("p (c e) -> p c e", e=P),
                in0=iota_b[:, 0:cbb * P].rearrange("p (c e) -> p c e", e=P),
                in1=_bc(dstlocsb[:, coff[b]:coff[b] + cbb], P),
                op=OP.is_equal)
            return oh

        # ------------------------------------------------ conv1 (gather-free)
        with nc.named_scope("conv1"), (
                tc.tile_pool(name="msg1", bufs=3)) as msgpool, (
                tc.tile_pool(name="oh1", bufs=3)) as ohpool, (
                tc.tile_pool(name="epi1", bufs=3)) as epipool:
            for b in range(tpc):
                cbb = cb[b]
                mt = msgpool.tile([P, cmax, F0], BF16, tag="msg")
                nc.sync.dma_start(
                    out=mt[:, 0:cbb, :],
                    in_=xe_d.ap().rearrange(
                        "p (c f) -> p c f", f=F0)[:, coff[b]:coff[b] + cbb, :])
                oh = build_onehot(b, ohpool)
                xps = psapool.tile([P, P], F32, tag="agg")
                for ci in range(cbb):
                    nc.tensor.matmul(
                        out=xps[:], lhsT=mt[:, ci, :],
                        rhs=oh[:, ci * P:(ci + 1) * P],
                        start=(ci == 0), stop=(ci == cbb - 1))
                xaggt = epipool.tile([P, P], BF16, tag="xaggt")
                nc.vector.tensor_copy(xaggt[:], xps[:])
                hps = pspool.tile([P, F1], F32, tag="ps")
                nc.tensor.matmul(out=hps[:], lhsT=xaggt[:], rhs=W1sb[:],
                                 start=True, stop=True)
                # g1s = dinv*relu(dinv*agg + b1); zero-bias: dinv2*relu(agg)
                if zb:
                    nc.scalar.activation(g1rows[:, b, 0:F1], hps[:], AF.Relu,
                                         scale=dinv2_own[:, b:b + 1])
                else:
                    ta = epipool.tile([P, F1], F32, tag="epi1a")
                    nc.scalar.activation(ta[:], hps[:], AF.Copy,
                                         scale=dinv_own[:, b:b + 1])
                    nc.vector.tensor_add(ta[:], ta[:], b1sb[:])
                    nc.vector.tensor_relu(ta[:], ta[:])
                    nc.scalar.activation(g1rows[:, b, 0:F1], ta[:], AF.Copy,
                                         scale=dinv_own[:, b:b + 1])

        with nc.named_scope("ag1"):
            nc.sync.dma_start(
                out=ag1_in[:].rearrange("(t p) f -> p t f", p=P),
                in_=g1rows[:])
            nc.gpsimd.collective_compute(
                "AllGather", OP.bypass, replica_groups=rg,
                ins=[ag1_in.opt()], outs=[ag1_out.opt()])

        # ------------------------------------------------- sim block (bf16)
        with nc.named_scope("sim"), (
                tc.tile_pool(name="sim", bufs=1)) as spool, (
                tc.tile_pool(name="simw", bufs=1)) as swpool, (
                tc.tile_pool(name="pssim", bufs=2, space="PSUM")) as psbpool:
            L0sb = load(spool, L0b_d, [P, BT, B], rearr="(t p) c -> p t c")
            L0Tsb = load(spool, L0Tb_d, [P, BT, B], rearr="(t p) c -> p t c")
            L1sb = load(spool, L1b_d, [P, BT, B], rearr="(t p) c -> p t c")
            L1Tsb = load(spool, L1Tb_d, [P, BT, B], rearr="(t p) c -> p t c")
            wodTsb = load(spool, wodT_d, [P, BT, B], rearr="(t p) c -> p t c")
            wL0sb = load(spool, wL0_d, [P, BT, F2], rearr="(t p) c -> p t c")
            wL1sb = load(spool, wL1_d, [P, BT, F2], rearr="(t p) c -> p t c")
            embTsb = load(spool, embT_d, [P, SBT, F0],
                          rearr="(t p) c -> p t c")
            wsimsb = load(spool, wsim_d, [P, SBT, DIM],
                          rearr="(t p) c -> p t c")

            def mm_accum(out_ap, pairs):
                for i, (lhsT, rhs) in enumerate(pairs):
                    nc.tensor.matmul(out=out_ap, lhsT=lhsT, rhs=rhs,
                                     start=(i == 0),
                                     stop=(i == len(pairs) - 1))

            def big_mm(dst_sb, lhsT_tiles, rhs_tiles, nf):
                for m in range(BT):
                    ps = psbpool.tile([P, nf], F32, tag="simps")
                    mm_accum(ps[:], [(lhsT_tiles(k, m), rhs_tiles(k))
                                     for k in range(BT)])
                    nc.vector.tensor_copy(dst_sb[:, m, :], ps[:])

            L0r = spool.tile([P, BT, B], BF16)
            L0rT = spool.tile([P, BT, B], BF16)
            L1r = spool.tile([P, BT, B], BF16)
            L1rT = spool.tile([P, BT, B], BF16)
            big_mm(L0r, lambda k, m: L0Tsb[:, k, m * P:(m + 1) * P],
                   lambda k: L0sb[:, k, :], B)
            big_mm(L0rT, lambda k, m: L0sb[:, k, m * P:(m + 1) * P],
                   lambda k: L0Tsb[:, k, :], B)
            big_mm(L1r, lambda k, m: L1Tsb[:, k, m * P:(m + 1) * P],
                   lambda k: L1sb[:, k, :], B)
            big_mm(L1rT, lambda k, m: L1sb[:, k, m * P:(m + 1) * P],
                   lambda k: L1Tsb[:, k, :], B)

            P0 = swpool.tile([P, BT, F2], BF16)
            P1 = swpool.tile([P, BT, F2], BF16)
            Qm = swpool.tile([P, BT, F2], BF16)
            big_mm(P0, lambda k, m: L0rT[:, k, m * P:(m + 1) * P],
                   lambda k: wL0sb[:, k, :], F2)
            big_mm(P1, lambda k, m: L1rT[:, k, m * P:(m + 1) * P],
                   lambda k: wL1sb[:, k, :], F2)
            big_mm(Qm, lambda k, m: wodTsb[:, k, m * P:(m + 1) * P],
                   lambda k: P0[:, k, :], F2)

            def transp_small(src_sb, tg):
                dst = swpool.tile([F2, BT, P], BF16, tag=tg)
                for m in range(BT):
                    pt = pspool.tile([P, P], BF16, tag="ps")
                    nc.tensor.transpose(out=pt[0:F2, :], in_=src_sb[:, m, :],
                                        identity=ident[:])
                    nc.vector.tensor_copy(dst[:, m, :], pt[0:F2, :])
                return dst

            Qt = transp_small(Qm, "Qt")
            P1t = transp_small(P1, "P1t")
            relm = spool.tile([P, BT, B], BF16)
            relT = spool.tile([P, BT, B], BF16)
            for m in range(BT):
                ps = psbpool.tile([P, B], F32, tag="simps")
                nc.tensor.matmul(out=ps[:], lhsT=Qt[:, m, :],
                                 rhs=P1t[:].rearrange("p t c -> p (t c)"),
                                 start=True, stop=True)
                nc.vector.tensor_copy(relm[:, m, :], ps[:])
                ps2 = psbpool.tile([P, B], F32, tag="simps")
                nc.tensor.matmul(out=ps2[:], lhsT=P1t[:, m, :],
                                 rhs=Qt[:].rearrange("p t c -> p (t c)"),
                                 start=True, stop=True)
                nc.vector.tensor_copy(relT[:, m, :], ps2[:])

            # softmax(relu(x)): E = max(1, exp(x)); 1/rowsum scales embT cols
            Esb = spool.tile([P, SBT, 2 * B], BF16)
            Ssum = swpool.tile([P, SBT, 2], F32)
            for rt in range(SBT):
                if rt < BT:
                    left, right = L0r[:, rt, :], relm[:, rt, :]
                else:
                    left, right = relT[:, rt - BT, :], L1r[:, rt - BT, :]
                nc.scalar.activation(Esb[:, rt, 0:B], left, AF.Exp)
                nc.scalar.activation(Esb[:, rt, B:2 * B], right, AF.Exp)
                nc.vector.tensor_scalar(
                    out=Esb[:, rt, 0:B], in0=Esb[:, rt, 0:B], scalar1=1.0,
                    scalar2=None, op0=OP.max, op1=OP.add,
                    accum_out=Ssum[:, rt, 0:1])
                nc.vector.tensor_scalar(
                    out=Esb[:, rt, B:2 * B], in0=Esb[:, rt, B:2 * B],
                    scalar1=1.0, scalar2=None, op0=OP.max, op1=OP.add,
                    accum_out=Ssum[:, rt, 1:2])
            rsc = swpool.tile([P, SBT], F32)
            nc.vector.reduce_sum(rsc[:], Ssum[:], axis=mybir.AxisListType.X)
            nc.vector.reciprocal(rsc[:], rsc[:])

            embS = swpool.tile([P, SBT, F0], BF16)
            for kt in range(SBT):
                nc.scalar.activation(embS[:, kt, :], embTsb[:, kt, :],
                                     AF.Copy, scale=rsc[:, kt:kt + 1])

            with tc.tile_pool(name="pst", bufs=1, space="PSUM") as pstpool:
                Tps = pstpool.tile([P, 2 * B], F32, tag="Tps")
                for half in range(2):
                    mm_accum(Tps[:, half * B:(half + 1) * B],
                             [(embS[:, kt, :],
                               Esb[:, kt, half * B:(half + 1) * B])
                              for kt in range(SBT)])
                Tsb = swpool.tile([P, 2 * B], BF16)
                nc.vector.tensor_copy(Tsb[:], Tps[:])
            Tt = swpool.tile([P, SBT, P], BF16)
            for j in range(SBT):
                pt = pspool.tile([P, P], BF16, tag="ps")
                nc.tensor.transpose(out=pt[0:F0, :],
                                    in_=Tsb[:, j * P:(j + 1) * P],
                                    identity=ident[:])
                nc.vector.tensor_copy(Tt[:, j, :], pt[0:F0, :])
            M3ps = pspool.tile([P, DIM], F32, tag="ps")
            mm_accum(M3ps[:], [(Tt[:, kt, :], wsimsb[:, kt, :])
                               for kt in range(SBT)])
            nc.vector.tensor_copy(M3sb[:], M3ps[0:F0, :])

        # s_emb (own rows): accumulate all tiles into one PSUM, one copy out
        with tc.tile_pool(name="pse", bufs=1, space="PSUM") as psepool:
            sps = psepool.tile([P, tpc * DIM], F32, tag="sps")
            for j in range(tpc):
                nc.tensor.matmul(out=sps[:, j * DIM:(j + 1) * DIM],
                                 lhsT=xTownsb[:, j * P:(j + 1) * P],
                                 rhs=M3sb[:], start=True, stop=True)
            nc.vector.tensor_copy(
                s_all[:].rearrange("p t d -> p (t d)"), sps[:])

        with tc.tile_pool(name="rn", bufs=1) as rnpool:
            def renorm_write(src_all, fdim, col0, post_scale):
                sq = rnpool.tile([P, tpc, fdim], F32, tag=f"rn{col0}")
                nc.vector.tensor_mul(sq[:], src_all[:], src_all[:])
                s2 = rnpool.tile([P, tpc], F32, tag=f"rns{col0}")
                nc.vector.reduce_sum(s2[:], sq[:], axis=mybir.AxisListType.X)
                nc.scalar.activation(s2[:], s2[:], AF.Sqrt)
                nc.vector.tensor_scalar_add(s2[:], s2[:], 1e-7)
                nc.vector.reciprocal(s2[:], s2[:])
                if post_scale != 1.0:
                    nc.vector.tensor_scalar(
                        out=s2[:], in0=s2[:], scalar1=post_scale,
                        scalar2=post_scale, op0=OP.mult, op1=OP.min)
                else:
                    nc.vector.tensor_scalar_min(s2[:], s2[:], 1.0)
                nc.vector.tensor_tensor(
                    out=Tqown[:, :, col0:col0 + fdim], in0=src_all[:],
                    in1=_bc(s2[:], fdim), op=OP.mult)

            renorm_write(s_all, DIM, F2, math.sqrt(BETA))

            # --------------------------------------------- conv2 agg
            # gather ops are capped at RING idx; groups of GSZ chunks,
            # deep-buffered so desc-gen pipelines with transfers/compute
            with nc.named_scope("conv2"), (
                    tc.tile_pool(name="msg2", bufs=10)) as msg2pool, (
                    tc.tile_pool(name="oh2", bufs=3)) as oh2pool, (
                    tc.tile_pool(name="epi2", bufs=3)) as epi2pool:
                g2tiles = {}

                def fetch2(c0, gc):
                    mt = msg2pool.tile([P, GSZ, P], BF16, tag="msg")
                    nc.gpsimd.dma_gather(
                        out_ap=mt[:, 0:gc, :], in_ap=T2g,
                        idxs_ap=src16sb[:, c0 * 8:(c0 + gc) * 8],
                        num_idxs=gc * P, num_idxs_reg=gc * P, elem_size=P,
                        queue_num=next_q())
                    g2tiles[c0] = (gc, mt)

                for c0 in range(0, CT, GSZ):
                    fetch2(c0, min(GSZ, CT - c0))

                def chunk2(c):
                    c0 = c // GSZ * GSZ
                    gc, mt = g2tiles[c0]
                    return mt[:, c - c0, 0:F1]

                for b in range(tpc):
                    cbb = cb[b]
                    oh = build_onehot(b, oh2pool)
                    # agg.T directly: lhsT=msg rows, rhs=onehot
                    aps = psapool.tile([F1, P], F32, tag="agg")
                    for ci in range(cbb):
                        c = coff[b] + ci
                        nc.tensor.matmul(
                            out=aps[:], lhsT=chunk2(c),
                            rhs=oh[:, ci * P:(ci + 1) * P],
                            start=(ci == 0), stop=(ci == cbb - 1))
                    a2t = epi2pool.tile([F1, P], BF16, tag="e2a")
                    nc.vector.tensor_copy(a2t[:], aps[:])
                    hps = pspool.tile([P, F2], F32, tag="ps")
                    nc.tensor.matmul(out=hps[:], lhsT=a2t[:], rhs=W2sb[:],
                                     start=True, stop=True)
                    # g2 = relu(dinv*agg2 + b2); zero-bias: relu(dinv*agg2)
                    if zb:
                        nc.scalar.activation(g2_all[:, b, :], hps[:], AF.Relu,
                                             scale=dinv_own[:, b:b + 1])
                    else:
                        nc.scalar.activation(g2_all[:, b, :], hps[:], AF.Copy,
                                             scale=dinv_own[:, b:b + 1])
                        nc.vector.tensor_add(g2_all[:, b, :], g2_all[:, b, :],
                                             b2sb[:])
                        nc.vector.tensor_relu(g2_all[:, b, :],
                                              g2_all[:, b, :])

            renorm_write(g2_all, F2, 0, math.sqrt(ALPHA))

        with nc.named_scope("ag2"):
            nc.sync.dma_start(
                out=ag2_in[:].rearrange("(t p) f -> p t f", p=P),
                in_=Tqown[:])
            nc.gpsimd.collective_compute(
                "AllGather", OP.bypass, replica_groups=rg,
                ins=[ag2_in.opt()], outs=[ag2_out.opt()])

        # ------------------------------------------------- query phase
        SLC = min(512, QOP)
        nslice = QOP // SLC
        with nc.named_scope("query"), (
                tc.tile_pool(name="qg", bufs=8)) as qgpool, (
                tc.tile_pool(name="qw", bufs=3)) as qwpool, (
                tc.tile_pool(name="psq", bufs=2, space="PSUM")) as psqpool, (
                tc.tile_pool(name="psz", bufs=1, space="PSUM")) as pszpool:
            zps = pszpool.tile([P, nzcol], F32)
            for op_i in range(nqops):
                qt = qgpool.tile([P, 1, 2 * QOP], BF16, tag="qga")
                i0 = op_i * (2 * QOP // 16)
                nc.gpsimd.dma_gather(
                    out_ap=qt[:], in_ap=TQ,
                    idxs_ap=qidxsb[:, i0:i0 + 2 * QOP // 16],
                    num_idxs=2 * QOP, num_idxs_reg=2 * QOP, elem_size=P,
                    transpose=True, queue_num=next_q())
                dd = qwpool.tile([FQ, QOP], BF16, tag="qd")
                nc.vector.tensor_sub(dd[:], qt[0:FQ, 0, 0:QOP],
                                     qt[0:FQ, 0, QOP:2 * QOP])
                sq = qwpool.tile([FQ, QOP], BF16, tag="qsq")
                nc.scalar.activation(sq[:], dd[:], AF.Square)
                for s in range(nslice):
                    hps = psqpool.tile([F2, SLC], F32, tag="qps")
                    nc.tensor.matmul(out=hps[:], lhsT=lin1Wsb[:],
                                     rhs=sq[:, s * SLC:(s + 1) * SLC],
                                     start=True, stop=True)
                    hq = qwpool.tile([F2, SLC], BF16, tag="qhq")
                    if not zb:
                        nc.vector.tensor_tensor(out=hps[:], in0=hps[:],
                                                in1=_bc(lin1bcol[:], SLC),
                                                op=OP.add)
                    tmp = qwpool.tile([F2, SLC], F32, tag="qtmp")
                    nc.vector.tensor_scalar_mul(tmp[:], hps[:], LEAKY_SLOPE)
                    nc.vector.tensor_max(hq[:], hps[:], tmp[:])
                    c0 = op_i * (QOP // P) + s * (SLC // P)
                    for t in range(SLC // P):
                        nc.tensor.matmul(
                            out=zps[:, c0 + t:c0 + t + 1],
                            lhsT=hq[:, t * P:(t + 1) * P],
                            rhs=linWsb[:], start=True, stop=True)

            za = ppool.tile([P, nzcol], F32)
            two = cpool.tile([P, 1], F32)
            nc.vector.memset(two[:], 2.0)
            nc.scalar.activation(za[:], zps[:], AF.Abs, bias=g["lin_b"])
            nc.vector.tensor_scalar_min(za[:], za[:], CLAMP_MAX)
            nc.scalar.activation(za[:], za[:], AF.Sigmoid, bias=two[:],
                                 scale=-1.0)
            nc.sync.dma_start(out=out_d.ap().rearrange("j p -> p j"),
                              in_=za[:])

    nc.compile()
    return nc


# ----------------------------------------------------------------------------
# entry point
# ----------------------------------------------------------------------------

def kernel(**inputs):
    geom, in_maps = build_host(inputs, NCORES)
    nc = build_nc(geom)
    res = bass_utils.run_bass_kernel_spmd(
        nc, in_maps, core_ids=list(range(NCORES)))
    outs = []
    for k in range(NCORES):
        o = np.asarray(res.results[k]["out"], np.float32).reshape(-1)
        lo = k * geom["nqc"]
        hi = min((k + 1) * geom["nqc"], geom["NQ"])
        outs.append(o[: hi - lo])
    return np.concatenate(outs).astype(np.float32)


# revision 19
# speedup vs baseline: 1.4912x; 1.1270x over previous
"""Trainium2 Bass kernel for nn_BlockNet (GNN message passing + block-sim MLP).

Strategy (8 NeuronCores, SPMD, single NEFF):
  - GCN aggregation sharded by destination-node tile ranges (tpc x 128-node
    tiles per core).  Edges sorted by dst on host; segment-sum done as one-hot
    matmuls accumulating in PSUM per dst block.  Self loops kept as edges.
  - conv1 needs no device gather: host stages edge-ordered rows
    xe = (x * dinv)[src]; kernel computes (A @ xe) @ W1 per dst block.
  - conv2 messages fetched with per-dst-block dma_gather ops (2304 idx each,
    256B rows) from the Shared AllGather'd node table; SWDGE queues
    round-robin, deep-buffered so descriptor-gen pipelines with transfers.
  - Degree normalization folded into scalar-engine epilogues (biases are
    zero in this problem -- verified on host, generic fallback otherwise).
  - (x @ emb_sim) @ sim_block @ w_sim collapsed to x @ M3 with
    M3 = emb_sim @ sim_block @ w_sim; the BxB block-sim math is replicated
    per core in bf16.
  - Two Shared-output AllGathers exchange (1) the conv1 row table and
    (2) the final 40-wide node feature table.
  - Query phase: data-parallel over query edges; ONE transposed dma_gather
    per 2048 queries fetches both endpoints (4096 idx); MLP runs on
    512-query slices with biases folded into scalar activations.

kernel(**inputs) takes full unsharded inputs, returns the full [NQ] f32
output.
"""

import math
import os
import sys

import numpy as np

for _p in ("/opt/trn_rl_repo", "/root/.axon_site/_ro/trn_rl_repo"):
    if os.path.isdir(_p) and _p not in sys.path:
        sys.path.insert(0, _p)

import concourse.bass as bass
import concourse.bacc as bacc
import concourse.mybir as mybir
import concourse.tile as tile
from concourse import bass_utils
from concourse.masks import make_identity

BF16 = mybir.dt.bfloat16
F32 = mybir.dt.float32
I16 = mybir.dt.int16
I32 = mybir.dt.int32
NP_BF16 = mybir.dt.np(BF16)

P = 128
NCORES = 8
LEAKY_SLOPE = 0.2
ALPHA, BETA = 1.0, 0.1
CLAMP_MAX = 40.0

AF = mybir.ActivationFunctionType
OP = mybir.AluOpType

NQUEUES = 4       # SWDGE queues; gathers round-robin
# SWDGE descriptor-ring capacity is dynamic_dma_scratch_size/16 per queue;
# a single dma_gather's num_idxs must stay below it.
DMA_SCRATCH = int(os.environ.get("DMA_SCRATCH", "16384"))
RING = DMA_SCRATCH // 16
# queries per combined (i0|i1) transposed gather op (2*QOP idx per op)
QOP = int(os.environ.get("QOP", "256"))
GSZ = RING // P   # conv2 message chunks per gather op


def _bc(ap, n):
    """Append a stride-0 broadcast inner dim of size n to an AP."""
    return bass.AP(ap.tensor, ap.offset, list(ap.ap) + [[0, n]])


# ----------------------------------------------------------------------------
# host-side data prep
# ----------------------------------------------------------------------------

def _wrap16(idx):
    """int16 index array in dma_gather wrapped layout [128, n/16]."""
    idx = np.asarray(idx, np.int64)
    n = idx.shape[0]
    assert n % 16 == 0
    w = idx.reshape(n // 16, 16).T.astype(np.int16)        # [16, n/16]
    return np.ascontiguousarray(np.tile(w, (8, 1)))         # [128, n/16]


def build_host(inputs, ncores=NCORES):
    x = np.asarray(inputs["x"], np.float32)
    L0 = np.asarray(inputs["L0"], np.float32)
    L1 = np.asarray(inputs["L1"], np.float32)
    ei = np.asarray(inputs["edge_index"]).astype(np.int64)
    te = np.asarray(inputs["total_edges"]).astype(np.int64)
    W1 = np.asarray(inputs["conv1_W"], np.float32)
    b1 = np.asarray(inputs["conv1_b"], np.float32)
    W2 = np.asarray(inputs["conv2_W"], np.float32)
    b2 = np.asarray(inputs["conv2_b"], np.float32)
    w_sim = np.asarray(inputs["weights_sim"], np.float32)
    emb_sim = np.asarray(inputs["embeddings_sim"], np.float32)
    w_od = np.asarray(inputs["weights_off_diagonal"], np.float32)
    wL0 = np.asarray(inputs["weights_L_0"], np.float32)
    wL1 = np.asarray(inputs["weights_L_1"], np.float32)
    lin1_W = np.asarray(inputs["lin1_W"], np.float32)
    lin1_b = np.asarray(inputs["lin1_b"], np.float32)
    lin_W = np.asarray(inputs["lin_W"], np.float32)
    lin_b = np.asarray(inputs["lin_b"], np.float32)

    N, F0 = x.shape
    F1 = W1.shape[1]
    F2 = W2.shape[1]
    DIM = w_sim.shape[1]
    B = L0.shape[0]
    NQ = te.shape[0]

    # zero-bias fast path (true for this problem; checked, not assumed)
    zb = (not b1.any()) and (not b2.any()) and (not lin1_b.any())

    tpc = math.ceil(math.ceil(N / P) / ncores)
    ntiles = ncores * tpc
    nslot = ntiles * P

    src, dst = ei[0], ei[1]
    deg = (np.bincount(dst, minlength=N) + 1).astype(np.float32)
    dinv = (1.0 / np.sqrt(deg)).astype(np.float32)
    xs = x * dinv[:, None]                                  # x~ = dinv * x

    loops = np.arange(N, dtype=np.int64)
    src_s = np.concatenate([src, loops])
    dst_s = np.concatenate([dst, loops])
    order = np.argsort(dst_s, kind="stable")
    src_s, dst_s = src_s[order], dst_s[order]

    blk_of = dst_s // P
    counts = np.bincount(blk_of, minlength=ntiles)
    starts = np.concatenate([[0], np.cumsum(counts)])
    cb = []
    for b in range(tpc):
        mx = max(int(counts[k * tpc + b]) for k in range(ncores))
        cb.append(max(1, math.ceil(mx / P)))
    CT = int(sum(cb))
    EPAD = CT * P

    src16_cores, dstloc_cores, xe_cores = [], [], []
    # padding slots gather irrelevant data (their one-hot rows are all zero)
    # but MUST spread across the node table: thousands of same-address
    # gathers serialize in the DMA path (cost core 7 ~100us of skew).
    spread = (np.arange(EPAD, dtype=np.int64) * 97) % N
    for k in range(ncores):
        s_pad = spread.copy()
        d_pad = np.full(EPAD, -1.0, np.float32)
        off = 0
        for b in range(tpc):
            t = k * tpc + b
            e0, e1 = int(starts[t]), int(starts[t + 1])
            cnt = e1 - e0
            s_pad[off : off + cnt] = src_s[e0:e1]
            d_pad[off : off + cnt] = (dst_s[e0:e1] - t * P).astype(np.float32)
            off += cb[b] * P
        src16_cores.append(_wrap16(s_pad))
        dstloc_cores.append(
            np.ascontiguousarray(d_pad.reshape(CT, P).T.astype(NP_BF16)))
        # edge-ordered x~ rows: edge e at [e%128, e//128, :]
        xe = xs[s_pad].reshape(CT, P, F0).transpose(1, 0, 2)
        xe_cores.append(np.ascontiguousarray(xe.astype(NP_BF16)))

    # queries: pad each core's slice to a multiple of QOP; per-op combined
    # index list [i0 (QOP) | i1 (QOP)]
    nqc = math.ceil(NQ / ncores)
    nqcp = math.ceil(nqc / QOP) * QOP
    nqops = nqcp // QOP
    qidx_cores = []
    qspread = (np.arange(nqcp, dtype=np.int64) * 89) % N
    for k in range(ncores):
        q = np.stack([qspread, qspread], axis=1)
        chunk = te[k * nqc : min((k + 1) * nqc, NQ)]
        q[: chunk.shape[0]] = chunk
        per_op = q.reshape(nqops, QOP, 2)
        comb = np.concatenate([per_op[:, :, 0], per_op[:, :, 1]],
                              axis=1).reshape(-1)            # [nqops*2*QOP]
        qidx_cores.append(_wrap16(comb))

    # per-core per-partition 1/sqrt(deg) and 1/deg for own tiles
    dinv_all = np.ones((nslot,), np.float32)
    dinv_all[:N] = dinv
    dinv_all = dinv_all.reshape(ntiles, P).T                 # [P, ntiles]

    shared = {
        "W1": W1.astype(NP_BF16),
        "W2": W2.astype(NP_BF16),
        "L0b": L0.astype(NP_BF16),
        "L0Tb": np.ascontiguousarray(L0.T).astype(NP_BF16),
        "L1b": L1.astype(NP_BF16),
        "L1Tb": np.ascontiguousarray(L1.T).astype(NP_BF16),
        "wodT": np.ascontiguousarray(w_od.T).astype(NP_BF16),
        "wL0": wL0.astype(NP_BF16),
        "wL1": wL1.astype(NP_BF16),
        "embT": np.ascontiguousarray(emb_sim.T).astype(NP_BF16),
        "wsim": w_sim.astype(NP_BF16),
        "lin1Wb": lin1_W.astype(NP_BF16),
        "linWb": lin_W.astype(NP_BF16),
        "lin1bcol": np.ascontiguousarray(lin1_b[:, None]).astype(np.float32),
    }
    if not zb:
        shared["b1bc"] = np.ascontiguousarray(
            np.tile(b1, (P, 1)).astype(np.float32))
        shared["b2bc"] = np.ascontiguousarray(
            np.tile(b2, (P, 1)).astype(np.float32))

    in_maps = []
    for k in range(ncores):
        m = dict(shared)
        m["xe"] = xe_cores[k].reshape(P, CT * F0)
        m["src16"] = src16_cores[k]
        m["dstloc"] = dstloc_cores[k]
        m["qidx16"] = qidx_cores[k]
        dv = dinv_all[:, k * tpc:(k + 1) * tpc]
        m["dinv_own"] = np.ascontiguousarray(dv)
        m["dinv2_own"] = np.ascontiguousarray(dv * dv)
        xo = np.zeros((P, tpc * P), np.float32)
        lo, hi = k * tpc * P, min((k + 1) * tpc * P, N)
        if hi > lo:
            xo[:, : hi - lo] = x[lo:hi].T
        m["xTown"] = xo.astype(NP_BF16)
        in_maps.append(m)

    geom = dict(
        N=N, F0=F0, F1=F1, F2=F2, DIM=DIM, B=B, BT=B // P, SBT=2 * B // P,
        NQ=NQ, tpc=tpc, ntiles=ntiles, nslot=nslot,
        cb=cb, CT=CT, nqc=nqc, nqcp=nqcp, nqops=nqops, ncores=ncores,
        zb=zb, lin_b=float(lin_b[0]),
    )
    return geom, in_maps


# ----------------------------------------------------------------------------
# device kernel
# ----------------------------------------------------------------------------

def build_nc(g):
    tpc, ntiles, nslot = g["tpc"], g["ntiles"], g["nslot"]
    F0, F1, F2, DIM = g["F0"], g["F1"], g["F2"], g["DIM"]
    B, BT, SBT = g["B"], g["BT"], g["SBT"]
    cb, CT = g["cb"], g["CT"]
    nqops = g["nqops"]
    ncores = g["ncores"]
    zb = g["zb"]
    cmax = max(cb)
    rg = [list(range(ncores))]
    FQ = F2 + DIM
    nzcol = nqops * (QOP // P)        # output cols in zps per... per op: QOP/128

    coff = [0]
    for c in cb:
        coff.append(coff[-1] + c)

    nc = bacc.Bacc("TRN2", target_bir_lowering=False, debug=False,
                   num_devices=ncores, num_swdge_queues=NQUEUES,
                   dynamic_dma_scratch_size=DMA_SCRATCH)
    qrr = [0]

    def next_q():
        q = qrr[0] % NQUEUES
        qrr[0] += 1
        return q

    def din(name, shape, dt):
        return nc.dram_tensor(name, shape, dt, kind="ExternalInput")

    xe_d = din("xe", [P, CT * F0], BF16)
    W1_d = din("W1", [F0, F1], BF16)
    W2_d = din("W2", [F1, F2], BF16)
    dinv_own_d = din("dinv_own", [P, tpc], F32)
    dinv2_own_d = din("dinv2_own", [P, tpc], F32)
    xTown_d = din("xTown", [P, tpc * P], BF16)
    src16_d = din("src16", [P, CT * 8], I16)
    dstloc_d = din("dstloc", [P, CT], BF16)
    qidx_d = din("qidx16", [P, nqops * 2 * QOP // 16], I16)
    L0b_d = din("L0b", [B, B], BF16)
    L0Tb_d = din("L0Tb", [B, B], BF16)
    L1b_d = din("L1b", [B, B], BF16)
    L1Tb_d = din("L1Tb", [B, B], BF16)
    wodT_d = din("wodT", [B, B], BF16)
    wL0_d = din("wL0", [B, F2], BF16)
    wL1_d = din("wL1", [B, F2], BF16)
    embT_d = din("embT", [2 * B, F0], BF16)
    wsim_d = din("wsim", [2 * B, DIM], BF16)
    lin1Wb_d = din("lin1Wb", [FQ, F2], BF16)
    linWb_d = din("linWb", [F2, 1], BF16)
    lin1bcol_d = din("lin1bcol", [F2, 1], F32)
    if not zb:
        b1bc_d = din("b1bc", [P, F1], F32)
        b2bc_d = din("b2bc", [P, F2], F32)
    out_d = nc.dram_tensor("out", [g["nqcp"] // P, P], F32,
                           kind="ExternalOutput")

    with tile.TileContext(nc) as tc, (
        tc.tile_pool(name="const", bufs=1)) as cpool, (
        tc.tile_pool(name="persist", bufs=1)) as ppool, (
        tc.tile_pool(name="ps", bufs=2, space="PSUM")) as pspool, (
        tc.tile_pool(name="psagg", bufs=2, space="PSUM")) as psapool, (
        tc.tile_pool(name="dram", bufs=1, space="DRAM")) as dpool:

        # Warm up the collective path first: the first collective of a NEFF
        # pays ~57us of CC cold-start (trigger -> ALGO_MESH_BEGIN); a dummy
        # AllGather absorbs it while conv1 runs.
        with nc.named_scope("agwarm"):
            warm_in = dpool.tile([16, 16], F32)
            warm_out = dpool.tile([ncores, 16, 16], F32)
            warm_sb = cpool.tile([16, 16], F32)
            nc.vector.memset(warm_sb[:], 0.0)
            nc.sync.dma_start(out=warm_in[:], in_=warm_sb[:])
            nc.gpsimd.collective_compute(
                "AllGather", OP.bypass, replica_groups=rg,
                ins=[warm_in.opt()], outs=[warm_out.opt()])

        # ------------------------------------------------- constants / loads
        ident = cpool.tile([P, P], BF16)
        make_identity(nc, ident[:])

        iota_b = cpool.tile([P, cmax * P], BF16)
        with tc.tile_pool(name="iotatmp", bufs=1) as itpool:
            iota_i = itpool.tile([P, cmax * P], I32)
            nc.gpsimd.iota(iota_i[:], pattern=[[0, cmax], [1, P]], base=0,
                           channel_multiplier=0)
            nc.vector.tensor_copy(iota_b[:], iota_i[:])

        def load(pool, dram_t, shape, dt=BF16, rearr=None):
            t = pool.tile(shape, dt, tag="ld_" + dram_t.name)
            src = dram_t.ap()
            if rearr is not None:
                src = src.rearrange(rearr, p=P)
            nc.sync.dma_start(out=t[:], in_=src)
            return t

        W1sb = load(cpool, W1_d, [F0, F1])
        W2sb = load(cpool, W2_d, [F1, F2])
        dinv_own = load(cpool, dinv_own_d, [P, tpc], F32)
        dinv2_own = load(cpool, dinv2_own_d, [P, tpc], F32)
        src16sb = load(cpool, src16_d, [P, CT * 8], I16)
        dstlocsb = load(cpool, dstloc_d, [P, CT])
        qidxsb = load(cpool, qidx_d, [P, nqops * 2 * QOP // 16], I16)
        lin1Wsb = load(cpool, lin1Wb_d, [FQ, F2])
        linWsb = load(cpool, linWb_d, [F2, 1])
        lin1bcol = load(cpool, lin1bcol_d, [F2, 1], F32)
        xTownsb = load(cpool, xTown_d, [P, tpc * P])
        if not zb:
            b1sb = load(cpool, b1bc_d, [P, F1], F32)
            b2sb = load(cpool, b2bc_d, [P, F2], F32)

        ag_shared = os.environ.get("AG_SHARED", "1") == "1"
        ag_space = "Shared" if ag_shared else "Local"
        ag1_in = dpool.tile([tpc * P, P], BF16)
        ag1_out = dpool.tile([ncores, tpc * P, P], BF16, addr_space=ag_space)
        ag2_in = dpool.tile([tpc * P, P], BF16)
        ag2_out = dpool.tile([ncores, tpc * P, P], BF16, addr_space=ag_space)
        T2g = ag1_out[:].rearrange("r n f -> (r n) f")
        TQ = ag2_out[:].rearrange("r n f -> (r n) f")

        g1rows = ppool.tile([P, tpc, P], BF16)
        g2_all = ppool.tile([P, tpc, F2], F32)
        s_all = ppool.tile([P, tpc, DIM], F32)
        Tqown = ppool.tile([P, tpc, P], BF16)
        nc.vector.memset(Tqown[:], 0.0)
        nc.vector.memset(g1rows[:], 0.0)
        M3sb = ppool.tile([F0, DIM], BF16)

        def build_onehot(b, ohpool):
            cbb = cb[b]
            oh = ohpool.tile([P, cmax * P], BF16, tag="oh")
            nc.vector.tensor_tensor(
                out=oh[:, 0:cbb * P].rearrange("p (c e) -> p c e", e=P),
                in0=iota_b[:, 0:cbb * P].rearrange("p (c e) -> p c e", e=P),
                in1=_bc(dstlocsb[:, coff[b]:coff[b] + cbb], P),
                op=OP.is_equal)
            return oh

        # ------------------------------------------------ conv1 (gather-free)
        with nc.named_scope("conv1"), (
                tc.tile_pool(name="msg1", bufs=3)) as msgpool, (
                tc.tile_pool(name="oh1", bufs=3)) as ohpool, (
                tc.tile_pool(name="epi1", bufs=3)) as epipool:
            for b in range(tpc):
                cbb = cb[b]
                mt = msgpool.tile([P, cmax, F0], BF16, tag="msg")
                nc.sync.dma_start(
                    out=mt[:, 0:cbb, :],
                    in_=xe_d.ap().rearrange(
                        "p (c f) -> p c f", f=F0)[:, coff[b]:coff[b] + cbb, :])
                oh = build_onehot(b, ohpool)
                xps = psapool.tile([P, P], F32, tag="agg")
                for ci in range(cbb):
                    nc.tensor.matmul(
                        out=xps[:], lhsT=mt[:, ci, :],
                        rhs=oh[:, ci * P:(ci + 1) * P],
                        start=(ci == 0), stop=(ci == cbb - 1))
                xaggt = epipool.tile([P, P], BF16, tag="xaggt")
                nc.vector.tensor_copy(xaggt[:], xps[:])
                hps = pspool.tile([P, F1], F32, tag="ps")
                nc.tensor.matmul(out=hps[:], lhsT=xaggt[:], rhs=W1sb[:],
                                 start=True, stop=True)
                # g1s = dinv*relu(dinv*agg + b1); zero-bias: dinv2*relu(agg)
                if zb:
                    nc.scalar.activation(g1rows[:, b, 0:F1], hps[:], AF.Relu,
                                         scale=dinv2_own[:, b:b + 1])
                else:
                    ta = epipool.tile([P, F1], F32, tag="epi1a")
                    nc.scalar.activation(ta[:], hps[:], AF.Copy,
                                         scale=dinv_own[:, b:b + 1])
                    nc.vector.tensor_add(ta[:], ta[:], b1sb[:])
                    nc.vector.tensor_relu(ta[:], ta[:])
                    nc.scalar.activation(g1rows[:, b, 0:F1], ta[:], AF.Copy,
                                         scale=dinv_own[:, b:b + 1])
                # stage this block's rows for the AllGather right away so the
                # collective can trigger as soon as the last block lands
                nc.sync.dma_start(
                    out=ag1_in[:].rearrange(
                        "(t p) f -> p t f", p=P)[:, b:b + 1, :],
                    in_=g1rows[:, b:b + 1, :])

        with nc.named_scope("ag1"):
            nc.gpsimd.collective_compute(
                "AllGather", OP.bypass, replica_groups=rg,
                ins=[ag1_in.opt()], outs=[ag1_out.opt()])

        # conv2 message/onehot pools are opened BEFORE the sim pools so their
        # SBUF ranges don't overlap sim tiles -- otherwise the conv2 gathers
        # stall until the last sim-tile read vacates the space (~35us).
        conv2_pools = (
            tc.tile_pool(name="msg2", bufs=10),
            tc.tile_pool(name="oh2", bufs=3),
            tc.tile_pool(name="epi2", bufs=3),
        )
        msg2pool = conv2_pools[0].__enter__()
        oh2pool = conv2_pools[1].__enter__()
        epi2pool = conv2_pools[2].__enter__()

        # ------------------------------------------------- sim block (bf16)
        with nc.named_scope("sim"), (
                tc.tile_pool(name="sim", bufs=1)) as spool, (
                tc.tile_pool(name="simw", bufs=1)) as swpool, (
                tc.tile_pool(name="pssim", bufs=2, space="PSUM")) as psbpool:
            L0sb = load(spool, L0b_d, [P, BT, B], rearr="(t p) c -> p t c")
            L0Tsb = load(spool, L0Tb_d, [P, BT, B], rearr="(t p) c -> p t c")
            L1sb = load(spool, L1b_d, [P, BT, B], rearr="(t p) c -> p t c")
            L1Tsb = load(spool, L1Tb_d, [P, BT, B], rearr="(t p) c -> p t c")
            wodTsb = load(spool, wodT_d, [P, BT, B], rearr="(t p) c -> p t c")
            wL0sb = load(spool, wL0_d, [P, BT, F2], rearr="(t p) c -> p t c")
            wL1sb = load(spool, wL1_d, [P, BT, F2], rearr="(t p) c -> p t c")
            embTsb = load(spool, embT_d, [P, SBT, F0],
                          rearr="(t p) c -> p t c")
            wsimsb = load(spool, wsim_d, [P, SBT, DIM],
                          rearr="(t p) c -> p t c")

            def mm_accum(out_ap, pairs):
                for i, (lhsT, rhs) in enumerate(pairs):
                    nc.tensor.matmul(out=out_ap, lhsT=lhsT, rhs=rhs,
                                     start=(i == 0),
                                     stop=(i == len(pairs) - 1))

            def big_mm(dst_sb, lhsT_tiles, rhs_tiles, nf):
                for m in range(BT):
                    ps = psbpool.tile([P, nf], F32, tag="simps")
                    mm_accum(ps[:], [(lhsT_tiles(k, m), rhs_tiles(k))
                                     for k in range(BT)])
                    nc.vector.tensor_copy(dst_sb[:, m, :], ps[:])

            L0r = spool.tile([P, BT, B], BF16)
            L0rT = spool.tile([P, BT, B], BF16)
            L1r = spool.tile([P, BT, B], BF16)
            L1rT = spool.tile([P, BT, B], BF16)
            big_mm(L0r, lambda k, m: L0Tsb[:, k, m * P:(m + 1) * P],
                   lambda k: L0sb[:, k, :], B)
            big_mm(L0rT, lambda k, m: L0sb[:, k, m * P:(m + 1) * P],
                   lambda k: L0Tsb[:, k, :], B)
            big_mm(L1r, lambda k, m: L1Tsb[:, k, m * P:(m + 1) * P],
                   lambda k: L1sb[:, k, :], B)
            big_mm(L1rT, lambda k, m: L1sb[:, k, m * P:(m + 1) * P],
                   lambda k: L1Tsb[:, k, :], B)

            P0 = swpool.tile([P, BT, F2], BF16)
            P1 = swpool.tile([P, BT, F2], BF16)
            Qm = swpool.tile([P, BT, F2], BF16)
            big_mm(P0, lambda k, m: L0rT[:, k, m * P:(m + 1) * P],
                   lambda k: wL0sb[:, k, :], F2)
            big_mm(P1, lambda k, m: L1rT[:, k, m * P:(m + 1) * P],
                   lambda k: wL1sb[:, k, :], F2)
            big_mm(Qm, lambda k, m: wodTsb[:, k, m * P:(m + 1) * P],
                   lambda k: P0[:, k, :], F2)

            def transp_small(src_sb, tg):
                dst = swpool.tile([F2, BT, P], BF16, tag=tg)
                for m in range(BT):
                    pt = pspool.tile([P, P], BF16, tag="ps")
                    nc.tensor.transpose(out=pt[0:F2, :], in_=src_sb[:, m, :],
                                        identity=ident[:])
                    nc.vector.tensor_copy(dst[:, m, :], pt[0:F2, :])
                return dst

            Qt = transp_small(Qm, "Qt")
            P1t = transp_small(P1, "P1t")
            relm = spool.tile([P, BT, B], BF16)
            relT = spool.tile([P, BT, B], BF16)
            for m in range(BT):
                ps = psbpool.tile([P, B], F32, tag="simps")
                nc.tensor.matmul(out=ps[:], lhsT=Qt[:, m, :],
                                 rhs=P1t[:].rearrange("p t c -> p (t c)"),
                                 start=True, stop=True)
                nc.vector.tensor_copy(relm[:, m, :], ps[:])
                ps2 = psbpool.tile([P, B], F32, tag="simps")
                nc.tensor.matmul(out=ps2[:], lhsT=P1t[:, m, :],
                                 rhs=Qt[:].rearrange("p t c -> p (t c)"),
                                 start=True, stop=True)
                nc.vector.tensor_copy(relT[:, m, :], ps2[:])

            # softmax(relu(x)): E = max(1, exp(x)); 1/rowsum scales embT cols
            Esb = spool.tile([P, SBT, 2 * B], BF16)
            Ssum = swpool.tile([P, SBT, 2], F32)
            for rt in range(SBT):
                if rt < BT:
                    left, right = L0r[:, rt, :], relm[:, rt, :]
                else:
                    left, right = relT[:, rt - BT, :], L1r[:, rt - BT, :]
                nc.scalar.activation(Esb[:, rt, 0:B], left, AF.Exp)
                nc.scalar.activation(Esb[:, rt, B:2 * B], right, AF.Exp)
                nc.vector.tensor_scalar(
                    out=Esb[:, rt, 0:B], in0=Esb[:, rt, 0:B], scalar1=1.0,
                    scalar2=None, op0=OP.max, op1=OP.add,
                    accum_out=Ssum[:, rt, 0:1])
                nc.vector.tensor_scalar(
                    out=Esb[:, rt, B:2 * B], in0=Esb[:, rt, B:2 * B],
                    scalar1=1.0, scalar2=None, op0=OP.max, op1=OP.add,
                    accum_out=Ssum[:, rt, 1:2])
            rsc = swpool.tile([P, SBT], F32)
            nc.vector.reduce_sum(rsc[:], Ssum[:], axis=mybir.AxisListType.X)
            nc.vector.reciprocal(rsc[:], rsc[:])

            embS = swpool.tile([P, SBT, F0], BF16)
            for kt in range(SBT):
                nc.scalar.activation(embS[:, kt, :], embTsb[:, kt, :],
                                     AF.Copy, scale=rsc[:, kt:kt + 1])

            with tc.tile_pool(name="pst", bufs=1, space="PSUM") as pstpool:
                Tps = pstpool.tile([P, 2 * B], F32, tag="Tps")
                for half in range(2):
                    mm_accum(Tps[:, half * B:(half + 1) * B],
                             [(embS[:, kt, :],
                               Esb[:, kt, half * B:(half + 1) * B])
                              for kt in range(SBT)])
                Tsb = swpool.tile([P, 2 * B], BF16)
                nc.vector.tensor_copy(Tsb[:], Tps[:])
            Tt = swpool.tile([P, SBT, P], BF16)
            for j in range(SBT):
                pt = pspool.tile([P, P], BF16, tag="ps")
                nc.tensor.transpose(out=pt[0:F0, :],
                                    in_=Tsb[:, j * P:(j + 1) * P],
                                    identity=ident[:])
                nc.vector.tensor_copy(Tt[:, j, :], pt[0:F0, :])
            M3ps = pspool.tile([P, DIM], F32, tag="ps")
            mm_accum(M3ps[:], [(Tt[:, kt, :], wsimsb[:, kt, :])
                               for kt in range(SBT)])
            nc.vector.tensor_copy(M3sb[:], M3ps[0:F0, :])

        # s_emb (own rows): accumulate all tiles into one PSUM, one copy out
        with tc.tile_pool(name="pse", bufs=1, space="PSUM") as psepool:
            sps = psepool.tile([P, tpc * DIM], F32, tag="sps")
            for j in range(tpc):
                nc.tensor.matmul(out=sps[:, j * DIM:(j + 1) * DIM],
                                 lhsT=xTownsb[:, j * P:(j + 1) * P],
                                 rhs=M3sb[:], start=True, stop=True)
            nc.vector.tensor_copy(
                s_all[:].rearrange("p t d -> p (t d)"), sps[:])

        with tc.tile_pool(name="rn", bufs=1) as rnpool:
            def renorm_write(src_all, fdim, col0, post_scale):
                sq = rnpool.tile([P, tpc, fdim], F32, tag=f"rn{col0}")
                nc.vector.tensor_mul(sq[:], src_all[:], src_all[:])
                s2 = rnpool.tile([P, tpc], F32, tag=f"rns{col0}")
                nc.vector.reduce_sum(s2[:], sq[:], axis=mybir.AxisListType.X)
                nc.scalar.activation(s2[:], s2[:], AF.Sqrt)
                nc.vector.tensor_scalar_add(s2[:], s2[:], 1e-7)
                nc.vector.reciprocal(s2[:], s2[:])
                if post_scale != 1.0:
                    nc.vector.tensor_scalar(
                        out=s2[:], in0=s2[:], scalar1=post_scale,
                        scalar2=post_scale, op0=OP.mult, op1=OP.min)
                else:
                    nc.vector.tensor_scalar_min(s2[:], s2[:], 1.0)
                nc.vector.tensor_tensor(
                    out=Tqown[:, :, col0:col0 + fdim], in0=src_all[:],
                    in1=_bc(s2[:], fdim), op=OP.mult)

            renorm_write(s_all, DIM, F2, math.sqrt(BETA))

            # --------------------------------------------- conv2 agg
            # gather ops are capped at RING idx; groups of GSZ chunks,
            # deep-buffered so desc-gen pipelines with transfers/compute
            with nc.named_scope("conv2"):
                g2tiles = {}

                def fetch2(c0, gc):
                    mt = msg2pool.tile([P, GSZ, P], BF16, tag="msg")
                    nc.gpsimd.dma_gather(
                        out_ap=mt[:, 0:gc, :], in_ap=T2g,
                        idxs_ap=src16sb[:, c0 * 8:(c0 + gc) * 8],
                        num_idxs=gc * P, num_idxs_reg=gc * P, elem_size=P,
                        queue_num=next_q())
                    g2tiles[c0] = (gc, mt)

                for c0 in range(0, CT, GSZ):
                    fetch2(c0, min(GSZ, CT - c0))

                def chunk2(c):
                    c0 = c // GSZ * GSZ
                    gc, mt = g2tiles[c0]
                    return mt[:, c - c0, 0:F1]

                for b in range(tpc):
                    cbb = cb[b]
                    oh = build_onehot(b, oh2pool)
                    # agg.T directly: lhsT=msg rows, rhs=onehot
                    aps = psapool.tile([F1, P], F32, tag="agg")
                    for ci in range(cbb):
                        c = coff[b] + ci
                        nc.tensor.matmul(
                            out=aps[:], lhsT=chunk2(c),
                            rhs=oh[:, ci * P:(ci + 1) * P],
                            start=(ci == 0), stop=(ci == cbb - 1))
                    a2t = epi2pool.tile([F1, P], BF16, tag="e2a")
                    nc.vector.tensor_copy(a2t[:], aps[:])
                    hps = pspool.tile([P, F2], F32, tag="ps")
                    nc.tensor.matmul(out=hps[:], lhsT=a2t[:], rhs=W2sb[:],
                                     start=True, stop=True)
                    # g2 = relu(dinv*agg2 + b2); zero-bias: relu(dinv*agg2)
                    if zb:
                        nc.scalar.activation(g2_all[:, b, :], hps[:], AF.Relu,
                                             scale=dinv_own[:, b:b + 1])
                    else:
                        nc.scalar.activation(g2_all[:, b, :], hps[:], AF.Copy,
                                             scale=dinv_own[:, b:b + 1])
                        nc.vector.tensor_add(g2_all[:, b, :], g2_all[:, b, :],
                                             b2sb[:])
                        nc.vector.tensor_relu(g2_all[:, b, :],
                                              g2_all[:, b, :])

            renorm_write(g2_all, F2, 0, math.sqrt(ALPHA))

        for cp in reversed(conv2_pools):
            cp.__exit__(None, None, None)

        with nc.named_scope("ag2"):
            nc.sync.dma_start(
                out=ag2_in[:].rearrange("(t p) f -> p t f", p=P),
                in_=Tqown[:])
            nc.gpsimd.collective_compute(
                "AllGather", OP.bypass, replica_groups=rg,
                ins=[ag2_in.opt()], outs=[ag2_out.opt()])

        # ------------------------------------------------- query phase
        SLC = min(512, QOP)
        nslice = QOP // SLC
        with nc.named_scope("query"), (
                tc.tile_pool(name="qg", bufs=8)) as qgpool, (
                tc.tile_pool(name="qw", bufs=3)) as qwpool, (
                tc.tile_pool(name="psq", bufs=2, space="PSUM")) as psqpool, (
                tc.tile_pool(name="psz", bufs=1, space="PSUM")) as pszpool:
            zps = pszpool.tile([P, nzcol], F32)
            for op_i in range(nqops):
                qt = qgpool.tile([P, 1, 2 * QOP], BF16, tag="qga")
                i0 = op_i * (2 * QOP // 16)
                nc.gpsimd.dma_gather(
                    out_ap=qt[:], in_ap=TQ,
                    idxs_ap=qidxsb[:, i0:i0 + 2 * QOP // 16],
                    num_idxs=2 * QOP, num_idxs_reg=2 * QOP, elem_size=P,
                    transpose=True, queue_num=next_q())
                dd = qwpool.tile([FQ, QOP], BF16, tag="qd")
                nc.vector.tensor_sub(dd[:], qt[0:FQ, 0, 0:QOP],
                                     qt[0:FQ, 0, QOP:2 * QOP])
                sq = qwpool.tile([FQ, QOP], BF16, tag="qsq")
                nc.scalar.activation(sq[:], dd[:], AF.Square)
                for s in range(nslice):
                    hps = psqpool.tile([F2, SLC], F32, tag="qps")
                    nc.tensor.matmul(out=hps[:], lhsT=lin1Wsb[:],
                                     rhs=sq[:, s * SLC:(s + 1) * SLC],
                                     start=True, stop=True)
                    hq = qwpool.tile([F2, SLC], BF16, tag="qhq")
                    if not zb:
                        nc.vector.tensor_tensor(out=hps[:], in0=hps[:],
                                                in1=_bc(lin1bcol[:], SLC),
                                                op=OP.add)
                    tmp = qwpool.tile([F2, SLC], F32, tag="qtmp")
                    nc.vector.tensor_scalar_mul(tmp[:], hps[:], LEAKY_SLOPE)
                    nc.vector.tensor_max(hq[:], hps[:], tmp[:])
                    c0 = op_i * (QOP // P) + s * (SLC // P)
                    for t in range(SLC // P):
                        nc.tensor.matmul(
                            out=zps[:, c0 + t:c0 + t + 1],
                            lhsT=hq[:, t * P:(t + 1) * P],
                            rhs=linWsb[:], start=True, stop=True)

            za = ppool.tile([P, nzcol], F32)
            two = cpool.tile([P, 1], F32)
            nc.vector.memset(two[:], 2.0)
            nc.scalar.activation(za[:], zps[:], AF.Abs, bias=g["lin_b"])
            nc.vector.tensor_scalar_min(za[:], za[:], CLAMP_MAX)
            nc.scalar.activation(za[:], za[:], AF.Sigmoid, bias=two[:],
                                 scale=-1.0)
            nc.sync.dma_start(out=out_d.ap().rearrange("j p -> p j"),
                              in_=za[:])

    nc.compile()
    return nc


# ----------------------------------------------------------------------------
# entry point
# ----------------------------------------------------------------------------

def kernel(**inputs):
    geom, in_maps = build_host(inputs, NCORES)
    nc = build_nc(geom)
    res = bass_utils.run_bass_kernel_spmd(
        nc, in_maps, core_ids=list(range(NCORES)))
    outs = []
    for k in range(NCORES):
        o = np.asarray(res.results[k]["out"], np.float32).reshape(-1)
        lo = k * geom["nqc"]
        hi = min((k + 1) * geom["nqc"], geom["NQ"])
        outs.append(o[: hi - lo])
    return np.concatenate(outs).astype(np.float32)


# revision 22
# speedup vs baseline: 1.8120x; 1.2151x over previous
"""Trainium2 Bass kernel for nn_BlockNet (GNN message passing + block-sim MLP).

Strategy (8 NeuronCores, SPMD, single NEFF):
  - GCN aggregation sharded by destination-node tile ranges (tpc x 128-node
    tiles per core).  Edges sorted by dst on host; segment-sum done as one-hot
    matmuls accumulating in PSUM per dst block.  Self loops kept as edges.
  - conv1 needs no device gather: host stages edge-ordered rows
    xe = (x * dinv)[src]; kernel computes (A @ xe) @ W1 per dst block.
  - conv2 messages fetched with per-dst-block dma_gather ops (2304 idx each,
    256B rows) from the Shared AllGather'd node table; SWDGE queues
    round-robin, deep-buffered so descriptor-gen pipelines with transfers.
  - Degree normalization folded into scalar-engine epilogues (biases are
    zero in this problem -- verified on host, generic fallback otherwise).
  - (x @ emb_sim) @ sim_block @ w_sim collapsed to x @ M3 with
    M3 = emb_sim @ sim_block @ w_sim; the BxB block-sim math is replicated
    per core in bf16.
  - Two Shared-output AllGathers exchange (1) the conv1 row table and
    (2) the final 40-wide node feature table.
  - Query phase: data-parallel over query edges; ONE transposed dma_gather
    per 2048 queries fetches both endpoints (4096 idx); MLP runs on
    512-query slices with biases folded into scalar activations.

kernel(**inputs) takes full unsharded inputs, returns the full [NQ] f32
output.
"""

import math
import os
import sys

import numpy as np

for _p in ("/opt/trn_rl_repo", "/root/.axon_site/_ro/trn_rl_repo"):
    if os.path.isdir(_p) and _p not in sys.path:
        sys.path.insert(0, _p)

import concourse.bass as bass
import concourse.bacc as bacc
import concourse.mybir as mybir
import concourse.tile as tile
from concourse import bass_utils
from concourse.masks import make_identity

BF16 = mybir.dt.bfloat16
F32 = mybir.dt.float32
I16 = mybir.dt.int16
I32 = mybir.dt.int32
NP_BF16 = mybir.dt.np(BF16)

P = 128
NCORES = 8
LEAKY_SLOPE = 0.2
ALPHA, BETA = 1.0, 0.1
CLAMP_MAX = 40.0

AF = mybir.ActivationFunctionType
OP = mybir.AluOpType

NQUEUES = 4       # SWDGE queues; gathers round-robin
# SWDGE descriptor-ring capacity is dynamic_dma_scratch_size/16 per queue;
# a single dma_gather's num_idxs must stay below it.
DMA_SCRATCH = int(os.environ.get("DMA_SCRATCH", "16384"))
RING = DMA_SCRATCH // 16
# queries per combined (i0|i1) transposed gather op (2*QOP idx per op)
QOP = int(os.environ.get("QOP", "256"))
GSZ = RING // P   # conv2 message chunks per gather op


def _bc(ap, n):
    """Append a stride-0 broadcast inner dim of size n to an AP."""
    return bass.AP(ap.tensor, ap.offset, list(ap.ap) + [[0, n]])


# ----------------------------------------------------------------------------
# host-side data prep
# ----------------------------------------------------------------------------

def _wrap16(idx):
    """int16 index array in dma_gather wrapped layout [128, n/16]."""
    idx = np.asarray(idx, np.int64)
    n = idx.shape[0]
    assert n % 16 == 0
    w = idx.reshape(n // 16, 16).T.astype(np.int16)        # [16, n/16]
    return np.ascontiguousarray(np.tile(w, (8, 1)))         # [128, n/16]


def build_host(inputs, ncores=NCORES):
    x = np.asarray(inputs["x"], np.float32)
    L0 = np.asarray(inputs["L0"], np.float32)
    L1 = np.asarray(inputs["L1"], np.float32)
    ei = np.asarray(inputs["edge_index"]).astype(np.int64)
    te = np.asarray(inputs["total_edges"]).astype(np.int64)
    W1 = np.asarray(inputs["conv1_W"], np.float32)
    b1 = np.asarray(inputs["conv1_b"], np.float32)
    W2 = np.asarray(inputs["conv2_W"], np.float32)
    b2 = np.asarray(inputs["conv2_b"], np.float32)
    w_sim = np.asarray(inputs["weights_sim"], np.float32)
    emb_sim = np.asarray(inputs["embeddings_sim"], np.float32)
    w_od = np.asarray(inputs["weights_off_diagonal"], np.float32)
    wL0 = np.asarray(inputs["weights_L_0"], np.float32)
    wL1 = np.asarray(inputs["weights_L_1"], np.float32)
    lin1_W = np.asarray(inputs["lin1_W"], np.float32)
    lin1_b = np.asarray(inputs["lin1_b"], np.float32)
    lin_W = np.asarray(inputs["lin_W"], np.float32)
    lin_b = np.asarray(inputs["lin_b"], np.float32)

    N, F0 = x.shape
    F1 = W1.shape[1]
    F2 = W2.shape[1]
    DIM = w_sim.shape[1]
    B = L0.shape[0]
    NQ = te.shape[0]

    # zero-bias fast path (true for this problem; checked, not assumed)
    zb = (not b1.any()) and (not b2.any()) and (not lin1_b.any())

    tpc = math.ceil(math.ceil(N / P) / ncores)
    ntiles = ncores * tpc
    nslot = ntiles * P

    src, dst = ei[0], ei[1]
    deg = (np.bincount(dst, minlength=N) + 1).astype(np.float32)
    dinv = (1.0 / np.sqrt(deg)).astype(np.float32)
    xs = x * dinv[:, None]                                  # x~ = dinv * x

    loops = np.arange(N, dtype=np.int64)
    src_s = np.concatenate([src, loops])
    dst_s = np.concatenate([dst, loops])
    order = np.argsort(dst_s, kind="stable")
    src_s, dst_s = src_s[order], dst_s[order]

    blk_of = dst_s // P
    counts = np.bincount(blk_of, minlength=ntiles)
    starts = np.concatenate([[0], np.cumsum(counts)])
    cb = []
    for b in range(tpc):
        mx = max(int(counts[k * tpc + b]) for k in range(ncores))
        cb.append(max(1, math.ceil(mx / P)))
    CT = int(sum(cb))
    EPAD = CT * P

    src16_cores, dstloc_cores, xe_cores = [], [], []
    # padding slots gather irrelevant data (their one-hot rows are all zero)
    # but MUST spread across the node table: thousands of same-address
    # gathers serialize in the DMA path (cost core 7 ~100us of skew).
    spread = (np.arange(EPAD, dtype=np.int64) * 97) % N
    for k in range(ncores):
        s_pad = spread.copy()
        d_pad = np.full(EPAD, -1.0, np.float32)
        off = 0
        for b in range(tpc):
            t = k * tpc + b
            e0, e1 = int(starts[t]), int(starts[t + 1])
            cnt = e1 - e0
            s_pad[off : off + cnt] = src_s[e0:e1]
            d_pad[off : off + cnt] = (dst_s[e0:e1] - t * P).astype(np.float32)
            off += cb[b] * P
        src16_cores.append(_wrap16(s_pad))
        dstloc_cores.append(
            np.ascontiguousarray(d_pad.reshape(CT, P).T.astype(NP_BF16)))
        # edge-ordered x~ rows: edge e at [e%128, e//128, :]
        xe = xs[s_pad].reshape(CT, P, F0).transpose(1, 0, 2)
        xe_cores.append(np.ascontiguousarray(xe.astype(NP_BF16)))

    # queries: pad each core's slice to a multiple of QOP; per-op combined
    # index list [i0 (QOP) | i1 (QOP)]
    nqc = math.ceil(NQ / ncores)
    nqcp = math.ceil(nqc / QOP) * QOP
    nqops = nqcp // QOP
    qidx_cores = []
    qspread = (np.arange(nqcp, dtype=np.int64) * 89) % N
    for k in range(ncores):
        q = np.stack([qspread, qspread], axis=1)
        chunk = te[k * nqc : min((k + 1) * nqc, NQ)]
        q[: chunk.shape[0]] = chunk
        per_op = q.reshape(nqops, QOP, 2)
        comb = np.concatenate([per_op[:, :, 0], per_op[:, :, 1]],
                              axis=1).reshape(-1)            # [nqops*2*QOP]
        qidx_cores.append(_wrap16(comb))

    # per-core per-partition 1/sqrt(deg) and 1/deg for own tiles
    dinv_all = np.ones((nslot,), np.float32)
    dinv_all[:N] = dinv
    dinv_all = dinv_all.reshape(ntiles, P).T                 # [P, ntiles]

    shared = {
        "W1": W1.astype(NP_BF16),
        "W2": W2.astype(NP_BF16),
        "L0b": L0.astype(NP_BF16),
        "L0Tb": np.ascontiguousarray(L0.T).astype(NP_BF16),
        "L1b": L1.astype(NP_BF16),
        "L1Tb": np.ascontiguousarray(L1.T).astype(NP_BF16),
        "wodT": np.ascontiguousarray(w_od.T).astype(NP_BF16),
        "wL0": wL0.astype(NP_BF16),
        "wL1": wL1.astype(NP_BF16),
        "embT": np.ascontiguousarray(emb_sim.T).astype(NP_BF16),
        "wsim": w_sim.astype(NP_BF16),
        "lin1Wb": lin1_W.astype(NP_BF16),
        "linWb": lin_W.astype(NP_BF16),
        "lin1bcol": np.ascontiguousarray(lin1_b[:, None]).astype(np.float32),
    }
    if not zb:
        shared["b1bc"] = np.ascontiguousarray(
            np.tile(b1, (P, 1)).astype(np.float32))
        shared["b2bc"] = np.ascontiguousarray(
            np.tile(b2, (P, 1)).astype(np.float32))

    in_maps = []
    for k in range(ncores):
        m = dict(shared)
        m["xe"] = xe_cores[k].reshape(P, CT * F0)
        m["src16"] = src16_cores[k]
        m["dstloc"] = dstloc_cores[k]
        m["qidx16"] = qidx_cores[k]
        dv = dinv_all[:, k * tpc:(k + 1) * tpc]
        m["dinv_own"] = np.ascontiguousarray(dv)
        m["dinv2_own"] = np.ascontiguousarray(dv * dv)
        xo = np.zeros((P, tpc * P), np.float32)
        lo, hi = k * tpc * P, min((k + 1) * tpc * P, N)
        if hi > lo:
            xo[:, : hi - lo] = x[lo:hi].T
        m["xTown"] = xo.astype(NP_BF16)
        in_maps.append(m)

    geom = dict(
        N=N, F0=F0, F1=F1, F2=F2, DIM=DIM, B=B, BT=B // P, SBT=2 * B // P,
        NQ=NQ, tpc=tpc, ntiles=ntiles, nslot=nslot,
        cb=cb, CT=CT, nqc=nqc, nqcp=nqcp, nqops=nqops, ncores=ncores,
        zb=zb, lin_b=float(lin_b[0]),
    )
    return geom, in_maps


# ----------------------------------------------------------------------------
# device kernel
# ----------------------------------------------------------------------------

def build_nc(g):
    tpc, ntiles, nslot = g["tpc"], g["ntiles"], g["nslot"]
    F0, F1, F2, DIM = g["F0"], g["F1"], g["F2"], g["DIM"]
    B, BT, SBT = g["B"], g["BT"], g["SBT"]
    cb, CT = g["cb"], g["CT"]
    nqops = g["nqops"]
    ncores = g["ncores"]
    zb = g["zb"]
    cmax = max(cb)
    rg = [list(range(ncores))]
    FQ = F2 + DIM
    nzcol = nqops * (QOP // P)        # output cols in zps per... per op: QOP/128

    coff = [0]
    for c in cb:
        coff.append(coff[-1] + c)

    nc = bacc.Bacc("TRN2", target_bir_lowering=False, debug=False,
                   num_devices=ncores, num_swdge_queues=NQUEUES,
                   dynamic_dma_scratch_size=DMA_SCRATCH)
    qrr = [0]

    def next_q():
        q = qrr[0] % NQUEUES
        qrr[0] += 1
        return q

    def din(name, shape, dt):
        return nc.dram_tensor(name, shape, dt, kind="ExternalInput")

    xe_d = din("xe", [P, CT * F0], BF16)
    W1_d = din("W1", [F0, F1], BF16)
    W2_d = din("W2", [F1, F2], BF16)
    dinv_own_d = din("dinv_own", [P, tpc], F32)
    dinv2_own_d = din("dinv2_own", [P, tpc], F32)
    xTown_d = din("xTown", [P, tpc * P], BF16)
    src16_d = din("src16", [P, CT * 8], I16)
    dstloc_d = din("dstloc", [P, CT], BF16)
    qidx_d = din("qidx16", [P, nqops * 2 * QOP // 16], I16)
    L0b_d = din("L0b", [B, B], BF16)
    L0Tb_d = din("L0Tb", [B, B], BF16)
    L1b_d = din("L1b", [B, B], BF16)
    L1Tb_d = din("L1Tb", [B, B], BF16)
    wodT_d = din("wodT", [B, B], BF16)
    wL0_d = din("wL0", [B, F2], BF16)
    wL1_d = din("wL1", [B, F2], BF16)
    embT_d = din("embT", [2 * B, F0], BF16)
    wsim_d = din("wsim", [2 * B, DIM], BF16)
    lin1Wb_d = din("lin1Wb", [FQ, F2], BF16)
    linWb_d = din("linWb", [F2, 1], BF16)
    lin1bcol_d = din("lin1bcol", [F2, 1], F32)
    if not zb:
        b1bc_d = din("b1bc", [P, F1], F32)
        b2bc_d = din("b2bc", [P, F2], F32)
    # out is stored partition-major [P, nqcp/P]: the DMA writes contiguous
    # per-partition rows (a "j p" layout fragments into 4-byte descriptors,
    # ~100us); the host transposes for free at unshard time.
    out_d = nc.dram_tensor("out", [P, g["nqcp"] // P], F32,
                           kind="ExternalOutput")

    with tile.TileContext(nc) as tc, (
        tc.tile_pool(name="const", bufs=1)) as cpool, (
        tc.tile_pool(name="persist", bufs=1)) as ppool, (
        tc.tile_pool(name="ps", bufs=2, space="PSUM")) as pspool, (
        tc.tile_pool(name="psagg", bufs=2, space="PSUM")) as psapool, (
        tc.tile_pool(name="dram", bufs=1, space="DRAM")) as dpool:

        # Warm up the collective path first: the first collective of a NEFF
        # pays ~57us of CC cold-start (trigger -> ALGO_MESH_BEGIN); a dummy
        # AllGather absorbs it while conv1 runs.
        with nc.named_scope("agwarm"):
            warm_in = dpool.tile([16, 16], F32)
            warm_out = dpool.tile([ncores, 16, 16], F32)
            warm_sb = cpool.tile([16, 16], F32)
            nc.vector.memset(warm_sb[:], 0.0)
            nc.sync.dma_start(out=warm_in[:], in_=warm_sb[:])
            nc.gpsimd.collective_compute(
                "AllGather", OP.bypass, replica_groups=rg,
                ins=[warm_in.opt()], outs=[warm_out.opt()])

        # ------------------------------------------------- constants / loads
        ident = cpool.tile([P, P], BF16)
        make_identity(nc, ident[:])

        iota_b = cpool.tile([P, cmax * P], BF16)
        with tc.tile_pool(name="iotatmp", bufs=1) as itpool:
            iota_i = itpool.tile([P, cmax * P], I32)
            nc.gpsimd.iota(iota_i[:], pattern=[[0, cmax], [1, P]], base=0,
                           channel_multiplier=0)
            nc.vector.tensor_copy(iota_b[:], iota_i[:])

        def load(pool, dram_t, shape, dt=BF16, rearr=None):
            t = pool.tile(shape, dt, tag="ld_" + dram_t.name)
            src = dram_t.ap()
            if rearr is not None:
                src = src.rearrange(rearr, p=P)
            nc.sync.dma_start(out=t[:], in_=src)
            return t

        W1sb = load(cpool, W1_d, [F0, F1])
        W2sb = load(cpool, W2_d, [F1, F2])
        dinv_own = load(cpool, dinv_own_d, [P, tpc], F32)
        dinv2_own = load(cpool, dinv2_own_d, [P, tpc], F32)
        src16sb = load(cpool, src16_d, [P, CT * 8], I16)
        dstlocsb = load(cpool, dstloc_d, [P, CT])
        qidxsb = load(cpool, qidx_d, [P, nqops * 2 * QOP // 16], I16)
        lin1Wsb = load(cpool, lin1Wb_d, [FQ, F2])
        linWsb = load(cpool, linWb_d, [F2, 1])
        lin1bcol = load(cpool, lin1bcol_d, [F2, 1], F32)
        xTownsb = load(cpool, xTown_d, [P, tpc * P])
        if not zb:
            b1sb = load(cpool, b1bc_d, [P, F1], F32)
            b2sb = load(cpool, b2bc_d, [P, F2], F32)

        ag_shared = os.environ.get("AG_SHARED", "1") == "1"
        ag_space = "Shared" if ag_shared else "Local"
        ag1_in = dpool.tile([tpc * P, P], BF16)
        ag1_out = dpool.tile([ncores, tpc * P, P], BF16, addr_space=ag_space)
        ag2_in = dpool.tile([tpc * P, P], BF16)
        ag2_out = dpool.tile([ncores, tpc * P, P], BF16, addr_space=ag_space)
        T2g = ag1_out[:].rearrange("r n f -> (r n) f")
        TQ = ag2_out[:].rearrange("r n f -> (r n) f")

        g1rows = ppool.tile([P, tpc, P], BF16)
        g2_all = ppool.tile([P, tpc, F2], F32)
        s_all = ppool.tile([P, tpc, DIM], F32)
        Tqown = ppool.tile([P, tpc, P], BF16)
        nc.vector.memset(Tqown[:], 0.0)
        nc.vector.memset(g1rows[:], 0.0)
        M3sb = ppool.tile([F0, DIM], BF16)

        def build_onehot(b, ohpool):
            cbb = cb[b]
            oh = ohpool.tile([P, cmax * P], BF16, tag="oh")
            nc.vector.tensor_tensor(
                out=oh[:, 0:cbb * P].rearrange("p (c e) -> p c e", e=P),
                in0=iota_b[:, 0:cbb * P].rearrange("p (c e) -> p c e", e=P),
                in1=_bc(dstlocsb[:, coff[b]:coff[b] + cbb], P),
                op=OP.is_equal)
            return oh

        # ------------------------------------------------ conv1 (gather-free)
        with nc.named_scope("conv1"), (
                tc.tile_pool(name="msg1", bufs=3)) as msgpool, (
                tc.tile_pool(name="oh1", bufs=3)) as ohpool, (
                tc.tile_pool(name="epi1", bufs=3)) as epipool:
            for b in range(tpc):
                cbb = cb[b]
                mt = msgpool.tile([P, cmax, F0], BF16, tag="msg")
                nc.sync.dma_start(
                    out=mt[:, 0:cbb, :],
                    in_=xe_d.ap().rearrange(
                        "p (c f) -> p c f", f=F0)[:, coff[b]:coff[b] + cbb, :])
                oh = build_onehot(b, ohpool)
                xps = psapool.tile([P, P], F32, tag="agg")
                for ci in range(cbb):
                    nc.tensor.matmul(
                        out=xps[:], lhsT=mt[:, ci, :],
                        rhs=oh[:, ci * P:(ci + 1) * P],
                        start=(ci == 0), stop=(ci == cbb - 1))
                xaggt = epipool.tile([P, P], BF16, tag="xaggt")
                nc.vector.tensor_copy(xaggt[:], xps[:])
                hps = pspool.tile([P, F1], F32, tag="ps")
                nc.tensor.matmul(out=hps[:], lhsT=xaggt[:], rhs=W1sb[:],
                                 start=True, stop=True)
                # g1s = dinv*relu(dinv*agg + b1); zero-bias: dinv2*relu(agg)
                if zb:
                    nc.scalar.activation(g1rows[:, b, 0:F1], hps[:], AF.Relu,
                                         scale=dinv2_own[:, b:b + 1])
                else:
                    ta = epipool.tile([P, F1], F32, tag="epi1a")
                    nc.scalar.activation(ta[:], hps[:], AF.Copy,
                                         scale=dinv_own[:, b:b + 1])
                    nc.vector.tensor_add(ta[:], ta[:], b1sb[:])
                    nc.vector.tensor_relu(ta[:], ta[:])
                    nc.scalar.activation(g1rows[:, b, 0:F1], ta[:], AF.Copy,
                                         scale=dinv_own[:, b:b + 1])
                # stage this block's rows for the AllGather right away so the
                # collective can trigger as soon as the last block lands
                nc.sync.dma_start(
                    out=ag1_in[:].rearrange(
                        "(t p) f -> p t f", p=P)[:, b:b + 1, :],
                    in_=g1rows[:, b:b + 1, :])

        with nc.named_scope("ag1"):
            nc.gpsimd.collective_compute(
                "AllGather", OP.bypass, replica_groups=rg,
                ins=[ag1_in.opt()], outs=[ag1_out.opt()])

        # conv2 message/onehot pools are opened BEFORE the sim pools so their
        # SBUF ranges don't overlap sim tiles -- otherwise the conv2 gathers
        # stall until the last sim-tile read vacates the space (~35us).
        conv2_pools = (
            tc.tile_pool(name="msg2", bufs=10),
            tc.tile_pool(name="oh2", bufs=3),
            tc.tile_pool(name="epi2", bufs=3),
        )
        msg2pool = conv2_pools[0].__enter__()
        oh2pool = conv2_pools[1].__enter__()
        epi2pool = conv2_pools[2].__enter__()

        # ------------------------------------------------- sim block (bf16)
        with nc.named_scope("sim"), (
                tc.tile_pool(name="sim", bufs=1)) as spool, (
                tc.tile_pool(name="simw", bufs=1)) as swpool, (
                tc.tile_pool(name="pssim", bufs=2, space="PSUM")) as psbpool:
            L0sb = load(spool, L0b_d, [P, BT, B], rearr="(t p) c -> p t c")
            L0Tsb = load(spool, L0Tb_d, [P, BT, B], rearr="(t p) c -> p t c")
            L1sb = load(spool, L1b_d, [P, BT, B], rearr="(t p) c -> p t c")
            L1Tsb = load(spool, L1Tb_d, [P, BT, B], rearr="(t p) c -> p t c")
            wodTsb = load(spool, wodT_d, [P, BT, B], rearr="(t p) c -> p t c")
            wL0sb = load(spool, wL0_d, [P, BT, F2], rearr="(t p) c -> p t c")
            wL1sb = load(spool, wL1_d, [P, BT, F2], rearr="(t p) c -> p t c")
            embTsb = load(spool, embT_d, [P, SBT, F0],
                          rearr="(t p) c -> p t c")
            wsimsb = load(spool, wsim_d, [P, SBT, DIM],
                          rearr="(t p) c -> p t c")

            def mm_accum(out_ap, pairs):
                for i, (lhsT, rhs) in enumerate(pairs):
                    nc.tensor.matmul(out=out_ap, lhsT=lhsT, rhs=rhs,
                                     start=(i == 0),
                                     stop=(i == len(pairs) - 1))

            def big_mm(dst_sb, lhsT_tiles, rhs_tiles, nf):
                for m in range(BT):
                    ps = psbpool.tile([P, nf], F32, tag="simps")
                    mm_accum(ps[:], [(lhsT_tiles(k, m), rhs_tiles(k))
                                     for k in range(BT)])
                    nc.vector.tensor_copy(dst_sb[:, m, :], ps[:])

            L0r = spool.tile([P, BT, B], BF16)
            L0rT = spool.tile([P, BT, B], BF16)
            L1r = spool.tile([P, BT, B], BF16)
            L1rT = spool.tile([P, BT, B], BF16)
            big_mm(L0r, lambda k, m: L0Tsb[:, k, m * P:(m + 1) * P],
                   lambda k: L0sb[:, k, :], B)
            big_mm(L0rT, lambda k, m: L0sb[:, k, m * P:(m + 1) * P],
                   lambda k: L0Tsb[:, k, :], B)
            big_mm(L1r, lambda k, m: L1Tsb[:, k, m * P:(m + 1) * P],
                   lambda k: L1sb[:, k, :], B)
            big_mm(L1rT, lambda k, m: L1sb[:, k, m * P:(m + 1) * P],
                   lambda k: L1Tsb[:, k, :], B)

            P0 = swpool.tile([P, BT, F2], BF16)
            P1 = swpool.tile([P, BT, F2], BF16)
            Qm = swpool.tile([P, BT, F2], BF16)
            big_mm(P0, lambda k, m: L0rT[:, k, m * P:(m + 1) * P],
                   lambda k: wL0sb[:, k, :], F2)
            big_mm(P1, lambda k, m: L1rT[:, k, m * P:(m + 1) * P],
                   lambda k: wL1sb[:, k, :], F2)
            big_mm(Qm, lambda k, m: wodTsb[:, k, m * P:(m + 1) * P],
                   lambda k: P0[:, k, :], F2)

            def transp_small(src_sb, tg):
                dst = swpool.tile([F2, BT, P], BF16, tag=tg)
                for m in range(BT):
                    pt = pspool.tile([P, P], BF16, tag="ps")
                    nc.tensor.transpose(out=pt[0:F2, :], in_=src_sb[:, m, :],
                                        identity=ident[:])
                    nc.vector.tensor_copy(dst[:, m, :], pt[0:F2, :])
                return dst

            Qt = transp_small(Qm, "Qt")
            P1t = transp_small(P1, "P1t")
            relm = spool.tile([P, BT, B], BF16)
            relT = spool.tile([P, BT, B], BF16)
            for m in range(BT):
                ps = psbpool.tile([P, B], F32, tag="simps")
                nc.tensor.matmul(out=ps[:], lhsT=Qt[:, m, :],
                                 rhs=P1t[:].rearrange("p t c -> p (t c)"),
                                 start=True, stop=True)
                nc.vector.tensor_copy(relm[:, m, :], ps[:])
                ps2 = psbpool.tile([P, B], F32, tag="simps")
                nc.tensor.matmul(out=ps2[:], lhsT=P1t[:, m, :],
                                 rhs=Qt[:].rearrange("p t c -> p (t c)"),
                                 start=True, stop=True)
                nc.vector.tensor_copy(relT[:, m, :], ps2[:])

            # softmax(relu(x)): E = max(1, exp(x)); 1/rowsum scales embT cols
            Esb = spool.tile([P, SBT, 2 * B], BF16)
            Ssum = swpool.tile([P, SBT, 2], F32)
            for rt in range(SBT):
                if rt < BT:
                    left, right = L0r[:, rt, :], relm[:, rt, :]
                else:
                    left, right = relT[:, rt - BT, :], L1r[:, rt - BT, :]
                nc.scalar.activation(Esb[:, rt, 0:B], left, AF.Exp)
                nc.scalar.activation(Esb[:, rt, B:2 * B], right, AF.Exp)
                nc.vector.tensor_scalar(
                    out=Esb[:, rt, 0:B], in0=Esb[:, rt, 0:B], scalar1=1.0,
                    scalar2=None, op0=OP.max, op1=OP.add,
                    accum_out=Ssum[:, rt, 0:1])
                nc.vector.tensor_scalar(
                    out=Esb[:, rt, B:2 * B], in0=Esb[:, rt, B:2 * B],
                    scalar1=1.0, scalar2=None, op0=OP.max, op1=OP.add,
                    accum_out=Ssum[:, rt, 1:2])
            rsc = swpool.tile([P, SBT], F32)
            nc.vector.reduce_sum(rsc[:], Ssum[:], axis=mybir.AxisListType.X)
            nc.vector.reciprocal(rsc[:], rsc[:])

            embS = swpool.tile([P, SBT, F0], BF16)
            for kt in range(SBT):
                nc.scalar.activation(embS[:, kt, :], embTsb[:, kt, :],
                                     AF.Copy, scale=rsc[:, kt:kt + 1])

            with tc.tile_pool(name="pst", bufs=1, space="PSUM") as pstpool:
                Tps = pstpool.tile([P, 2 * B], F32, tag="Tps")
                for half in range(2):
                    mm_accum(Tps[:, half * B:(half + 1) * B],
                             [(embS[:, kt, :],
                               Esb[:, kt, half * B:(half + 1) * B])
                              for kt in range(SBT)])
                Tsb = swpool.tile([P, 2 * B], BF16)
                nc.vector.tensor_copy(Tsb[:], Tps[:])
            Tt = swpool.tile([P, SBT, P], BF16)
            for j in range(SBT):
                pt = pspool.tile([P, P], BF16, tag="ps")
                nc.tensor.transpose(out=pt[0:F0, :],
                                    in_=Tsb[:, j * P:(j + 1) * P],
                                    identity=ident[:])
                nc.vector.tensor_copy(Tt[:, j, :], pt[0:F0, :])
            M3ps = pspool.tile([P, DIM], F32, tag="ps")
            mm_accum(M3ps[:], [(Tt[:, kt, :], wsimsb[:, kt, :])
                               for kt in range(SBT)])
            nc.vector.tensor_copy(M3sb[:], M3ps[0:F0, :])

        # s_emb (own rows): accumulate all tiles into one PSUM, one copy out
        with tc.tile_pool(name="pse", bufs=1, space="PSUM") as psepool:
            sps = psepool.tile([P, tpc * DIM], F32, tag="sps")
            for j in range(tpc):
                nc.tensor.matmul(out=sps[:, j * DIM:(j + 1) * DIM],
                                 lhsT=xTownsb[:, j * P:(j + 1) * P],
                                 rhs=M3sb[:], start=True, stop=True)
            nc.vector.tensor_copy(
                s_all[:].rearrange("p t d -> p (t d)"), sps[:])

        with tc.tile_pool(name="rn", bufs=1) as rnpool:
            def renorm_write(src_all, fdim, col0, post_scale):
                sq = rnpool.tile([P, tpc, fdim], F32, tag=f"rn{col0}")
                nc.vector.tensor_mul(sq[:], src_all[:], src_all[:])
                s2 = rnpool.tile([P, tpc], F32, tag=f"rns{col0}")
                nc.vector.reduce_sum(s2[:], sq[:], axis=mybir.AxisListType.X)
                nc.scalar.activation(s2[:], s2[:], AF.Sqrt)
                nc.vector.tensor_scalar_add(s2[:], s2[:], 1e-7)
                nc.vector.reciprocal(s2[:], s2[:])
                if post_scale != 1.0:
                    nc.vector.tensor_scalar(
                        out=s2[:], in0=s2[:], scalar1=post_scale,
                        scalar2=post_scale, op0=OP.mult, op1=OP.min)
                else:
                    nc.vector.tensor_scalar_min(s2[:], s2[:], 1.0)
                nc.vector.tensor_tensor(
                    out=Tqown[:, :, col0:col0 + fdim], in0=src_all[:],
                    in1=_bc(s2[:], fdim), op=OP.mult)

            renorm_write(s_all, DIM, F2, math.sqrt(BETA))

            # --------------------------------------------- conv2 agg
            # gather ops are capped at RING idx; groups of GSZ chunks,
            # deep-buffered so desc-gen pipelines with transfers/compute
            with nc.named_scope("conv2"):
                g2tiles = {}

                def fetch2(c0, gc):
                    mt = msg2pool.tile([P, GSZ, P], BF16, tag="msg")
                    nc.gpsimd.dma_gather(
                        out_ap=mt[:, 0:gc, :], in_ap=T2g,
                        idxs_ap=src16sb[:, c0 * 8:(c0 + gc) * 8],
                        num_idxs=gc * P, num_idxs_reg=gc * P, elem_size=P,
                        queue_num=next_q())
                    g2tiles[c0] = (gc, mt)

                for c0 in range(0, CT, GSZ):
                    fetch2(c0, min(GSZ, CT - c0))

                def chunk2(c):
                    c0 = c // GSZ * GSZ
                    gc, mt = g2tiles[c0]
                    return mt[:, c - c0, 0:F1]

                for b in range(tpc):
                    cbb = cb[b]
                    oh = build_onehot(b, oh2pool)
                    # agg.T directly: lhsT=msg rows, rhs=onehot
                    aps = psapool.tile([F1, P], F32, tag="agg")
                    for ci in range(cbb):
                        c = coff[b] + ci
                        nc.tensor.matmul(
                            out=aps[:], lhsT=chunk2(c),
                            rhs=oh[:, ci * P:(ci + 1) * P],
                            start=(ci == 0), stop=(ci == cbb - 1))
                    a2t = epi2pool.tile([F1, P], BF16, tag="e2a")
                    nc.vector.tensor_copy(a2t[:], aps[:])
                    hps = pspool.tile([P, F2], F32, tag="ps")
                    nc.tensor.matmul(out=hps[:], lhsT=a2t[:], rhs=W2sb[:],
                                     start=True, stop=True)
                    # g2 = relu(dinv*agg2 + b2); zero-bias: relu(dinv*agg2)
                    if zb:
                        nc.scalar.activation(g2_all[:, b, :], hps[:], AF.Relu,
                                             scale=dinv_own[:, b:b + 1])
                    else:
                        nc.scalar.activation(g2_all[:, b, :], hps[:], AF.Copy,
                                             scale=dinv_own[:, b:b + 1])
                        nc.vector.tensor_add(g2_all[:, b, :], g2_all[:, b, :],
                                             b2sb[:])
                        nc.vector.tensor_relu(g2_all[:, b, :],
                                              g2_all[:, b, :])

            renorm_write(g2_all, F2, 0, math.sqrt(ALPHA))

        for cp in reversed(conv2_pools):
            cp.__exit__(None, None, None)

        with nc.named_scope("ag2"):
            nc.sync.dma_start(
                out=ag2_in[:].rearrange("(t p) f -> p t f", p=P),
                in_=Tqown[:])
            nc.gpsimd.collective_compute(
                "AllGather", OP.bypass, replica_groups=rg,
                ins=[ag2_in.opt()], outs=[ag2_out.opt()])

        # ------------------------------------------------- query phase
        SLC = min(512, QOP)
        nslice = QOP // SLC
        with nc.named_scope("query"), (
                tc.tile_pool(name="qg", bufs=8)) as qgpool, (
                tc.tile_pool(name="qw", bufs=3)) as qwpool, (
                tc.tile_pool(name="psq", bufs=2, space="PSUM")) as psqpool, (
                tc.tile_pool(name="psz", bufs=1, space="PSUM")) as pszpool:
            zps = pszpool.tile([P, nzcol], F32)
            for op_i in range(nqops):
                qt = qgpool.tile([P, 1, 2 * QOP], BF16, tag="qga")
                i0 = op_i * (2 * QOP // 16)
                nc.gpsimd.dma_gather(
                    out_ap=qt[:], in_ap=TQ,
                    idxs_ap=qidxsb[:, i0:i0 + 2 * QOP // 16],
                    num_idxs=2 * QOP, num_idxs_reg=2 * QOP, elem_size=P,
                    transpose=True, queue_num=next_q())
                dd = qwpool.tile([FQ, QOP], BF16, tag="qd")
                nc.vector.tensor_sub(dd[:], qt[0:FQ, 0, 0:QOP],
                                     qt[0:FQ, 0, QOP:2 * QOP])
                sq = qwpool.tile([FQ, QOP], BF16, tag="qsq")
                nc.scalar.activation(sq[:], dd[:], AF.Square)
                for s in range(nslice):
                    hps = psqpool.tile([F2, SLC], F32, tag="qps")
                    nc.tensor.matmul(out=hps[:], lhsT=lin1Wsb[:],
                                     rhs=sq[:, s * SLC:(s + 1) * SLC],
                                     start=True, stop=True)
                    hq = qwpool.tile([F2, SLC], BF16, tag="qhq")
                    if not zb:
                        nc.vector.tensor_tensor(out=hps[:], in0=hps[:],
                                                in1=_bc(lin1bcol[:], SLC),
                                                op=OP.add)
                    tmp = qwpool.tile([F2, SLC], F32, tag="qtmp")
                    nc.vector.tensor_scalar_mul(tmp[:], hps[:], LEAKY_SLOPE)
                    nc.vector.tensor_max(hq[:], hps[:], tmp[:])
                    c0 = op_i * (QOP // P) + s * (SLC // P)
                    for t in range(SLC // P):
                        nc.tensor.matmul(
                            out=zps[:, c0 + t:c0 + t + 1],
                            lhsT=hq[:, t * P:(t + 1) * P],
                            rhs=linWsb[:], start=True, stop=True)

            za = ppool.tile([P, nzcol], F32)
            two = cpool.tile([P, 1], F32)
            nc.vector.memset(two[:], 2.0)
            nc.scalar.activation(za[:], zps[:], AF.Abs, bias=g["lin_b"])
            nc.vector.tensor_scalar_min(za[:], za[:], CLAMP_MAX)
            nc.scalar.activation(za[:], za[:], AF.Sigmoid, bias=two[:],
                                 scale=-1.0)
            nc.sync.dma_start(out=out_d.ap(), in_=za[:])

    nc.compile()
    return nc


# ----------------------------------------------------------------------------
# entry point
# ----------------------------------------------------------------------------

def kernel(**inputs):
    geom, in_maps = build_host(inputs, NCORES)
    nc = build_nc(geom)
    res = bass_utils.run_bass_kernel_spmd(
        nc, in_maps, core_ids=list(range(NCORES)))
    outs = []
    for k in range(NCORES):
        o = np.asarray(res.results[k]["out"], np.float32).T.reshape(-1)
        lo = k * geom["nqc"]
        hi = min((k + 1) * geom["nqc"], geom["NQ"])
        outs.append(o[: hi - lo])
    return np.concatenate(outs).astype(np.float32)
